# revision 1
# baseline (speedup 1.0000x reference)
"""Causal multi-head attention on 8 Trainium2 NeuronCores.

Sharding: core c -> (batch g = c // 4, head-group p = c % 4, heads 4p..4p+3).
Each core projects Q/K/V for its batch with its 256 feature columns
(column-sharded w_q/w_k/w_v), runs causal attention for its 4 heads in
transposed (scores.T) layout with an augmented-ones column on V to get the
softmax denominators for free, computes the partial output projection with
its 256 rows of w_o, and a ReduceScatter over each batch group sums the
partials and hands every core its own 512-row output shard.

Matmuls run as float32r (full PE rate, ~1.5e-4 rel err); accumulation fp32.
"""

import numpy as np

B, S, D, H = 2, 2048, 1024, 16
DK = D // H  # 64
N_CORES = 8
FPC = 256  # features per core

_CACHE = {}


def _build_nc():
    import os as os_mod
    import concourse.mybir as mybir
    import concourse.tile as tile
    from concourse import bacc

    F32 = mybir.dt.float32
    F32R = mybir.dt.float32r
    BF16 = mybir.dt.bfloat16
    x_bf16 = bool(os_mod.environ.get("BASS_X_BF16"))
    XD = BF16 if x_bf16 else F32
    XDR = BF16 if x_bf16 else F32R
    Exp = mybir.ActivationFunctionType.Exp

    nc = bacc.Bacc("TRN2", target_bir_lowering=False, debug=False, num_devices=8)

    xq = nc.dram_tensor("xq", [D, S], XD, kind="ExternalInput")
    xk = nc.dram_tensor("xk", [D, S], XD, kind="ExternalInput")
    xv = nc.dram_tensor("xv", [D, S], XD, kind="ExternalInput")
    wq = nc.dram_tensor("wq", [D, FPC], XD, kind="ExternalInput")
    wk = nc.dram_tensor("wk", [D, FPC], XD, kind="ExternalInput")
    wv = nc.dram_tensor("wv", [D, FPC], XD, kind="ExternalInput")
    wo = nc.dram_tensor("wo", [FPC, D], F32, kind="ExternalInput")
    bq = nc.dram_tensor("bq", [FPC, 1], F32, kind="ExternalInput")
    bk = nc.dram_tensor("bk", [FPC, 1], F32, kind="ExternalInput")
    bv = nc.dram_tensor("bv", [FPC, 1], F32, kind="ExternalInput")
    bo4 = nc.dram_tensor("bo4", [128, D], F32, kind="ExternalInput")
    masks = nc.dram_tensor("masks", [128, 2048], F32, kind="ExternalInput")
    ident = nc.dram_tensor("ident", [128, 128], F32, kind="ExternalInput")
    out = nc.dram_tensor("out", [512, D], F32, kind="ExternalOutput")

    NKT = S // 128  # 16 kpos tiles
    NQB = S // 512  # 4 q blocks

    from contextlib import ExitStack
    stack = ExitStack()
    with tile.TileContext(nc) as tc:
        with (
            tc.tile_pool(name="consts", bufs=1) as consts,
            tc.tile_pool(name="persist", bufs=1) as persist,
            tc.tile_pool(name="xin", bufs=3) as xin,
            tc.tile_pool(name="probs", bufs=4) as probs,
            tc.tile_pool(name="small", bufs=2) as small,
            tc.tile_pool(name="oout", bufs=3) as oout,
            tc.tile_pool(name="dram", bufs=1, space="DRAM") as dram,
        ):
            # ---- constants; only wq chunk0 + first x chunk gate startup ----
            wq_s = consts.tile([128, 8, FPC], XDR, tag="wq")
            wk_s = consts.tile([128, 8, FPC], XDR, tag="wk")
            wv_s = consts.tile([128, 8, FPC], XDR, tag="wv")
            wo_s = consts.tile([128, 2, D], F32R, tag="wo")
            masks_s = consts.tile([128, 2048], F32R, tag="masks")
            ident_s = consts.tile([128, 128], F32, tag="ident")
            bq_s = consts.tile([128, 2], F32, tag="bq")
            bk_s = consts.tile([128, 2], F32, tag="bk")
            bv_s = consts.tile([128, 2], F32, tag="bv")
            bo4_s = consts.tile([128, D], F32, tag="bo4")

            nc.sync.dma_start(wq_s[:, 0, :], wq[0:128, :].bitcast(XDR))
            xt0 = xin.tile([128, S], XDR, tag="x", name="xt0")
            for qb0 in range(4):
                nc.sync.dma_start(
                    xt0[:, 512 * qb0 : 512 * (qb0 + 1)],
                    xq[0:128, 512 * qb0 : 512 * (qb0 + 1)].bitcast(XDR))
            for kc in range(1, 8):
                nc.sync.dma_start(
                    wq_s[:, kc, :], wq[128 * kc : 128 * (kc + 1), :].bitcast(XDR))
            nc.sync.dma_start(bq_s[:], bq.ap().rearrange("(t p) o -> p (t o)", p=128))

            # ---- persistent activations ----
            qT_s = [persist.tile([128, S], F32R, tag=f"qT{i}", name=f"qT{i}") for i in range(2)]
            kT_s = [persist.tile([128, S], F32R, tag=f"kT{i}", name=f"kT{i}") for i in range(2)]
            v_s = persist.tile([128, NKT, 4 * 65], F32R, tag="v")
            ctx_s = [persist.tile([128, S], F32R, tag=f"ctx{i}", name=f"ctx{i}") for i in range(2)]

            # ones columns of V_aug (col 64 of each head's 65-wide strip)
            for h in range(4):
                nc.vector.memset(v_s[:, :, 65 * h + 64 : 65 * h + 65].bitcast(F32), 1.0)

            # ---- phase 1: projections (shared PSUM pool, slots recycle) ----
            with (
                tc.tile_pool(name="psProj", bufs=8, space="PSUM") as psP,
                tc.tile_pool(name="vtp", bufs=1) as vtp,
            ):
                # Q pass: psum [2pt x 4qb] accumulate over 8 k-chunks
                psq = {(pt, qb): psP.tile([128, 512], F32, tag="pp", name=f"psq{pt}{qb}")
                       for pt in range(2) for qb in range(NQB)}
                dma_eng = [nc.sync, nc.scalar, nc.gpsimd]
                for kc in range(8):
                    if kc == 0:
                        xt = xt0
                    else:
                        xt = xin.tile([128, S], XDR, tag="x")
                        dma_eng[kc % 3].dma_start(xt[:], xq[128 * kc : 128 * (kc + 1), :].bitcast(XDR))
                    for pt in range(2):
                        for qb in range(NQB):
                            nc.tensor.matmul(
                                psq[(pt, qb)][:],
                                wq_s[:, kc, 128 * pt : 128 * (pt + 1)],
                                xt[:, 512 * qb : 512 * (qb + 1)],
                                start=(kc == 0), stop=(kc == 7),
                            )
                for pt in range(2):
                    for qb in range(NQB):
                        nc.vector.tensor_scalar_add(
                            qT_s[pt][:, 512 * qb : 512 * (qb + 1)],
                            psq[(pt, qb)][:], bq_s[:, pt : pt + 1],
                        )
                # K pass
                nc.sync.dma_start(wk_s[:], wk.ap().rearrange("(kc p) f -> p kc f", p=128).bitcast(XDR))
                nc.sync.dma_start(bk_s[:], bk.ap().rearrange("(t p) o -> p (t o)", p=128))
                psk = {(pt, qb): psP.tile([128, 512], F32, tag="pp", name=f"psk{pt}{qb}")
                       for pt in range(2) for qb in range(NQB)}
                for kc in range(8):
                    xt = xin.tile([128, S], XDR, tag="x")
                    dma_eng[kc % 3].dma_start(xt[:], xk[128 * kc : 128 * (kc + 1), :].bitcast(XDR))
                    for pt in range(2):
                        for qb in range(NQB):
                            nc.tensor.matmul(
                                psk[(pt, qb)][:],
                                wk_s[:, kc, 128 * pt : 128 * (pt + 1)],
                                xt[:, 512 * qb : 512 * (qb + 1)],
                                start=(kc == 0), stop=(kc == 7),
                            )
                for pt in range(2):
                    for qb in range(NQB):
                        nc.vector.tensor_scalar_add(
                            kT_s[pt][:, 512 * qb : 512 * (qb + 1)],
                            psk[(pt, qb)][:], bk_s[:, pt : pt + 1],
                        )
                # V pass: compute V.T like Q/K, then PE-transpose to natural
                nc.sync.dma_start(wv_s[:], wv.ap().rearrange("(kc p) f -> p kc f", p=128).bitcast(XDR))
                nc.sync.dma_start(bv_s[:], bv.ap().rearrange("(t p) o -> p (t o)", p=128))
                nc.sync.dma_start(masks_s[:], masks.ap().bitcast(F32R))
                nc.sync.dma_start(ident_s[:], ident.ap())
                vT_s = [vtp.tile([128, S], F32, tag=f"vT{i}", name=f"vT{i}") for i in range(2)]
                psv = {(pt, kb): psP.tile([128, 512], F32, tag="pp", name=f"psv{pt}{kb}")
                       for pt in range(2) for kb in range(NQB)}
                for kc in range(8):
                    xt = xin.tile([128, S], XDR, tag="x")
                    dma_eng[kc % 3].dma_start(xt[:], xv[128 * kc : 128 * (kc + 1), :].bitcast(XDR))
                    for pt in range(2):
                        for kb in range(NQB):
                            nc.tensor.matmul(
                                psv[(pt, kb)][:],
                                wv_s[:, kc, 128 * pt : 128 * (pt + 1)],
                                xt[:, 512 * kb : 512 * (kb + 1)],
                                start=(kc == 0), stop=(kc == 7),
                            )
                for pt in range(2):
                    for kb in range(NQB):
                        nc.vector.tensor_scalar_add(
                            vT_s[pt][:, 512 * kb : 512 * (kb + 1)],
                            psv[(pt, kb)][:], bv_s[:, pt : pt + 1],
                        )
                # PE transposes: 2 s-tiles (= 4 [128,128] blocks) per PSUM bank
                for sp in range(8):
                    pst = psP.tile([128, 512], F32, tag="pp", name=f"pst{sp}")
                    blk = 0
                    for k2 in range(2):
                        st = 2 * sp + k2
                        for pt in range(2):
                            nc.tensor.matmul(
                                pst[:, 256 * k2 + 128 * pt : 256 * k2 + 128 * pt + 128],
                                vT_s[pt][:, 128 * st : 128 * (st + 1)],
                                ident_s[:],
                                is_transpose=True,
                                start=(blk == 0), stop=(blk == 3),
                                skip_group_check=True,
                            )
                            blk += 1
                    for k2 in range(2):
                        st = 2 * sp + k2
                        dst = v_s[:, st, :].rearrange("p (h x) -> p h x", x=65)[:, :, 0:64]
                        nc.vector.tensor_copy(
                            dst,
                            pst[:, 256 * k2 : 256 * k2 + 256].rearrange("p (h x) -> p h x", x=64),
                        )

            # ---- output projection halves + split ReduceScatter ----
            rs_in = [dram.tile([S // 2, D], F32, name=f"rs_in{i}") for i in range(2)]
            rs_out = [dram.tile([256, D], F32, name=f"rs_out{i}") for i in range(2)]

            def emit_oproj_half(half, psO):
                # q rows [1024*half, 1024*half+1024) = ctx_s cols of qb-pair `half`
                for sl in range(8):
                    st = 8 * half + sl
                    po = psO.tile([128, 2, 512], F32, tag="po", bufs=4, name="po")
                    for nb in range(2):
                        for fc in range(2):
                            nc.tensor.matmul(
                                po[:, nb, :],
                                ctx_s[fc][:, 128 * st : 128 * (st + 1)],
                                wo_s[:, fc, 512 * nb : 512 * (nb + 1)],
                                start=(fc == 0), stop=(fc == 1),
                            )
                    ot = oout.tile([128, D], F32, tag="ot")
                    nc.vector.tensor_add(
                        ot[:].rearrange("p (n x) -> p n x", n=2), po[:], 
                        bo4_s[:].rearrange("p (n x) -> p n x", n=2))
                    nc.sync.dma_start(rs_in[half][128 * sl : 128 * (sl + 1), :], ot[:])
                if not os_mod.environ.get("BASS_SIM_NO_RS"):
                    import concourse.mybir as mybir_mod
                    nc.gpsimd.collective_compute(
                        "ReduceScatter", mybir_mod.AluOpType.add,
                        replica_groups=[[0, 1, 2, 3], [4, 5, 6, 7]],
                        ins=[rs_in[half].opt()], outs=[rs_out[half].opt()],
                    )
                    nc.sync.dma_start(
                        out[256 * half : 256 * (half + 1), :], rs_out[half][:])
                else:
                    nc.sync.dma_start(
                        out[256 * half : 256 * (half + 1), :],
                        rs_in[half][0:256, :])

            # ---- phase 2: attention, two (head, qb-pair) streams in flight ----
            first = True
            with tc.tile_pool(name="rbcp", bufs=2) as rbcp:
                for qbp in range(2):
                    psS = stack.enter_context(
                        tc.tile_pool(name=f"psS{qbp}", bufs=1, space="PSUM"))
                    psA = stack.enter_context(
                        tc.tile_pool(name=f"psA{qbp}", bufs=1, space="PSUM"))
                    nkt = 8 * qbp + 8  # k-tiles needed by this qb pair
                    for hp in range(2):
                        heads = (2 * hp, 2 * hp + 1)
                        ctx_ps = {
                            si: psA.tile([65, 1024], F32, tag="ctx", bufs=2, name=f"ctxps{si}")
                            for si in range(2)
                        }
                        for ki in range(nkt):
                            qbs = [qb for qb in (2 * qbp, 2 * qbp + 1) if qb >= ki // 4]
                            w = 512 * len(qbs)
                            for si, h in enumerate(heads):
                                pt, row = h // 2, 64 * (h % 2)
                                qT_h = qT_s[pt][row : row + 64, :]
                                kT_h = kT_s[pt][row : row + 64, :]
                                sc = psS.tile([128, 1024], F32, tag="sc", bufs=2, name=f"scps{si}")
                                for j, qb in enumerate(qbs):
                                    nc.tensor.matmul(
                                        sc[:, 512 * j : 512 * j + 512],
                                        kT_h[:, 128 * ki : 128 * (ki + 1)],
                                        qT_h[:, 512 * qb : 512 * (qb + 1)],
                                        start=True, stop=True,
                                    )
                                pr = probs.tile([128, 1024], F32R, tag="pr", name="pr")
                                nc.scalar.activation(
                                    out=pr[:, :w], in_=sc[:, :w], func=Exp, scale=0.125
                                )
                                prm = None
                                if qbs and qbs[0] == ki // 4:  # diagonal block present
                                    prm = probs.tile([128, 512], F32R, tag="prm", bufs=3, name="prm")
                                    nc.vector.tensor_mul(
                                        prm[:], pr[:, 0:512],
                                        masks_s[:, 512 * (ki % 4) : 512 * (ki % 4) + 512],
                                    )
                                for j, qb in enumerate(qbs):
                                    rhs = prm[:] if (j == 0 and prm is not None) else pr[:, 512 * j : 512 * j + 512]
                                    nc.tensor.matmul(
                                        ctx_ps[si][:, 512 * (qb - 2 * qbp) : 512 * (qb - 2 * qbp) + 512],
                                        v_s[:, ki, 65 * h : 65 * h + 65],
                                        rhs,
                                        start=(ki == 0), stop=(ki == 4 * qb + 3),
                                    )
                        if first:
                            # prefetch phase-3 constants during attention
                            nc.sync.dma_start(wo_s[:], wo.ap().rearrange("(c p) d -> p c d", p=128).bitcast(F32R))
                            nc.sync.dma_start(bo4_s[:], bo4.ap())
                            first = False
                        for si, h in enumerate(heads):
                            pt, row = h // 2, 64 * (h % 2)
                            # copy ctx psum out (frees banks), normalize off-PSUM
                            ctmp = rbcp.tile([65, 1024], F32, tag="ctmp", name="ctmp", bufs=2)
                            nc.vector.tensor_copy(ctmp[:], ctx_ps[si][:])
                            recip = small.tile([1, 1024], F32, tag="recip")
                            nc.vector.reciprocal(recip[:], ctmp[64:65, :])
                            rbc = rbcp.tile([64, 1024], F32, tag="rbc", bufs=2)
                            nc.gpsimd.partition_broadcast(rbc[:], recip[:])
                            nc.vector.tensor_mul(
                                ctx_s[pt][row : row + 64, 1024 * qbp : 1024 * (qbp + 1)],
                                ctmp[0:64, :], rbc[:],
                            )
                    if hp == 1:
                        stack.pop_all().close()  # close psS/psA for this qbp
                        with tc.tile_pool(name=f"psO{qbp}", bufs=4, space="PSUM") as psO:
                            emit_oproj_half(qbp, psO)


    nc.compile()
    return nc


def _prep_inputs(query, key_, value, w_q, b_q, w_k, b_k, w_v, b_v, w_o, b_o):
    """Build the 8 per-core input maps (host-side sharding / re-layout)."""
    f32 = np.float32
    # triangular mask patterns: t in 0..3, allowed iff j >= r + 128*t
    r = np.arange(128)[:, None]
    j = np.arange(512)[None, :]
    masks = np.concatenate(
        [(j >= r + 128 * t).astype(f32) for t in range(4)], axis=1
    )  # [128, 2048]
    ident = np.eye(128, dtype=f32)
    bo4 = np.broadcast_to(np.asarray(b_o, f32) / 4.0, (128, D)).copy()

    import os as os_mod
    if os_mod.environ.get("BASS_X_BF16"):
        import ml_dtypes
        xdt = ml_dtypes.bfloat16
    else:
        xdt = f32
    wqT = np.ascontiguousarray(np.asarray(w_q, f32).T)  # [D_in, D_out]
    wkT = np.ascontiguousarray(np.asarray(w_k, f32).T)
    wvT = np.ascontiguousarray(np.asarray(w_v, f32).T)
    woT = np.ascontiguousarray(np.asarray(w_o, f32).T)  # [D_in, D_out]

    xT = {}
    for g in range(B):
        xT[("q", g)] = np.ascontiguousarray(np.asarray(query[g], f32).T.astype(xdt))
        xT[("k", g)] = np.ascontiguousarray(np.asarray(key_[g], f32).T.astype(xdt))
        xT[("v", g)] = np.ascontiguousarray(np.asarray(value[g], f32).T.astype(xdt))

    in_maps = []
    for c in range(N_CORES):
        g, p = c // 4, c % 4
        fsel = slice(FPC * p, FPC * (p + 1))
        in_maps.append({
            "xq": xT[("q", g)],
            "xk": xT[("k", g)],
            "xv": xT[("v", g)],
            "wq": np.ascontiguousarray(wqT[:, fsel].astype(xdt)),
            "wk": np.ascontiguousarray(wkT[:, fsel].astype(xdt)),
            "wv": np.ascontiguousarray(wvT[:, fsel].astype(xdt)),
            "wo": np.ascontiguousarray(woT[fsel, :]),
            "bq": np.ascontiguousarray(np.asarray(b_q, f32)[fsel].reshape(FPC, 1)),
            "bk": np.ascontiguousarray(np.asarray(b_k, f32)[fsel].reshape(FPC, 1)),
            "bv": np.ascontiguousarray(np.asarray(b_v, f32)[fsel].reshape(FPC, 1)),
            "bo4": bo4,
            "masks": masks,
            "ident": ident,
        })
    return in_maps


def run(inputs, trace=False):
    from concourse.bass_utils import run_bass_kernel_spmd

    if "nc" not in _CACHE:
        _CACHE["nc"] = _build_nc()
    nc = _CACHE["nc"]
    in_maps = _prep_inputs(
        inputs["query"], inputs["key_"], inputs["value"],
        inputs["w_q"], inputs["b_q"], inputs["w_k"], inputs["b_k"],
        inputs["w_v"], inputs["b_v"], inputs["w_o"], inputs["b_o"],
    )
    res = run_bass_kernel_spmd(
        nc, in_maps, core_ids=list(range(N_CORES)), trace=trace,
    )
    out = np.empty((B, S, D), np.float32)
    for c in range(N_CORES):
        g, p = c // 4, c % 4
        # RS half i scatters q rows [1024*i + 256*p, 1024*i + 256*(p+1))
        out[g, 256 * p : 256 * (p + 1), :] = res.results[c]["out"][0:256]
        out[g, 1024 + 256 * p : 1024 + 256 * (p + 1), :] = res.results[c]["out"][256:512]
    return out, res


def kernel(**inputs):
    out, _ = run(inputs, trace=False)
    return out



# revision 5
# speedup vs baseline: 1.2715x; 1.2715x over previous
"""Causal multi-head attention on 8 Trainium2 NeuronCores.

Sharding: core c -> (batch g = c // 4, head-group p = c % 4, heads 4p..4p+3).
Each core projects Q/K/V for its batch with its 256 feature columns
(column-sharded w_q/w_k/w_v), runs causal attention for its 4 heads, computes
the partial output projection with its 256 rows of w_o, and a ReduceScatter
over each batch group sums the partials.

All matmul operands are bf16 (fp32 PSUM accumulation).  Tricks:
  - K bias is dropped: softmax((q+bq)(k+bk)^T) == softmax((q+bq) k^T) since
    the (q+bq)*bk term is constant along the softmax (k) axis.
  - Q/V/O biases enter PSUM as rank-1 ones-row matmuls on the PE (no vector
    engine bias adds).
  - V is computed in natural [kpos, feature] layout directly (no transpose),
    with an appended ones column per head so PV accumulates the softmax
    denominators for free.
  - Scores/exp/PV touch only the causally valid column range of each k tile;
    only the [128,128] triangular diagonal block needs a mask multiply.
  - Software pipeline: the V projection pass is interleaved with the first
    head-pair's score matmuls (feeds the activation engine early); o-proj
    half 0 is interleaved into attention as PE filler so its ReduceScatter
    fully overlaps the remaining attention work.
"""

import numpy as np

B, S, D, H = 2, 2048, 1024, 16
DK = D // H  # 64
N_CORES = 8
FPC = 256  # features per core

_CACHE = {}


def _build_nc():
    import os as os_mod
    from contextlib import ExitStack

    import concourse.mybir as mybir
    import concourse.tile as tile
    from concourse import bacc

    F32 = mybir.dt.float32
    BF16 = mybir.dt.bfloat16
    Exp = mybir.ActivationFunctionType.Exp
    Copy = mybir.ActivationFunctionType.Copy

    nc = bacc.Bacc("TRN2", target_bir_lowering=False, debug=False, num_devices=8)

    xq = nc.dram_tensor("xq", [D, S], BF16, kind="ExternalInput")
    xk = nc.dram_tensor("xk", [D, S], BF16, kind="ExternalInput")
    xv = nc.dram_tensor("xv", [D, S], BF16, kind="ExternalInput")
    wq = nc.dram_tensor("wq", [D, FPC], BF16, kind="ExternalInput")
    wk = nc.dram_tensor("wk", [D, FPC], BF16, kind="ExternalInput")
    wv = nc.dram_tensor("wv", [D, FPC], BF16, kind="ExternalInput")
    wo = nc.dram_tensor("wo", [FPC, D], BF16, kind="ExternalInput")
    bq = nc.dram_tensor("bq", [1, FPC], BF16, kind="ExternalInput")
    bv = nc.dram_tensor("bv", [1, FPC], BF16, kind="ExternalInput")
    bo4 = nc.dram_tensor("bo4", [1, D], BF16, kind="ExternalInput")
    mtri = nc.dram_tensor("mtri", [128, 128], BF16, kind="ExternalInput")
    out = nc.dram_tensor("out", [512, D], F32, kind="ExternalOutput")

    debug_taps = bool(os_mod.environ.get("BASS_DEBUG_TAPS"))
    if debug_taps:
        dbg_q = nc.dram_tensor("dbg_q", [128, 2 * S], BF16, kind="ExternalOutput")
        dbg_k = nc.dram_tensor("dbg_k", [128, 2 * S], BF16, kind="ExternalOutput")
        dbg_v = nc.dram_tensor("dbg_v", [128, 16 * 4 * 65], BF16, kind="ExternalOutput")
        dbg_c = nc.dram_tensor("dbg_c", [128, 2 * S], BF16, kind="ExternalOutput")

    with tile.TileContext(nc) as tc:
        with (
            tc.tile_pool(name="consts", bufs=1) as consts,
            tc.tile_pool(name="persist", bufs=1) as persist,
            tc.tile_pool(name="xin", bufs=3) as xin,
            tc.tile_pool(name="prs", bufs=20) as prs,
            tc.tile_pool(name="normp", bufs=2) as normp,
            tc.tile_pool(name="oout", bufs=3) as oout,
            tc.tile_pool(name="dram", bufs=1, space="DRAM") as dram,
        ):
            # ---- SBUF constants ----
            wq_s = consts.tile([128, 8, FPC], BF16, tag="wq")
            wk_s = consts.tile([128, 8, FPC], BF16, tag="wk")
            wv_s = consts.tile([128, 8, FPC], BF16, tag="wv")
            wo_s = consts.tile([128, 2, D], BF16, tag="wo")
            bq_s = consts.tile([1, FPC], BF16, tag="bq")
            bv_s = consts.tile([1, FPC], BF16, tag="bv")
            bo4_s = consts.tile([1, D], BF16, tag="bo4")
            mask_s = consts.tile([128, 128], BF16, tag="mask")
            ones_s = consts.tile([1, 512], BF16, tag="ones")

            # ---- persistent activations ----
            # feature f of the core maps to (pt = f // 128, row = f % 128);
            # local head h lives at [64*(h%2) : 64*(h%2)+64, h//2, :]
            qT_s = persist.tile([128, 2, S], BF16, tag="qT")
            kT_s = persist.tile([128, 2, S], BF16, tag="kT")
            v_s = persist.tile([128, 16, 4, 65], BF16, tag="v")
            ctx_s = persist.tile([128, 2, S], BF16, tag="ctx")

            nc.vector.memset(ones_s[:], 1.0)
            nc.vector.memset(v_s[:, :, :, 64:65], 1.0)

            # ---- constant DMAs on the gpsimd queue ----
            gq = nc.gpsimd
            gq.dma_start(wq_s[:], wq.ap().rearrange("(kc p) f -> p kc f", p=128))
            gq.dma_start(bq_s[:], bq.ap())
            gq.dma_start(wk_s[:], wk.ap().rearrange("(kc p) f -> p kc f", p=128))
            gq.dma_start(wv_s[:], wv.ap().rearrange("(kc p) f -> p kc f", p=128))
            gq.dma_start(bv_s[:], bv.ap())
            gq.dma_start(mask_s[:], mtri.ap())
            gq.dma_start(wo_s[:], wo.ap().rearrange("(fc p) d -> p fc d", p=128))
            gq.dma_start(bo4_s[:], bo4.ap())

            # xq chunk 0 split in 4 pieces so the first matmuls start early
            xt0 = xin.tile([128, S], BF16, tag="x", name="xt0")
            for qb in range(4):
                nc.sync.dma_start(
                    xt0[:, 512 * qb : 512 * (qb + 1)],
                    xq[0:128, 512 * qb : 512 * (qb + 1)],
                )

            # xv is resident for the natural-layout V pass
            xvp_stack = ExitStack()
            xvp = xvp_stack.enter_context(tc.tile_pool(name="xvp", bufs=1))
            xv_all = xvp.tile([128, 8, S], BF16, tag="xva")
            for kc in range(8):
                gq.dma_start(xv_all[:, kc, :], xv[128 * kc : 128 * (kc + 1), :])

            # ---- phase 1a: Q and K projections (transposed layout) ----
            st1 = ExitStack()
            psP = st1.enter_context(tc.tile_pool(name="psP", bufs=8, space="PSUM"))

            def proj_pass(x_dram, w_tile, b_tile, dst, first):
                ps = {}
                for kc in range(8):
                    if first and kc == 0:
                        xt = xt0
                    else:
                        xt = xin.tile([128, S], BF16, tag="x")
                        nc.sync.dma_start(xt[:], x_dram[128 * kc : 128 * (kc + 1), :])
                    for pt in range(2):
                        for qb in range(4):
                            if kc == 0:
                                ps[(pt, qb)] = psP.tile(
                                    [128, 512], F32, tag="pp", name=f"pp{pt}{qb}"
                                )
                            nc.tensor.matmul(
                                ps[(pt, qb)][:],
                                w_tile[:, kc, 128 * pt : 128 * (pt + 1)],
                                xt[:, 512 * qb : 512 * (qb + 1)],
                                start=(kc == 0),
                                stop=(kc == 7 and b_tile is None),
                            )
                for pt in range(2):
                    for qb in range(4):
                        if b_tile is not None:
                            nc.tensor.matmul(
                                ps[(pt, qb)][:],
                                b_tile[0:1, 128 * pt : 128 * (pt + 1)],
                                ones_s[0:1, 0:512],
                                start=False,
                                stop=True,
                                skip_group_check=True,
                            )
                        nc.scalar.activation(
                            dst[:, pt, 512 * qb : 512 * (qb + 1)],
                            ps[(pt, qb)][:],
                            Copy,
                        )

            proj_pass(xq, wq_s, bq_s, qT_s, first=True)
            proj_pass(xk, wk_s, None, kT_s, first=False)
            st1.close()

            # ---- attention state/helpers ----
            attn_stack = ExitStack()
            psS = attn_stack.enter_context(
                tc.tile_pool(name="psS", bufs=2, space="PSUM")
            )
            pr_map = {}
            ctx_map = {}

            def segments(s0):
                if s0 < 512:
                    return [(s0, 512), (512, 1024)]
                return [(s0, 1024)]

            def emit_scores(qbp, hp, si, ki):
                s0 = max(0, 128 * ki - 1024 * qbp)
                sc = psS.tile([128, 1024], F32, tag="sc", name=f"sc{qbp}{hp}{si}_{ki}")
                kT = kT_s[64 * si : 64 * si + 64, hp, 128 * ki : 128 * (ki + 1)]
                for a, b in segments(s0):
                    nc.tensor.matmul(
                        sc[:, a:b],
                        kT,
                        qT_s[64 * si : 64 * si + 64, hp, 1024 * qbp + a : 1024 * qbp + b],
                        start=True,
                        stop=True,
                    )
                pr = prs.tile([128, 1024], BF16, tag="pr", name=f"pr{qbp}{hp}{si}_{ki}")
                nc.scalar.activation(
                    out=pr[:, s0:1024], in_=sc[:, s0:1024], func=Exp, scale=0.125
                )
                if 128 * ki >= 1024 * qbp:  # diagonal tile inside this window
                    nc.vector.tensor_mul(
                        pr[:, s0 : s0 + 128], pr[:, s0 : s0 + 128], mask_s[:]
                    )
                pr_map[(qbp, hp, si, ki)] = (pr, s0)

            def emit_pv(qbp, hp, si, ki, psA):
                nkt = 8 * (qbp + 1)
                pr, s0 = pr_map.pop((qbp, hp, si, ki))
                key = (qbp, hp, si)
                if key not in ctx_map:
                    ctx_map[key] = psA.tile(
                        [65, 1024], F32, tag="ctx", name=f"ctx{qbp}{hp}{si}"
                    )
                ctx = ctx_map[key]
                last_a = (512 + 1024 * qbp) // 128 - 1
                for a, b in segments(s0):
                    last = last_a if b == 512 else nkt - 1
                    nc.tensor.matmul(
                        ctx[:, a:b],
                        v_s[:, ki, 2 * hp + si, :],
                        pr[:, a:b],
                        start=(ki == 0),
                        stop=(ki == last),
                        skip_group_check=True,
                    )

            def emit_norm(qbp, hp, si):
                ctx = ctx_map.pop((qbp, hp, si))
                ctmp = normp.tile([65, 1024], BF16, tag="ctmp")
                nc.vector.tensor_copy(ctmp[:], ctx[:])
                rc = normp.tile([1, 1024], BF16, tag="rc")
                with nc.allow_low_precision("softmax denominator recip in bf16"):
                    nc.vector.reciprocal(rc[:], ctmp[64:65, :])
                rbc = normp.tile([64, 1024], BF16, tag="rbc")
                nc.gpsimd.partition_broadcast(rbc[:], rc[:])
                nc.vector.tensor_mul(
                    ctx_s[64 * si : 64 * si + 64, hp, 1024 * qbp : 1024 * (qbp + 1)],
                    ctmp[0:64, :],
                    rbc[:],
                )

            rs_in = [dram.tile([S // 2, D], F32, name=f"rs_in{i}") for i in range(2)]
            rs_out = [dram.tile([256, D], F32, name=f"rs_out{i}") for i in range(2)]

            def emit_oproj_sl(h, sl, pool, evac):
                st = 8 * h + sl
                po = pool.tile([128, 1024], F32, tag="sc", name=f"po{h}_{sl}")
                for nb in range(2):
                    for fc in range(2):
                        nc.tensor.matmul(
                            po[:, 512 * nb : 512 * (nb + 1)],
                            ctx_s[:, fc, 128 * st : 128 * (st + 1)],
                            wo_s[:, fc, 512 * nb : 512 * (nb + 1)],
                            start=(fc == 0),
                            stop=False,
                        )
                    nc.tensor.matmul(
                        po[:, 512 * nb : 512 * (nb + 1)],
                        ones_s[0:1, 0:128],
                        bo4_s[0:1, 512 * nb : 512 * (nb + 1)],
                        start=False,
                        stop=True,
                        skip_group_check=True,
                    )
                ot = oout.tile([128, 1024], F32, tag="ot")
                if evac == "act":
                    nc.scalar.activation(ot[:], po[:], Copy)
                else:
                    nc.vector.tensor_copy(ot[:], po[:])
                nc.sync.dma_start(rs_in[h][128 * sl : 128 * (sl + 1), :], ot[:])

            def emit_rs(h):
                if not os_mod.environ.get("BASS_SIM_NO_RS"):
                    import concourse.mybir as mybir_mod

                    nc.gpsimd.collective_compute(
                        "ReduceScatter",
                        mybir_mod.AluOpType.add,
                        replica_groups=[[0, 1, 2, 3], [4, 5, 6, 7]],
                        ins=[rs_in[h].opt()],
                        outs=[rs_out[h].opt()],
                    )
                    nc.sync.dma_start(
                        out[256 * h : 256 * (h + 1), :], rs_out[h][:]
                    )
                else:
                    nc.sync.dma_start(
                        out[256 * h : 256 * (h + 1), :], rs_in[h][0:256, :]
                    )

            # ---- phase 1b: natural-layout V pass, interleaved with the ----
            # ---- first head-pair's qbp0 scores (feeds Act early)        ----
            stV = ExitStack()
            psV = stV.enter_context(tc.tile_pool(name="psV", bufs=4, space="PSUM"))
            for sp in range(8):
                pv = psV.tile([128, 512], F32, tag="pv", name=f"pv{sp}")
                # the two accumulation groups sharing this bank must not
                # interleave: a group's first matmul clears has_written for
                # the WHOLE bank, wiping the other group's progress
                for j in range(2):
                    st = 2 * sp + j
                    for kc in range(8):
                        nc.tensor.matmul(
                            pv[:, 256 * j : 256 * (j + 1)],
                            xv_all[:, kc, 128 * st : 128 * (st + 1)],
                            wv_s[:, kc, :],
                            start=(kc == 0),
                            stop=False,
                        )
                    nc.tensor.matmul(
                        pv[:, 256 * j : 256 * (j + 1)],
                        ones_s[0:1, 0:128],
                        bv_s[0:1, :],
                        start=False,
                        stop=True,
                        skip_group_check=True,
                    )
                nc.scalar.activation(
                    v_s[:, 2 * sp : 2 * sp + 2, :, 0:64],
                    pv[:].rearrange("p (j h x) -> p j h x", j=2, h=4),
                    Copy,
                )
                # interleave: hp0 qbp0 scores for ki == sp
                emit_scores(0, 0, 0, sp)
                emit_scores(0, 0, 1, sp)
            stV.close()
            xvp_stack.close()

            psA = attn_stack.enter_context(
                tc.tile_pool(name="psA", bufs=2, space="PSUM")
            )

            # ---- stage (qbp0, hp1): scores + hp0 PVs ----
            for ki in range(8):
                for si in range(2):
                    emit_scores(0, 1, si, ki)
                for si in range(2):
                    emit_pv(0, 0, si, ki, psA)
            for si in range(2):
                emit_norm(0, 0, si)

            # ---- stage (qbp1, hp0): scores + qbp0-hp1 PVs + own PVs ----
            # ---- + o-proj half0 as PE filler after norm(0,1)         ----
            for ki in range(16):
                for si in range(2):
                    emit_scores(1, 0, si, ki)
                if ki < 8:
                    for si in range(2):
                        emit_pv(0, 1, si, ki, psA)
                if ki >= 1:
                    for si in range(2):
                        emit_pv(1, 0, si, ki - 1, psA)
                if ki == 8:
                    for si in range(2):
                        emit_norm(0, 1, si)
                if ki >= 8:
                    emit_oproj_sl(0, ki - 8, psS, "dve")
            for si in range(2):
                emit_pv(1, 0, si, 15, psA)
            for si in range(2):
                emit_norm(1, 0, si)
            emit_rs(0)

            # ---- stage (qbp1, hp1): scores + own PVs (1-ki lag) ----
            for ki in range(16):
                for si in range(2):
                    emit_scores(1, 1, si, ki)
                if ki >= 1:
                    for si in range(2):
                        emit_pv(1, 1, si, ki - 1, psA)
            for si in range(2):
                emit_pv(1, 1, si, 15, psA)
            for si in range(2):
                emit_norm(1, 1, si)
            attn_stack.close()

            if debug_taps:
                nc.sync.dma_start(dbg_q.ap(), qT_s[:].rearrange("p a b -> p (a b)"))
                nc.sync.dma_start(dbg_k.ap(), kT_s[:].rearrange("p a b -> p (a b)"))
                nc.sync.dma_start(dbg_v.ap(), v_s[:].rearrange("p a b c -> p (a b c)"))
                nc.sync.dma_start(dbg_c.ap(), ctx_s[:].rearrange("p a b -> p (a b)"))

            # ---- o-proj half 1 + final ReduceScatter ----
            stO = ExitStack()
            psO = stO.enter_context(tc.tile_pool(name="psO", bufs=4, space="PSUM"))
            for sl in range(8):
                emit_oproj_sl(1, sl, psO, "act")
            stO.close()
            emit_rs(1)

    nc.compile()
    return nc


def _prep_inputs(query, key_, value, w_q, b_q, w_k, b_k, w_v, b_v, w_o, b_o):
    """Build the 8 per-core input maps (host-side sharding / re-layout)."""
    import ml_dtypes

    bf16 = ml_dtypes.bfloat16
    f32 = np.float32

    r = np.arange(128)[:, None]
    j = np.arange(128)[None, :]
    mtri = (j >= r).astype(bf16)  # allowed iff q >= k on the diagonal tile

    wqT = np.ascontiguousarray(np.asarray(w_q, f32).T)  # [D_in, D_out]
    wkT = np.ascontiguousarray(np.asarray(w_k, f32).T)
    wvT = np.ascontiguousarray(np.asarray(w_v, f32).T)
    woT = np.ascontiguousarray(np.asarray(w_o, f32).T)

    xT = {}
    for g in range(B):
        xT[("q", g)] = np.ascontiguousarray(np.asarray(query[g], f32).T.astype(bf16))
        xT[("k", g)] = np.ascontiguousarray(np.asarray(key_[g], f32).T.astype(bf16))
        xT[("v", g)] = np.ascontiguousarray(np.asarray(value[g], f32).T.astype(bf16))

    bo4 = (np.asarray(b_o, f32) / 4.0).reshape(1, D).astype(bf16)

    in_maps = []
    for c in range(N_CORES):
        g, p = c // 4, c % 4
        fsel = slice(FPC * p, FPC * (p + 1))
        in_maps.append({
            "xq": xT[("q", g)],
            "xk": xT[("k", g)],
            "xv": xT[("v", g)],
            "wq": np.ascontiguousarray(wqT[:, fsel].astype(bf16)),
            "wk": np.ascontiguousarray(wkT[:, fsel].astype(bf16)),
            "wv": np.ascontiguousarray(wvT[:, fsel].astype(bf16)),
            "wo": np.ascontiguousarray(woT[fsel, :].astype(bf16)),
            "bq": np.ascontiguousarray(
                np.asarray(b_q, f32)[fsel].reshape(1, FPC).astype(bf16)),
            "bv": np.ascontiguousarray(
                np.asarray(b_v, f32)[fsel].reshape(1, FPC).astype(bf16)),
            "bo4": bo4,
            "mtri": mtri,
        })
    return in_maps


def run(inputs, trace=False):
    from concourse.bass_utils import run_bass_kernel_spmd

    if "nc" not in _CACHE:
        _CACHE["nc"] = _build_nc()
    nc = _CACHE["nc"]
    in_maps = _prep_inputs(
        inputs["query"], inputs["key_"], inputs["value"],
        inputs["w_q"], inputs["b_q"], inputs["w_k"], inputs["b_k"],
        inputs["w_v"], inputs["b_v"], inputs["w_o"], inputs["b_o"],
    )
    res = run_bass_kernel_spmd(
        nc, in_maps, core_ids=list(range(N_CORES)), trace=trace,
    )
    out = np.empty((B, S, D), np.float32)
    for c in range(N_CORES):
        g, p = c // 4, c % 4
        # RS half i scatters q rows [1024*i + 256*p, 1024*i + 256*(p+1))
        out[g, 256 * p : 256 * (p + 1), :] = res.results[c]["out"][0:256]
        out[g, 1024 + 256 * p : 1024 + 256 * (p + 1), :] = res.results[c]["out"][256:512]
    return out, res


def kernel(**inputs):
    out, _ = run(inputs, trace=False)
    return out


# revision 25
# speedup vs baseline: 1.3201x; 1.0382x over previous
"""Causal multi-head attention on 8 Trainium2 NeuronCores.

Sharding: core c -> (batch g = c // 4, head-group p = c % 4, heads 4p..4p+3).
Each core projects Q/K/V for its batch with its 256 feature columns
(column-sharded w_q/w_k/w_v), runs causal attention for its 4 heads, computes
the partial output projection with its 256 rows of w_o, and a ReduceScatter
over each batch group sums the partials.

All matmul operands are bf16 (fp32 PSUM accumulation).  Tricks:
  - K bias is dropped: softmax((q+bq)(k+bk)^T) == softmax((q+bq) k^T) since
    the (q+bq)*bk term is constant along the softmax (k) axis.
  - Q/V/O biases enter PSUM as rank-1 ones-row matmuls on the PE (no vector
    engine bias adds).
  - V is computed in natural [kpos, feature] layout directly (no transpose),
    with an appended ones column per head so PV accumulates the softmax
    denominators for free.
  - Scores/exp/PV touch only the causally valid column range of each k tile;
    only the [128,128] triangular diagonal block needs a mask multiply.
  - Software pipeline: the V projection pass is interleaved with the first
    head-pair's score matmuls (feeds the activation engine early); o-proj
    half 0 is interleaved into attention as PE filler so its ReduceScatter
    fully overlaps the remaining attention work.
"""

import numpy as np

B, S, D, H = 2, 2048, 1024, 16
DK = D // H  # 64
N_CORES = 8
FPC = 256  # features per core

_CACHE = {}


def _build_nc():
    import os as os_mod
    from contextlib import ExitStack

    import concourse.mybir as mybir
    import concourse.tile as tile
    from concourse import bacc

    F32 = mybir.dt.float32
    BF16 = mybir.dt.bfloat16
    Exp = mybir.ActivationFunctionType.Exp
    Copy = mybir.ActivationFunctionType.Copy

    nc = bacc.Bacc("TRN2", target_bir_lowering=False, debug=False, num_devices=8)

    xq = nc.dram_tensor("xq", [D, S], BF16, kind="ExternalInput")
    xk = nc.dram_tensor("xk", [D, S], BF16, kind="ExternalInput")
    xv = nc.dram_tensor("xv", [D, S], BF16, kind="ExternalInput")
    wq = nc.dram_tensor("wq", [D, FPC], BF16, kind="ExternalInput")
    wk = nc.dram_tensor("wk", [D, FPC], BF16, kind="ExternalInput")
    wv = nc.dram_tensor("wv", [D, FPC], BF16, kind="ExternalInput")
    wo = nc.dram_tensor("wo", [FPC, D], BF16, kind="ExternalInput")
    bq = nc.dram_tensor("bq", [1, FPC], BF16, kind="ExternalInput")
    bv = nc.dram_tensor("bv", [1, FPC], BF16, kind="ExternalInput")
    bo4 = nc.dram_tensor("bo4", [128, D], BF16, kind="ExternalInput")
    mtri = nc.dram_tensor("mtri", [128, 2, 128], BF16, kind="ExternalInput")
    out = nc.dram_tensor("out", [512, D], F32, kind="ExternalOutput")

    debug_taps = bool(os_mod.environ.get("BASS_DEBUG_TAPS"))
    if debug_taps:
        dbg_q = nc.dram_tensor("dbg_q", [128, 2 * S], BF16, kind="ExternalOutput")
        dbg_k = nc.dram_tensor("dbg_k", [128, 2 * S], BF16, kind="ExternalOutput")
        dbg_v = nc.dram_tensor("dbg_v", [128, 16 * 4 * 65], BF16, kind="ExternalOutput")
        dbg_c = nc.dram_tensor("dbg_c", [128, 2 * S], BF16, kind="ExternalOutput")

    with tile.TileContext(nc) as tc:
        with (
            tc.tile_pool(name="consts", bufs=1) as consts,
            tc.tile_pool(name="persist", bufs=1) as persist,
            tc.tile_pool(name="xin", bufs=3) as xin,
            tc.tile_pool(name="prs", bufs=20) as prs,
            tc.tile_pool(name="normp", bufs=2) as normp,
            tc.tile_pool(name="oout", bufs=3) as oout,
            tc.tile_pool(name="dram", bufs=1, space="DRAM") as dram,
        ):
            # ---- SBUF constants ----
            wq_s = consts.tile([128, 8, FPC], BF16, tag="wq")
            wk_s = consts.tile([128, 8, FPC], BF16, tag="wk")
            wv_s = consts.tile([128, 8, FPC], BF16, tag="wv")
            wo_s = consts.tile([128, 2, D], BF16, tag="wo")
            bq_s = consts.tile([1, FPC], BF16, tag="bq")
            bv_s = consts.tile([1, FPC], BF16, tag="bv")
            bo4_s = consts.tile([128, D], BF16, tag="bo4")
            mask_s = consts.tile([128, 2, 128], BF16, tag="mask")
            ones_s = consts.tile([1, 512], BF16, tag="ones")

            # ---- persistent activations ----
            # feature f of the core maps to (pt = f // 128, row = f % 128);
            # local head h lives at [64*(h%2) : 64*(h%2)+64, h//2, :]
            qT_s = persist.tile([128, 2, S], BF16, tag="qT")
            kT_s = persist.tile([128, 2, S], BF16, tag="kT")
            v_s = persist.tile([128, 16, 4, 65], BF16, tag="v")
            ctx_s = persist.tile([128, 2, S], BF16, tag="ctx")

            nc.vector.memset(ones_s[:], 1.0)
            nc.vector.memset(v_s[:, :, :, 64:65], 1.0)

            # ---- constant DMAs on the gpsimd queue ----
            # only what phase 1a needs up front; the rest is marker-gated so
            # it doesn't preempt the just-in-time xq/xk chunk streams in the
            # shared DMA pool
            gq = nc.gpsimd
            gq.dma_start(wq_s[:], wq.ap().rearrange("(kc p) f -> p kc f", p=128))
            gq.dma_start(bq_s[:], bq.ap())
            gq.dma_start(wk_s[:], wk.ap().rearrange("(kc p) f -> p kc f", p=128))

            def issue_late_const_dmas():
                for tile_, src in (
                    (wv_s, wv.ap().rearrange("(kc p) f -> p kc f", p=128)),
                    (bv_s, bv.ap()),
                    (mask_s, mtri.ap()),
                    (wo_s, wo.ap().rearrange("(fc p) d -> p fc d", p=128)),
                    (bo4_s, bo4.ap()),
                ):
                    nc.vector.memset(tile_[0:1, 0:1], 0.0)
                    gq.dma_start(tile_[:], src)

            # xq chunk 0 split in 4 pieces so the first matmuls start early
            xt0 = xin.tile([128, S], BF16, tag="x", name="xt0")
            for qb in range(4):
                nc.sync.dma_start(
                    xt0[:, 512 * qb : 512 * (qb + 1)],
                    xq[0:128, 512 * qb : 512 * (qb + 1)],
                )

            # xv is resident for the natural-layout V pass.  Gate each chunk's
            # DMA on a DVE marker memset sequenced after the Q evacuations so
            # the xv transfers don't steal shared DMA bandwidth from the
            # just-in-time xq/xk chunk loads.
            xvp_stack = ExitStack()
            xvp = xvp_stack.enter_context(tc.tile_pool(name="xvp", bufs=1))
            xv_all = xvp.tile([128, 8, S], BF16, tag="xva")

            def issue_xv_dmas():
                for kc in range(8):
                    nc.vector.memset(xv_all[0:1, kc, 0:1], 0.0)
                    gq.dma_start(
                        xv_all[:, kc, :], xv[128 * kc : 128 * (kc + 1), :]
                    )

            # ---- phase 1a: Q and K projections (transposed layout) ----
            st1 = ExitStack()
            psP = st1.enter_context(tc.tile_pool(name="psP", bufs=8, space="PSUM"))

            def proj_pass(x_dram, w_tile, b_tile, dst, first):
                ps = {}
                for kc in range(8):
                    if first and kc == 0:
                        xt = xt0
                    else:
                        xt = xin.tile([128, S], BF16, tag="x")
                        nc.sync.dma_start(xt[:], x_dram[128 * kc : 128 * (kc + 1), :])
                    for pt in range(2):
                        for qb in range(4):
                            if kc == 0:
                                ps[(pt, qb)] = psP.tile(
                                    [128, 512], F32, tag="pp", name=f"pp{pt}{qb}"
                                )
                            nc.tensor.matmul(
                                ps[(pt, qb)][:],
                                w_tile[:, kc, 128 * pt : 128 * (pt + 1)],
                                xt[:, 512 * qb : 512 * (qb + 1)],
                                start=(kc == 0),
                                stop=(kc == 7 and b_tile is None),
                            )
                for pt in range(2):
                    for qb in range(4):
                        if b_tile is not None:
                            nc.tensor.matmul(
                                ps[(pt, qb)][:],
                                b_tile[0:1, 128 * pt : 128 * (pt + 1)],
                                ones_s[0:1, 0:512],
                                start=False,
                                stop=True,
                                skip_group_check=True,
                            )
                        nc.vector.tensor_copy(
                            dst[:, pt, 512 * qb : 512 * (qb + 1)],
                            ps[(pt, qb)][:],
                        )

            proj_pass(xq, wq_s, bq_s, qT_s, first=True)
            issue_xv_dmas()  # DVE markers sequence these after the Q evacs
            proj_pass(xk, wk_s, None, kT_s, first=False)
            issue_late_const_dmas()
            st1.close()

            # ---- attention state/helpers ----
            attn_stack = ExitStack()
            psS = attn_stack.enter_context(
                tc.tile_pool(name="psS", bufs=2, space="PSUM")
            )
            pr_map = {}
            ctx_map = {}

            def segments(s0):
                if s0 < 512:
                    return [(s0, 512), (512, 1024)]
                return [(s0, 1024)]

            def emit_scores(qbp, hp, ki):
                # both si heads of the pair share si-interleaved sc/pr tiles
                # so exp and the mask multiply cover two heads per
                # instruction; one sc tile per 512-column segment keeps the
                # PSUM footprint at 2 banks so bufs=2 still pipelines
                s0 = max(0, 128 * ki - 1024 * qbp)
                pr = prs.tile([128, 2, 1024], BF16, tag="pr", name=f"pr{qbp}{hp}_{ki}")
                for a, b in segments(s0):
                    sc = psS.tile(
                        [128, 2, 512], F32, tag="sc", name=f"sc{qbp}{hp}_{ki}_{a}"
                    )
                    for si in range(2):
                        nc.tensor.matmul(
                            sc[:, si, 0 : b - a],
                            kT_s[64 * si : 64 * si + 64, hp,
                                 128 * ki : 128 * (ki + 1)],
                            qT_s[64 * si : 64 * si + 64, hp,
                                 1024 * qbp + a : 1024 * qbp + b],
                            start=True,
                            stop=True,
                        )
                    nc.scalar.activation(
                        out=pr[:, :, a:b], in_=sc[:, :, 0 : b - a],
                        func=Exp, scale=0.125,
                    )
                if 128 * ki >= 1024 * qbp:  # diagonal tile inside this window
                    nc.vector.tensor_mul(
                        pr[:, :, s0 : s0 + 128], pr[:, :, s0 : s0 + 128], mask_s[:]
                    )
                pr_map[(qbp, hp, ki)] = (pr, s0)

            def emit_pv(qbp, hp, ki, psA):
                nkt = 8 * (qbp + 1)
                pr, s0 = pr_map.pop((qbp, hp, ki))
                last_a = (512 + 1024 * qbp) // 128 - 1
                for si in range(2):
                    key = (qbp, hp, si)
                    if key not in ctx_map:
                        ctx_map[key] = psA.tile(
                            [65, 1024], F32, tag="ctx", name=f"ctx{qbp}{hp}{si}"
                        )
                    ctx = ctx_map[key]
                    for a, b in segments(s0):
                        last = last_a if b == 512 else nkt - 1
                        nc.tensor.matmul(
                            ctx[:, a:b],
                            v_s[:, ki, 2 * hp + si, :],
                            pr[:, si, a:b],
                            start=(ki == 0),
                            stop=(ki == last),
                            skip_group_check=True,
                        )

            def emit_norm(qbp, hp):
                ctmp, rc, rbc = {}, {}, {}
                for si in range(2):
                    ctmp[si] = normp.tile([65, 1024], BF16, tag="ctmp", name=f"ctmp{si}")
                    nc.vector.tensor_copy(ctmp[si][:], ctx_map.pop((qbp, hp, si))[:])
                for si in range(2):
                    rc[si] = normp.tile([1, 1024], BF16, tag="rc", name=f"rc{si}")
                    with nc.allow_low_precision("softmax denom recip in bf16"):
                        nc.vector.reciprocal(rc[si][:], ctmp[si][64:65, :])
                for si in range(2):
                    rbc[si] = normp.tile([64, 1024], BF16, tag="rbc", name=f"rbc{si}")
                    nc.gpsimd.partition_broadcast(rbc[si][:], rc[si][:])
                for si in range(2):
                    nc.vector.tensor_mul(
                        ctx_s[64 * si : 64 * si + 64, hp,
                              1024 * qbp : 1024 * (qbp + 1)],
                        ctmp[si][0:64, :],
                        rbc[si][:],
                    )

            rs_in = [dram.tile([S // 2, D], F32, name=f"rs_in{i}") for i in range(2)]
            rs_out = [dram.tile([256, D], F32, name=f"rs_out{i}") for i in range(2)]

            def emit_oproj_sl(h, sl, pool, evac, po_shape):
                st = 8 * h + sl
                po_t = pool.tile(po_shape, F32, tag="sc", name=f"po{h}_{sl}")
                three_d = len(po_shape) == 3
                pe_bias = evac == "act"
                for nb in range(2):
                    po_nb = po_t[:, nb, :] if three_d else po_t[:, 512 * nb : 512 * (nb + 1)]
                    for fc in range(2):
                        nc.tensor.matmul(
                            po_nb,
                            ctx_s[:, fc, 128 * st : 128 * (st + 1)],
                            wo_s[:, fc, 512 * nb : 512 * (nb + 1)],
                            start=(fc == 0),
                            stop=(fc == 1 and not pe_bias),
                        )
                    if pe_bias:
                        nc.tensor.matmul(
                            po_nb,
                            ones_s[0:1, 0:128],
                            bo4_s[0:1, 512 * nb : 512 * (nb + 1)],
                            start=False,
                            stop=True,
                            skip_group_check=True,
                        )
                ot = oout.tile([128, 1024], F32, tag="ot")
                po_v = po_t[:] if three_d else po_t[:].rearrange("p (n x) -> p n x", n=2)
                ot_v = ot[:].rearrange("p (n x) -> p n x", n=2)
                if evac == "act":
                    nc.scalar.activation(ot_v, po_v, Copy)
                else:
                    # fold the b_o/4 bias into the PSUM evacuation
                    nc.vector.tensor_add(
                        ot_v, po_v, bo4_s[:].rearrange("p (n x) -> p n x", n=2)
                    )
                nc.sync.dma_start(rs_in[h][128 * sl : 128 * (sl + 1), :], ot[:])

            def emit_rs(h):
                if not os_mod.environ.get("BASS_SIM_NO_RS"):
                    import concourse.mybir as mybir_mod

                    nc.gpsimd.collective_compute(
                        "ReduceScatter",
                        mybir_mod.AluOpType.add,
                        replica_groups=[[0, 1, 2, 3], [4, 5, 6, 7]],
                        ins=[rs_in[h].opt()],
                        outs=[rs_out[h].opt()],
                    )
                    nc.sync.dma_start(
                        out[256 * h : 256 * (h + 1), :], rs_out[h][:]
                    )
                else:
                    nc.sync.dma_start(
                        out[256 * h : 256 * (h + 1), :], rs_in[h][0:256, :]
                    )

            # ---- phase 1b: natural-layout V pass, interleaved with the ----
            # ---- first head-pair's qbp0 scores (feeds Act early)        ----
            stV = ExitStack()
            psV = stV.enter_context(tc.tile_pool(name="psV", bufs=4, space="PSUM"))
            for sp in range(8):
                pv = psV.tile([128, 512], F32, tag="pv", name=f"pv{sp}")
                # the two accumulation groups sharing this bank must not
                # interleave: a group's first matmul clears has_written for
                # the WHOLE bank, wiping the other group's progress
                for j in range(2):
                    st = 2 * sp + j
                    for kc in range(8):
                        nc.tensor.matmul(
                            pv[:, 256 * j : 256 * (j + 1)],
                            xv_all[:, kc, 128 * st : 128 * (st + 1)],
                            wv_s[:, kc, :],
                            start=(kc == 0),
                            stop=False,
                        )
                    nc.tensor.matmul(
                        pv[:, 256 * j : 256 * (j + 1)],
                        ones_s[0:1, 0:128],
                        bv_s[0:1, :],
                        start=False,
                        stop=True,
                        skip_group_check=True,
                    )
                nc.scalar.activation(
                    v_s[:, 2 * sp : 2 * sp + 2, :, 0:64],
                    pv[:].rearrange("p (j h x) -> p j h x", j=2, h=4),
                    Copy,
                )
                # interleave: hp0 qbp0 scores for ki == sp
                emit_scores(0, 0, sp)
            stV.close()
            xvp_stack.close()

            psA = attn_stack.enter_context(
                tc.tile_pool(name="psA", bufs=2, space="PSUM")
            )

            # ---- stage (qbp0, hp1): scores + hp0 PVs ----
            for ki in range(8):
                emit_scores(0, 1, ki)
                emit_pv(0, 0, ki, psA)
            emit_norm(0, 0)

            # ---- stage (qbp1, hp0): scores + qbp0-hp1 PVs, then own  ----
            # ---- PVs + o-proj half0 as PE filler after norm(0,1)     ----
            # (ctx pool has 2 buffer pairs: (1,0)'s PVs may only start
            #  after norm(0,1) releases qbp0-hp1's ctx tiles)
            for ki in range(16):
                emit_scores(1, 0, ki)
                if ki < 8:
                    emit_pv(0, 1, ki, psA)
                if ki == 8:
                    emit_norm(0, 1)
                if ki >= 9:
                    emit_pv(1, 0, ki - 9, psA)
                if ki >= 8:
                    emit_oproj_sl(0, ki - 8, psS, "dve", [128, 2, 512])
            for k in range(7, 16):
                emit_pv(1, 0, k, psA)
            emit_norm(1, 0)
            emit_rs(0)

            # ---- stage (qbp1, hp1): scores + own PVs (1-ki lag) ----
            for ki in range(16):
                emit_scores(1, 1, ki)
                if ki >= 1:
                    emit_pv(1, 1, ki - 1, psA)
            emit_pv(1, 1, 15, psA)
            emit_norm(1, 1)
            attn_stack.close()

            if debug_taps:
                nc.sync.dma_start(dbg_q.ap(), qT_s[:].rearrange("p a b -> p (a b)"))
                nc.sync.dma_start(dbg_k.ap(), kT_s[:].rearrange("p a b -> p (a b)"))
                nc.sync.dma_start(dbg_v.ap(), v_s[:].rearrange("p a b c -> p (a b c)"))
                nc.sync.dma_start(dbg_c.ap(), ctx_s[:].rearrange("p a b -> p (a b)"))

            # ---- o-proj half 1 + final ReduceScatter ----
            stO = ExitStack()
            psO = stO.enter_context(tc.tile_pool(name="psO", bufs=4, space="PSUM"))
            for sl in range(8):
                emit_oproj_sl(1, sl, psO, "act" if sl % 2 else "dve", [128, 1024])
            stO.close()
            emit_rs(1)

    nc.compile()
    return nc


def _prep_inputs(query, key_, value, w_q, b_q, w_k, b_k, w_v, b_v, w_o, b_o):
    """Build the 8 per-core input maps (host-side sharding / re-layout)."""
    import ml_dtypes

    bf16 = ml_dtypes.bfloat16
    f32 = np.float32

    r = np.arange(128)[:, None, None]
    j = np.arange(128)[None, None, :]
    # allowed iff q >= k on the diagonal tile; doubled for the si-pair layout
    mtri = np.broadcast_to(j >= r, (128, 2, 128)).astype(bf16)

    wqT = np.ascontiguousarray(np.asarray(w_q, f32).T)  # [D_in, D_out]
    wkT = np.ascontiguousarray(np.asarray(w_k, f32).T)
    wvT = np.ascontiguousarray(np.asarray(w_v, f32).T)
    woT = np.ascontiguousarray(np.asarray(w_o, f32).T)

    xT = {}
    for g in range(B):
        xT[("q", g)] = np.ascontiguousarray(np.asarray(query[g], f32).T.astype(bf16))
        xT[("k", g)] = np.ascontiguousarray(np.asarray(key_[g], f32).T.astype(bf16))
        xT[("v", g)] = np.ascontiguousarray(np.asarray(value[g], f32).T.astype(bf16))

    bo4 = np.broadcast_to(
        (np.asarray(b_o, f32) / 4.0).reshape(1, D), (128, D)
    ).astype(bf16)

    in_maps = []
    for c in range(N_CORES):
        g, p = c // 4, c % 4
        fsel = slice(FPC * p, FPC * (p + 1))
        in_maps.append({
            "xq": xT[("q", g)],
            "xk": xT[("k", g)],
            "xv": xT[("v", g)],
            "wq": np.ascontiguousarray(wqT[:, fsel].astype(bf16)),
            "wk": np.ascontiguousarray(wkT[:, fsel].astype(bf16)),
            "wv": np.ascontiguousarray(wvT[:, fsel].astype(bf16)),
            "wo": np.ascontiguousarray(woT[fsel, :].astype(bf16)),
            "bq": np.ascontiguousarray(
                np.asarray(b_q, f32)[fsel].reshape(1, FPC).astype(bf16)),
            "bv": np.ascontiguousarray(
                np.asarray(b_v, f32)[fsel].reshape(1, FPC).astype(bf16)),
            "bo4": bo4,
            "mtri": mtri,
        })
    return in_maps


def run(inputs, trace=False):
    from concourse.bass_utils import run_bass_kernel_spmd

    if "nc" not in _CACHE:
        _CACHE["nc"] = _build_nc()
    nc = _CACHE["nc"]
    in_maps = _prep_inputs(
        inputs["query"], inputs["key_"], inputs["value"],
        inputs["w_q"], inputs["b_q"], inputs["w_k"], inputs["b_k"],
        inputs["w_v"], inputs["b_v"], inputs["w_o"], inputs["b_o"],
    )
    res = run_bass_kernel_spmd(
        nc, in_maps, core_ids=list(range(N_CORES)), trace=trace,
    )
    out = np.empty((B, S, D), np.float32)
    for c in range(N_CORES):
        g, p = c // 4, c % 4
        # RS half i scatters q rows [1024*i + 256*p, 1024*i + 256*(p+1))
        out[g, 256 * p : 256 * (p + 1), :] = res.results[c]["out"][0:256]
        out[g, 1024 + 256 * p : 1024 + 256 * (p + 1), :] = res.results[c]["out"][256:512]
    return out, res


def kernel(**inputs):
    out, _ = run(inputs, trace=False)
    return out


# revision 32
# speedup vs baseline: 1.3759x; 1.0423x over previous
"""Causal multi-head attention on 8 Trainium2 NeuronCores.

Sharding: core c -> (batch g = c // 4, head-group p = c % 4, heads 4p..4p+3).
Each core projects Q/K/V for its batch with its 256 feature columns
(column-sharded w_q/w_k/w_v), runs causal attention for its 4 heads, computes
the partial output projection with its 256 rows of w_o, and a ReduceScatter
over each batch group sums the partials.

All matmul operands are bf16 (fp32 PSUM accumulation).  Tricks:
  - K bias is dropped: softmax((q+bq)(k+bk)^T) == softmax((q+bq) k^T) since
    the (q+bq)*bk term is constant along the softmax (k) axis.
  - Q/V/O biases enter PSUM as rank-1 ones-row matmuls on the PE (no vector
    engine bias adds).
  - V is computed in natural [kpos, feature] layout directly (no transpose),
    with an appended ones column per head so PV accumulates the softmax
    denominators for free.
  - Scores/exp/PV touch only the causally valid column range of each k tile;
    only the [128,128] triangular diagonal block needs a mask multiply.
  - Software pipeline: the V projection pass is interleaved with the first
    head-pair's score matmuls (feeds the activation engine early); o-proj
    half 0 is interleaved into attention as PE filler so its ReduceScatter
    fully overlaps the remaining attention work.
"""

import numpy as np

B, S, D, H = 2, 2048, 1024, 16
DK = D // H  # 64
N_CORES = 8
FPC = 256  # features per core

_CACHE = {}


def _build_nc():
    import os as os_mod
    from contextlib import ExitStack

    import concourse.mybir as mybir
    import concourse.tile as tile
    from concourse import bacc

    F32 = mybir.dt.float32
    BF16 = mybir.dt.bfloat16
    Exp = mybir.ActivationFunctionType.Exp
    Copy = mybir.ActivationFunctionType.Copy

    nc = bacc.Bacc("TRN2", target_bir_lowering=False, debug=False, num_devices=8)

    xq = nc.dram_tensor("xq", [D, S], BF16, kind="ExternalInput")
    xk = nc.dram_tensor("xk", [D, S], BF16, kind="ExternalInput")
    xv = nc.dram_tensor("xv", [D, S], BF16, kind="ExternalInput")
    wq = nc.dram_tensor("wq", [D, FPC], BF16, kind="ExternalInput")
    wk = nc.dram_tensor("wk", [D, FPC], BF16, kind="ExternalInput")
    wv = nc.dram_tensor("wv", [D, FPC], BF16, kind="ExternalInput")
    wo = nc.dram_tensor("wo", [FPC, D], BF16, kind="ExternalInput")
    bq = nc.dram_tensor("bq", [1, FPC], BF16, kind="ExternalInput")
    bv = nc.dram_tensor("bv", [1, FPC], BF16, kind="ExternalInput")
    bo4 = nc.dram_tensor("bo4", [128, D], BF16, kind="ExternalInput")
    mtri = nc.dram_tensor("mtri", [128, 2, 128], BF16, kind="ExternalInput")
    out = nc.dram_tensor("out", [512, D], F32, kind="ExternalOutput")

    debug_taps = bool(os_mod.environ.get("BASS_DEBUG_TAPS"))
    if debug_taps:
        dbg_q = nc.dram_tensor("dbg_q", [128, 2 * S], BF16, kind="ExternalOutput")
        dbg_k = nc.dram_tensor("dbg_k", [128, 2 * S], BF16, kind="ExternalOutput")
        dbg_v = nc.dram_tensor("dbg_v", [128, 16 * 4 * 65], BF16, kind="ExternalOutput")
        dbg_c = nc.dram_tensor("dbg_c", [128, 2 * S], BF16, kind="ExternalOutput")

    with tile.TileContext(nc) as tc:
        with (
            tc.tile_pool(name="consts", bufs=1) as consts,
            tc.tile_pool(name="persist", bufs=1) as persist,
            tc.tile_pool(name="xin", bufs=3) as xin,
            tc.tile_pool(name="prs", bufs=20) as prs,
            tc.tile_pool(name="normp", bufs=2) as normp,
            tc.tile_pool(name="oout", bufs=3) as oout,
            tc.tile_pool(name="dram", bufs=1, space="DRAM") as dram,
        ):
            # ---- SBUF constants ----
            wq_s = consts.tile([128, 8, FPC], BF16, tag="wq")
            wk_s = consts.tile([128, 8, FPC], BF16, tag="wk")
            wv_s = consts.tile([128, 8, FPC], BF16, tag="wv")
            wo_s = consts.tile([128, 2, D], BF16, tag="wo")
            bq_s = consts.tile([1, FPC], BF16, tag="bq")
            bv_s = consts.tile([1, FPC], BF16, tag="bv")
            bo4_s = consts.tile([128, D], BF16, tag="bo4")
            mask_s = consts.tile([128, 2, 128], BF16, tag="mask")
            ones_s = consts.tile([1, 512], BF16, tag="ones")

            # ---- persistent activations ----
            # feature f of the core maps to (pt = f // 128, row = f % 128);
            # local head h lives at [64*(h%2) : 64*(h%2)+64, h//2, :]
            qT_s = persist.tile([128, 2, S], BF16, tag="qT")
            kT_s = persist.tile([128, 2, S], BF16, tag="kT")
            v_s = persist.tile([128, 16, 4, 65], BF16, tag="v")
            ctx_s = persist.tile([128, 2, S], BF16, tag="ctx")

            nc.vector.memset(ones_s[:], 1.0)
            nc.vector.memset(v_s[:, :, :, 64:65], 1.0)

            # ---- constant DMAs on the gpsimd queue ----
            # only what phase 1a needs up front; the rest is marker-gated so
            # it doesn't preempt the just-in-time xq/xk chunk streams in the
            # shared DMA pool
            gq = nc.gpsimd
            gq.dma_start(wq_s[:], wq.ap().rearrange("(kc p) f -> p kc f", p=128))
            gq.dma_start(bq_s[:], bq.ap())
            gq.dma_start(wk_s[:], wk.ap().rearrange("(kc p) f -> p kc f", p=128))

            def issue_late_const_dmas():
                # markers are 1-element DVE copies READING kT_s, so they get a
                # real RAW dependency on the K evacuations — the scheduler
                # cannot hoist these DMAs ahead of the jit xq/xk streams
                for marker, full, src, dep in (
                    (wv_s[0:1, 0:1, 0:1], wv_s[:],
                     wv.ap().rearrange("(kc p) f -> p kc f", p=128), 0),
                    (bv_s[0:1, 0:1], bv_s[:], bv.ap(), 0),
                    (mask_s[0:1, 0:1, 0:1], mask_s[:], mtri.ap(), 0),
                    (wo_s[0:1, 0:1, 0:1], wo_s[:],
                     wo.ap().rearrange("(fc p) d -> p fc d", p=128), 1),
                    (bo4_s[0:1, 0:1], bo4_s[:], bo4.ap(), 1),
                ):
                    nc.vector.tensor_copy(
                        marker, kT_s[0:1, dep, 1536 * dep : 1536 * dep + 1]
                    )
                    gq.dma_start(full, src)

            # xq chunk 0 split in 4 pieces so the first matmuls start early
            xt0 = xin.tile([128, S], BF16, tag="x", name="xt0")
            for qb in range(4):
                nc.sync.dma_start(
                    xt0[:, 512 * qb : 512 * (qb + 1)],
                    xq[0:128, 512 * qb : 512 * (qb + 1)],
                )

            # xv is resident for the natural-layout V pass.  Gate each chunk's
            # DMA on a DVE marker memset sequenced after the Q evacuations so
            # the xv transfers don't steal shared DMA bandwidth from the
            # just-in-time xq/xk chunk loads.
            xvp_stack = ExitStack()
            xvp = xvp_stack.enter_context(tc.tile_pool(name="xvp", bufs=1))
            xv_all = xvp.tile([128, 8, S], BF16, tag="xva")

            def issue_xv_dmas():
                # real RAW dependency on the first K evacuation (see
                # issue_late_const_dmas) keeps these out of the DMA pool
                # until the jit xq/xk chunk streams have been issued
                for kc in range(8):
                    nc.vector.tensor_copy(xv_all[0:1, kc, 0:1], kT_s[0:1, 0, 0:1])
                    gq.dma_start(
                        xv_all[:, kc, :], xv[128 * kc : 128 * (kc + 1), :]
                    )

            # ---- phase 1a: Q and K projections (transposed layout) ----
            st1 = ExitStack()
            psP = st1.enter_context(tc.tile_pool(name="psP", bufs=8, space="PSUM"))

            def proj_pass(x_dram, w_tile, b_tile, dst, first):
                ps = {}
                for kc in range(8):
                    if first and kc == 0:
                        xt = xt0
                    else:
                        xt = xin.tile([128, S], BF16, tag="x")
                        nc.sync.dma_start(xt[:], x_dram[128 * kc : 128 * (kc + 1), :])
                    for pt in range(2):
                        for qb in range(4):
                            if kc == 0:
                                ps[(pt, qb)] = psP.tile(
                                    [128, 512], F32, tag="pp", name=f"pp{pt}{qb}"
                                )
                            nc.tensor.matmul(
                                ps[(pt, qb)][:],
                                w_tile[:, kc, 128 * pt : 128 * (pt + 1)],
                                xt[:, 512 * qb : 512 * (qb + 1)],
                                start=(kc == 0),
                                stop=(kc == 7 and b_tile is None),
                            )
                for pt in range(2):
                    for qb in range(4):
                        if b_tile is not None:
                            nc.tensor.matmul(
                                ps[(pt, qb)][:],
                                b_tile[0:1, 128 * pt : 128 * (pt + 1)],
                                ones_s[0:1, 0:512],
                                start=False,
                                stop=True,
                                skip_group_check=True,
                            )
                        nc.vector.tensor_copy(
                            dst[:, pt, 512 * qb : 512 * (qb + 1)],
                            ps[(pt, qb)][:],
                        )

            proj_pass(xq, wq_s, bq_s, qT_s, first=True)
            proj_pass(xk, wk_s, None, kT_s, first=False)
            issue_xv_dmas()
            issue_late_const_dmas()
            st1.close()

            # ---- attention state/helpers ----
            attn_stack = ExitStack()
            psS = attn_stack.enter_context(
                tc.tile_pool(name="psS", bufs=2, space="PSUM")
            )
            pr_map = {}
            ctx_map = {}

            def segments(s0):
                if s0 < 512:
                    return [(s0, 512), (512, 1024)]
                return [(s0, 1024)]

            def emit_scores(qbp, hp, ki):
                # both si heads of the pair share si-interleaved sc/pr tiles
                # so exp and the mask multiply cover two heads per
                # instruction; one sc tile per 512-column segment keeps the
                # PSUM footprint at 2 banks so bufs=2 still pipelines
                s0 = max(0, 128 * ki - 1024 * qbp)
                pr = prs.tile([128, 2, 1024], BF16, tag="pr", name=f"pr{qbp}{hp}_{ki}")
                for a, b in segments(s0):
                    sc = psS.tile(
                        [128, 2, 512], F32, tag="sc", name=f"sc{qbp}{hp}_{ki}_{a}"
                    )
                    for si in range(2):
                        nc.tensor.matmul(
                            sc[:, si, 0 : b - a],
                            kT_s[64 * si : 64 * si + 64, hp,
                                 128 * ki : 128 * (ki + 1)],
                            qT_s[64 * si : 64 * si + 64, hp,
                                 1024 * qbp + a : 1024 * qbp + b],
                            start=True,
                            stop=True,
                        )
                    nc.scalar.activation(
                        out=pr[:, :, a:b], in_=sc[:, :, 0 : b - a],
                        func=Exp, scale=0.125,
                    )
                if 128 * ki >= 1024 * qbp:  # diagonal tile inside this window
                    nc.vector.tensor_mul(
                        pr[:, :, s0 : s0 + 128], pr[:, :, s0 : s0 + 128], mask_s[:]
                    )
                pr_map[(qbp, hp, ki)] = (pr, s0)

            def emit_pv(qbp, hp, ki, psA):
                nkt = 8 * (qbp + 1)
                pr, s0 = pr_map.pop((qbp, hp, ki))
                last_a = (512 + 1024 * qbp) // 128 - 1
                for si in range(2):
                    key = (qbp, hp, si)
                    if key not in ctx_map:
                        ctx_map[key] = psA.tile(
                            [65, 1024], F32, tag="ctx", name=f"ctx{qbp}{hp}{si}"
                        )
                    ctx = ctx_map[key]
                    for a, b in segments(s0):
                        last = last_a if b == 512 else nkt - 1
                        nc.tensor.matmul(
                            ctx[:, a:b],
                            v_s[:, ki, 2 * hp + si, :],
                            pr[:, si, a:b],
                            start=(ki == 0),
                            stop=(ki == last),
                            skip_group_check=True,
                        )

            def emit_norm(qbp, hp, cols=(0, 1024), release=True):
                a, b = cols
                w = b - a
                ctmp, rc, rbc = {}, {}, {}
                for si in range(2):
                    ctx = ctx_map[(qbp, hp, si)]
                    if release and b == 1024:
                        ctx_map.pop((qbp, hp, si))
                    ctmp[si] = normp.tile([65, 1024], BF16, tag="ctmp", name=f"ctmp{si}")
                    nc.vector.tensor_copy(ctmp[si][:, 0:w], ctx[:, a:b])
                for si in range(2):
                    rc[si] = normp.tile([1, 1024], BF16, tag="rc", name=f"rc{si}")
                    with nc.allow_low_precision("softmax denom recip in bf16"):
                        nc.vector.reciprocal(rc[si][:, 0:w], ctmp[si][64:65, 0:w])
                for si in range(2):
                    rbc[si] = normp.tile([64, 1024], BF16, tag="rbc", name=f"rbc{si}")
                    nc.gpsimd.partition_broadcast(rbc[si][:, 0:w], rc[si][:, 0:w])
                for si in range(2):
                    nc.vector.tensor_mul(
                        ctx_s[64 * si : 64 * si + 64, hp,
                              1024 * qbp + a : 1024 * qbp + b],
                        ctmp[si][0:64, 0:w],
                        rbc[si][:, 0:w],
                    )

            rs_in = [dram.tile([S // 2, D], F32, name=f"rs_in{i}") for i in range(2)]
            rs_out = [dram.tile([256, D], F32, name=f"rs_out{i}") for i in range(2)]

            def emit_oproj_sl(h, sl, pool, evac, po_shape):
                st = 8 * h + sl
                po_t = pool.tile(po_shape, F32, tag="sc", name=f"po{h}_{sl}")
                three_d = len(po_shape) == 3
                pe_bias = evac == "act"
                for nb in range(2):
                    po_nb = po_t[:, nb, :] if three_d else po_t[:, 512 * nb : 512 * (nb + 1)]
                    for fc in range(2):
                        nc.tensor.matmul(
                            po_nb,
                            ctx_s[:, fc, 128 * st : 128 * (st + 1)],
                            wo_s[:, fc, 512 * nb : 512 * (nb + 1)],
                            start=(fc == 0),
                            stop=(fc == 1 and not pe_bias),
                        )
                    if pe_bias:
                        nc.tensor.matmul(
                            po_nb,
                            ones_s[0:1, 0:128],
                            bo4_s[0:1, 512 * nb : 512 * (nb + 1)],
                            start=False,
                            stop=True,
                            skip_group_check=True,
                        )
                ot = oout.tile([128, 1024], F32, tag="ot")
                po_v = po_t[:] if three_d else po_t[:].rearrange("p (n x) -> p n x", n=2)
                ot_v = ot[:].rearrange("p (n x) -> p n x", n=2)
                if evac == "act":
                    nc.scalar.activation(ot_v, po_v, Copy)
                else:
                    # fold the b_o/4 bias into the PSUM evacuation
                    nc.vector.tensor_add(
                        ot_v, po_v, bo4_s[:].rearrange("p (n x) -> p n x", n=2)
                    )
                nc.sync.dma_start(rs_in[h][128 * sl : 128 * (sl + 1), :], ot[:])

            def emit_rs(h):
                if not os_mod.environ.get("BASS_SIM_NO_RS"):
                    import concourse.mybir as mybir_mod

                    nc.gpsimd.collective_compute(
                        "ReduceScatter",
                        mybir_mod.AluOpType.add,
                        replica_groups=[[0, 1, 2, 3], [4, 5, 6, 7]],
                        ins=[rs_in[h].opt()],
                        outs=[rs_out[h].opt()],
                    )
                    nc.sync.dma_start(
                        out[256 * h : 256 * (h + 1), :], rs_out[h][:]
                    )
                else:
                    nc.sync.dma_start(
                        out[256 * h : 256 * (h + 1), :], rs_in[h][0:256, :]
                    )

            # ---- phase 1b: qbp0-hp0 scores (PE/Act filler while the xv ----
            # ---- chunks stream in) + chunk-major natural-layout V pass ----
            for ki in range(8):
                emit_scores(0, 0, ki)

            stV = ExitStack()
            psV = stV.enter_context(tc.tile_pool(name="psV", bufs=4, space="PSUM"))
            # 4 single-bank tiles per group (one st each — a PSUM bank may
            # only hold ONE accumulation group at a time); chunk-major order
            # lets group 0 ride the incoming xv chunk DMAs
            for g in range(4):
                pvt = {}
                for kc in range(8):
                    for j in range(4):
                        st = 4 * g + j
                        if kc == 0:
                            pvt[j] = psV.tile(
                                [128, 512], F32, tag="pv", name=f"pv{g}_{j}"
                            )
                        nc.tensor.matmul(
                            pvt[j][:, 0:256],
                            xv_all[:, kc, 128 * st : 128 * (st + 1)],
                            wv_s[:, kc, :],
                            start=(kc == 0),
                            stop=False,
                        )
                for j in range(4):
                    st = 4 * g + j
                    nc.tensor.matmul(
                        pvt[j][:, 0:256],
                        ones_s[0:1, 0:128],
                        bv_s[0:1, :],
                        start=False,
                        stop=True,
                        skip_group_check=True,
                    )
                    nc.vector.tensor_copy(
                        v_s[:, st, :, 0:64],
                        pvt[j][:, 0:256].rearrange("p (h x) -> p h x", h=4),
                    )
            stV.close()
            xvp_stack.close()

            psA = attn_stack.enter_context(
                tc.tile_pool(name="psA", bufs=2, space="PSUM")
            )

            # ---- stage (qbp0, hp1): scores + hp0 PVs ----
            for ki in range(8):
                emit_scores(0, 1, ki)
                emit_pv(0, 0, ki, psA)
            emit_norm(0, 0)

            # ---- stage (qbp1, hp0): scores + qbp0-hp1 PVs, then own  ----
            # ---- PVs + o-proj half0 as PE filler after norm(0,1)     ----
            # (ctx pool has 2 buffer pairs: (1,0)'s PVs may only start
            #  after norm(0,1) releases qbp0-hp1's ctx tiles)
            for ki in range(16):
                emit_scores(1, 0, ki)
                if ki < 8:
                    emit_pv(0, 1, ki, psA)
                if ki == 8:
                    emit_norm(0, 1)
                if ki >= 9:
                    emit_pv(1, 0, ki - 9, psA)
                if ki >= 8:
                    emit_oproj_sl(0, ki - 8, psS, "dve", [128, 2, 512])
            for k in range(7, 16):
                emit_pv(1, 0, k, psA)
            emit_norm(1, 0)
            emit_rs(0)

            # ---- stage (qbp1, hp1): scores + own PVs (1-ki lag) ----
            for ki in range(16):
                emit_scores(1, 1, ki)
                if ki >= 1:
                    emit_pv(1, 1, ki - 1, psA)
            emit_pv(1, 1, 15, psA)
            emit_norm(1, 1)
            attn_stack.close()

            if debug_taps:
                nc.sync.dma_start(dbg_q.ap(), qT_s[:].rearrange("p a b -> p (a b)"))
                nc.sync.dma_start(dbg_k.ap(), kT_s[:].rearrange("p a b -> p (a b)"))
                nc.sync.dma_start(dbg_v.ap(), v_s[:].rearrange("p a b c -> p (a b c)"))
                nc.sync.dma_start(dbg_c.ap(), ctx_s[:].rearrange("p a b -> p (a b)"))

            # ---- o-proj half 1 + final ReduceScatter ----
            stO = ExitStack()
            psO = stO.enter_context(tc.tile_pool(name="psO", bufs=4, space="PSUM"))
            for sl in range(8):
                emit_oproj_sl(1, sl, psO, "act" if sl % 2 else "dve", [128, 1024])
            stO.close()
            emit_rs(1)

    nc.compile()
    return nc


def _prep_inputs(query, key_, value, w_q, b_q, w_k, b_k, w_v, b_v, w_o, b_o):
    """Build the 8 per-core input maps (host-side sharding / re-layout)."""
    import ml_dtypes

    bf16 = ml_dtypes.bfloat16
    f32 = np.float32

    r = np.arange(128)[:, None, None]
    j = np.arange(128)[None, None, :]
    # allowed iff q >= k on the diagonal tile; doubled for the si-pair layout
    mtri = np.broadcast_to(j >= r, (128, 2, 128)).astype(bf16)

    wqT = np.ascontiguousarray(np.asarray(w_q, f32).T)  # [D_in, D_out]
    wkT = np.ascontiguousarray(np.asarray(w_k, f32).T)
    wvT = np.ascontiguousarray(np.asarray(w_v, f32).T)
    woT = np.ascontiguousarray(np.asarray(w_o, f32).T)

    xT = {}
    for g in range(B):
        xT[("q", g)] = np.ascontiguousarray(np.asarray(query[g], f32).T.astype(bf16))
        xT[("k", g)] = np.ascontiguousarray(np.asarray(key_[g], f32).T.astype(bf16))
        xT[("v", g)] = np.ascontiguousarray(np.asarray(value[g], f32).T.astype(bf16))

    bo4 = np.broadcast_to(
        (np.asarray(b_o, f32) / 4.0).reshape(1, D), (128, D)
    ).astype(bf16)

    in_maps = []
    for c in range(N_CORES):
        g, p = c // 4, c % 4
        fsel = slice(FPC * p, FPC * (p + 1))
        in_maps.append({
            "xq": xT[("q", g)],
            "xk": xT[("k", g)],
            "xv": xT[("v", g)],
            "wq": np.ascontiguousarray(wqT[:, fsel].astype(bf16)),
            "wk": np.ascontiguousarray(wkT[:, fsel].astype(bf16)),
            "wv": np.ascontiguousarray(wvT[:, fsel].astype(bf16)),
            "wo": np.ascontiguousarray(woT[fsel, :].astype(bf16)),
            "bq": np.ascontiguousarray(
                np.asarray(b_q, f32)[fsel].reshape(1, FPC).astype(bf16)),
            "bv": np.ascontiguousarray(
                np.asarray(b_v, f32)[fsel].reshape(1, FPC).astype(bf16)),
            "bo4": bo4,
            "mtri": mtri,
        })
    return in_maps


def run(inputs, trace=False):
    from concourse.bass_utils import run_bass_kernel_spmd

    if "nc" not in _CACHE:
        _CACHE["nc"] = _build_nc()
    nc = _CACHE["nc"]
    in_maps = _prep_inputs(
        inputs["query"], inputs["key_"], inputs["value"],
        inputs["w_q"], inputs["b_q"], inputs["w_k"], inputs["b_k"],
        inputs["w_v"], inputs["b_v"], inputs["w_o"], inputs["b_o"],
    )
    res = run_bass_kernel_spmd(
        nc, in_maps, core_ids=list(range(N_CORES)), trace=trace,
    )
    out = np.empty((B, S, D), np.float32)
    for c in range(N_CORES):
        g, p = c // 4, c % 4
        # RS half i scatters q rows [1024*i + 256*p, 1024*i + 256*(p+1))
        out[g, 256 * p : 256 * (p + 1), :] = res.results[c]["out"][0:256]
        out[g, 1024 + 256 * p : 1024 + 256 * (p + 1), :] = res.results[c]["out"][256:512]
    return out, res


def kernel(**inputs):
    out, _ = run(inputs, trace=False)
    return out


# revision 40
# speedup vs baseline: 1.4528x; 1.0559x over previous
"""Causal multi-head attention on 8 Trainium2 NeuronCores.

Sharding: core c -> (batch g = c // 4, head-group p = c % 4, heads 4p..4p+3).
Each core projects Q/K/V for its batch with its 256 feature columns
(column-sharded w_q/w_k/w_v), runs causal attention for its 4 heads, computes
the partial output projection with its 256 rows of w_o, and a ReduceScatter
over each batch group sums the partials.

All matmul operands are bf16 (fp32 PSUM accumulation).  Tricks:
  - K bias is dropped: softmax((q+bq)(k+bk)^T) == softmax((q+bq) k^T) since
    the (q+bq)*bk term is constant along the softmax (k) axis.
  - Q/V/O biases enter PSUM as rank-1 ones-row matmuls on the PE (no vector
    engine bias adds).
  - V is computed in natural [kpos, feature] layout directly (no transpose),
    with an appended ones column per head so PV accumulates the softmax
    denominators for free.
  - Scores/exp/PV touch only the causally valid column range of each k tile;
    only the [128,128] triangular diagonal block needs a mask multiply.
  - Software pipeline: the V projection pass is interleaved with the first
    head-pair's score matmuls (feeds the activation engine early); o-proj
    half 0 is interleaved into attention as PE filler so its ReduceScatter
    fully overlaps the remaining attention work.
"""

import numpy as np

B, S, D, H = 2, 2048, 1024, 16
DK = D // H  # 64
N_CORES = 8
FPC = 256  # features per core

_CACHE = {}


def _build_nc():
    import os as os_mod
    from contextlib import ExitStack

    import concourse.mybir as mybir
    import concourse.tile as tile
    from concourse import bacc

    F32 = mybir.dt.float32
    BF16 = mybir.dt.bfloat16
    Exp = mybir.ActivationFunctionType.Exp
    Copy = mybir.ActivationFunctionType.Copy

    nc = bacc.Bacc("TRN2", target_bir_lowering=False, debug=False, num_devices=8)

    xq = nc.dram_tensor("xq", [D, S], BF16, kind="ExternalInput")
    xk = nc.dram_tensor("xk", [D, S], BF16, kind="ExternalInput")
    xv = nc.dram_tensor("xv", [D, S], BF16, kind="ExternalInput")
    wq = nc.dram_tensor("wq", [D, FPC], BF16, kind="ExternalInput")
    wk = nc.dram_tensor("wk", [D, FPC], BF16, kind="ExternalInput")
    wv = nc.dram_tensor("wv", [D, FPC], BF16, kind="ExternalInput")
    wo = nc.dram_tensor("wo", [FPC, D], BF16, kind="ExternalInput")
    bq = nc.dram_tensor("bq", [1, FPC], BF16, kind="ExternalInput")
    bv = nc.dram_tensor("bv", [1, FPC], BF16, kind="ExternalInput")
    bo4 = nc.dram_tensor("bo4", [128, D], BF16, kind="ExternalInput")
    mtri = nc.dram_tensor("mtri", [128, 2, 128], BF16, kind="ExternalInput")
    out = nc.dram_tensor("out", [512, D], F32, kind="ExternalOutput")

    debug_taps = bool(os_mod.environ.get("BASS_DEBUG_TAPS"))
    if debug_taps:
        dbg_q = nc.dram_tensor("dbg_q", [128, 2 * S], BF16, kind="ExternalOutput")
        dbg_k = nc.dram_tensor("dbg_k", [128, 2 * S], BF16, kind="ExternalOutput")
        dbg_v = nc.dram_tensor("dbg_v", [128, 16 * 4 * 65], BF16, kind="ExternalOutput")
        dbg_c = nc.dram_tensor("dbg_c", [128, 2 * S], BF16, kind="ExternalOutput")

    with tile.TileContext(nc) as tc:
        with (
            tc.tile_pool(name="consts", bufs=1) as consts,
            tc.tile_pool(name="persist", bufs=1) as persist,
            tc.tile_pool(name="xin", bufs=3) as xin,
            tc.tile_pool(name="prs", bufs=20) as prs,
            tc.tile_pool(name="normp", bufs=2) as normp,
            tc.tile_pool(name="oout", bufs=3) as oout,
            tc.tile_pool(name="dram", bufs=1, space="DRAM") as dram,
        ):
            # ---- SBUF constants ----
            wq_s = consts.tile([128, 8, FPC], BF16, tag="wq")
            wk_s = consts.tile([128, 8, FPC], BF16, tag="wk")
            wv_s = consts.tile([128, 8, FPC], BF16, tag="wv")
            wo_s = consts.tile([128, 2, D], BF16, tag="wo")
            bq_s = consts.tile([1, FPC], BF16, tag="bq")
            bv_s = consts.tile([1, FPC], BF16, tag="bv")
            bo4_s = consts.tile([128, D], BF16, tag="bo4")
            mask_s = consts.tile([128, 2, 128], BF16, tag="mask")
            ones_s = consts.tile([1, 512], BF16, tag="ones")

            # ---- persistent activations ----
            # feature f of the core maps to (pt = f // 128, row = f % 128);
            # local head h lives at [64*(h%2) : 64*(h%2)+64, h//2, :]
            qT_s = persist.tile([128, 2, S], BF16, tag="qT")
            kT_s = persist.tile([128, 2, S], BF16, tag="kT")
            v_s = persist.tile([128, 16, 4, 65], BF16, tag="v")
            ctx_s = persist.tile([128, 2, S], BF16, tag="ctx")

            nc.vector.memset(ones_s[:], 1.0)
            nc.vector.memset(v_s[:, :, :, 64:65], 1.0)

            # ---- constant DMAs on the gpsimd queue ----
            # only what phase 1a needs up front; the rest is marker-gated so
            # it doesn't preempt the just-in-time xq/xk chunk streams in the
            # shared DMA pool
            gq = nc.gpsimd
            gq.dma_start(wq_s[:, 0:1, :], wq[0:128, :].rearrange(
                "(kc p) f -> p kc f", p=128))
            gq.dma_start(wq_s[:, 1:8, :], wq[128:1024, :].rearrange(
                "(kc p) f -> p kc f", p=128))
            gq.dma_start(bq_s[:], bq.ap())
            gq.dma_start(wk_s[:], wk.ap().rearrange("(kc p) f -> p kc f", p=128))

            def issue_gated_dmas(entries):
                # markers are 1-element DVE copies READING qT_s/kT_s, so they
                # get a real RAW dependency on the projection evacuations —
                # the scheduler cannot hoist these DMAs ahead of the jit
                # xq/xk chunk streams
                for marker, full, src, dep_ap in entries:
                    nc.vector.tensor_copy(marker, dep_ap)
                    gq.dma_start(full, src)

            def issue_v_const_dmas():
                issue_gated_dmas([
                    (wv_s[0:1, 0:1, 0:1], wv_s[:],
                     wv.ap().rearrange("(kc p) f -> p kc f", p=128),
                     qT_s[0:1, 0, 0:1]),
                    (bv_s[0:1, 0:1], bv_s[:], bv.ap(), qT_s[0:1, 0, 0:1]),
                    (mask_s[0:1, 0:1, 0:1], mask_s[:], mtri.ap(),
                     qT_s[0:1, 0, 0:1]),
                ])

            def issue_o_const_dmas():
                issue_gated_dmas([
                    (wo_s[0:1, 0:1, 0:1], wo_s[:],
                     wo.ap().rearrange("(fc p) d -> p fc d", p=128),
                     kT_s[0:1, 1, 1536:1537]),
                    (bo4_s[0:1, 0:1], bo4_s[:], bo4.ap(),
                     kT_s[0:1, 1, 1536:1537]),
                ])

            # xq chunk 0 split in 4 pieces so the first matmuls start early
            xt0 = xin.tile([128, S], BF16, tag="x", name="xt0")
            for qb in range(4):
                nc.sync.dma_start(
                    xt0[:, 512 * qb : 512 * (qb + 1)],
                    xq[0:128, 512 * qb : 512 * (qb + 1)],
                )

            # xv is resident for the natural-layout V pass.  Gate each chunk's
            # DMA on a DVE marker memset sequenced after the Q evacuations so
            # the xv transfers don't steal shared DMA bandwidth from the
            # just-in-time xq/xk chunk loads.
            xvp_stack = ExitStack()
            xvp = xvp_stack.enter_context(tc.tile_pool(name="xvp", bufs=1))
            xv_all = xvp.tile([128, 8, S], BF16, tag="xva")

            def issue_xv_dmas():
                # real RAW dependency on the first K evacuation (see
                # issue_late_const_dmas) keeps these out of the DMA pool
                # until the jit xq/xk chunk streams have been issued
                for kc in range(8):
                    nc.vector.tensor_copy(xv_all[0:1, kc, 0:1], qT_s[0:1, 0, 0:1])
                    gq.dma_start(
                        xv_all[:, kc, :], xv[128 * kc : 128 * (kc + 1), :]
                    )

            # ---- phase 1a: Q and K projections (transposed layout) ----
            st1 = ExitStack()
            psP = st1.enter_context(tc.tile_pool(name="psP", bufs=8, space="PSUM"))

            def proj_pass(x_dram, w_tile, b_tile, dst, first):
                ps = {}
                for kc in range(8):
                    if first and kc == 0:
                        xt = xt0
                    else:
                        xt = xin.tile([128, S], BF16, tag="x")
                        nc.sync.dma_start(xt[:], x_dram[128 * kc : 128 * (kc + 1), :])
                    for pt in range(2):
                        for qb in range(4):
                            if kc == 0:
                                ps[(pt, qb)] = psP.tile(
                                    [128, 512], F32, tag="pp", name=f"pp{pt}{qb}"
                                )
                            nc.tensor.matmul(
                                ps[(pt, qb)][:],
                                w_tile[:, kc, 128 * pt : 128 * (pt + 1)],
                                xt[:, 512 * qb : 512 * (qb + 1)],
                                start=(kc == 0),
                                stop=(kc == 7 and b_tile is None),
                            )
                for pt in range(2):
                    for qb in range(4):
                        if b_tile is not None:
                            nc.tensor.matmul(
                                ps[(pt, qb)][:],
                                b_tile[0:1, 128 * pt : 128 * (pt + 1)],
                                ones_s[0:1, 0:512],
                                start=False,
                                stop=True,
                                skip_group_check=True,
                            )
                        # split the evacuations across DVE and Act so the
                        # next pass's PSUM buffers free up twice as fast
                        dst_ap = dst[:, pt, 512 * qb : 512 * (qb + 1)]
                        if qb % 2:
                            nc.scalar.activation(dst_ap, ps[(pt, qb)][:], Copy)
                        else:
                            nc.vector.tensor_copy(dst_ap, ps[(pt, qb)][:])

            proj_pass(xq, wq_s, bq_s, qT_s, first=True)
            issue_v_const_dmas()
            issue_xv_dmas()
            proj_pass(xk, wk_s, None, kT_s, first=False)
            issue_o_const_dmas()
            st1.close()

            # ---- attention state/helpers ----
            attn_stack = ExitStack()
            psS = attn_stack.enter_context(
                tc.tile_pool(name="psS", bufs=2, space="PSUM")
            )
            pr_map = {}
            ctx_map = {}

            def segments(s0):
                if s0 < 512:
                    return [(s0, 512), (512, 1024)]
                return [(s0, 1024)]

            def emit_scores(qbp, hp, ki):
                # both si heads of the pair share si-interleaved sc/pr tiles
                # so exp and the mask multiply cover two heads per
                # instruction; one sc tile per 512-column segment keeps the
                # PSUM footprint at 2 banks so bufs=2 still pipelines
                s0 = max(0, 128 * ki - 1024 * qbp)
                pr = prs.tile([128, 2, 1024], BF16, tag="pr", name=f"pr{qbp}{hp}_{ki}")
                for a, b in segments(s0):
                    sc = psS.tile(
                        [128, 2, 512], F32, tag="sc", name=f"sc{qbp}{hp}_{ki}_{a}"
                    )
                    for si in range(2):
                        nc.tensor.matmul(
                            sc[:, si, 0 : b - a],
                            kT_s[64 * si : 64 * si + 64, hp,
                                 128 * ki : 128 * (ki + 1)],
                            qT_s[64 * si : 64 * si + 64, hp,
                                 1024 * qbp + a : 1024 * qbp + b],
                            start=True,
                            stop=True,
                        )
                    nc.scalar.activation(
                        out=pr[:, :, a:b], in_=sc[:, :, 0 : b - a],
                        func=Exp, scale=0.125,
                    )
                if 128 * ki >= 1024 * qbp:  # diagonal tile inside this window
                    nc.vector.tensor_mul(
                        pr[:, :, s0 : s0 + 128], pr[:, :, s0 : s0 + 128], mask_s[:]
                    )
                pr_map[(qbp, hp, ki)] = (pr, s0)

            def emit_pv(qbp, hp, ki, psA):
                nkt = 8 * (qbp + 1)
                pr, s0 = pr_map.pop((qbp, hp, ki))
                last_a = (512 + 1024 * qbp) // 128 - 1
                for si in range(2):
                    key = (qbp, hp, si)
                    if key not in ctx_map:
                        ctx_map[key] = psA.tile(
                            [65, 1024], F32, tag="ctx", name=f"ctx{qbp}{hp}{si}"
                        )
                    ctx = ctx_map[key]
                    for a, b in segments(s0):
                        last = last_a if b == 512 else nkt - 1
                        nc.tensor.matmul(
                            ctx[:, a:b],
                            v_s[:, ki, 2 * hp + si, :],
                            pr[:, si, a:b],
                            start=(ki == 0),
                            stop=(ki == last),
                            skip_group_check=True,
                        )

            def emit_norm(qbp, hp, cols=(0, 1024), release=True):
                a, b = cols
                w = b - a
                ctmp, rc, rbc = {}, {}, {}
                for si in range(2):
                    ctx = ctx_map[(qbp, hp, si)]
                    if release and b == 1024:
                        ctx_map.pop((qbp, hp, si))
                    ctmp[si] = normp.tile([65, 1024], BF16, tag="ctmp", name=f"ctmp{si}")
                    nc.vector.tensor_copy(ctmp[si][:, 0:w], ctx[:, a:b])
                for si in range(2):
                    rc[si] = normp.tile([1, 1024], BF16, tag="rc", name=f"rc{si}")
                    with nc.allow_low_precision("softmax denom recip in bf16"):
                        nc.vector.reciprocal(rc[si][:, 0:w], ctmp[si][64:65, 0:w])
                for si in range(2):
                    rbc[si] = normp.tile([64, 1024], BF16, tag="rbc", name=f"rbc{si}")
                    nc.gpsimd.partition_broadcast(rbc[si][:, 0:w], rc[si][:, 0:w])
                for si in range(2):
                    nc.vector.tensor_mul(
                        ctx_s[64 * si : 64 * si + 64, hp,
                              1024 * qbp + a : 1024 * qbp + b],
                        ctmp[si][0:64, 0:w],
                        rbc[si][:, 0:w],
                    )

            rs_in = [dram.tile([S // 2, D], F32, name=f"rs_in{i}") for i in range(2)]
            rs_out = [dram.tile([256, D], F32, name=f"rs_out{i}") for i in range(2)]

            def emit_oproj_sl(h, sl, pool, evac, po_shape):
                st = 8 * h + sl
                po_t = pool.tile(po_shape, F32, tag="sc", name=f"po{h}_{sl}")
                three_d = len(po_shape) == 3
                pe_bias = evac == "act"
                for nb in range(2):
                    po_nb = po_t[:, nb, :] if three_d else po_t[:, 512 * nb : 512 * (nb + 1)]
                    for fc in range(2):
                        nc.tensor.matmul(
                            po_nb,
                            ctx_s[:, fc, 128 * st : 128 * (st + 1)],
                            wo_s[:, fc, 512 * nb : 512 * (nb + 1)],
                            start=(fc == 0),
                            stop=(fc == 1 and not pe_bias),
                        )
                    if pe_bias:
                        nc.tensor.matmul(
                            po_nb,
                            ones_s[0:1, 0:128],
                            bo4_s[0:1, 512 * nb : 512 * (nb + 1)],
                            start=False,
                            stop=True,
                            skip_group_check=True,
                        )
                ot = oout.tile([128, 1024], F32, tag="ot")
                po_v = po_t[:] if three_d else po_t[:].rearrange("p (n x) -> p n x", n=2)
                ot_v = ot[:].rearrange("p (n x) -> p n x", n=2)
                if evac == "act":
                    nc.scalar.activation(ot_v, po_v, Copy)
                else:
                    # fold the b_o/4 bias into the PSUM evacuation
                    nc.vector.tensor_add(
                        ot_v, po_v, bo4_s[:].rearrange("p (n x) -> p n x", n=2)
                    )
                nc.sync.dma_start(rs_in[h][128 * sl : 128 * (sl + 1), :], ot[:])

            def emit_rs(h):
                if not os_mod.environ.get("BASS_SIM_NO_RS"):
                    import concourse.mybir as mybir_mod

                    nc.gpsimd.collective_compute(
                        "ReduceScatter",
                        mybir_mod.AluOpType.add,
                        replica_groups=[[0, 1, 2, 3], [4, 5, 6, 7]],
                        ins=[rs_in[h].opt()],
                        outs=[rs_out[h].opt()],
                    )
                    nc.sync.dma_start(
                        out[256 * h : 256 * (h + 1), :], rs_out[h][:]
                    )
                else:
                    nc.sync.dma_start(
                        out[256 * h : 256 * (h + 1), :], rs_in[h][0:256, :]
                    )

            # ---- phase 1b: qbp0-hp0 scores (PE/Act filler while the xv ----
            # ---- chunks stream in) + chunk-major natural-layout V pass ----
            for ki in range(8):
                emit_scores(0, 0, ki)

            stV = ExitStack()
            psV = stV.enter_context(tc.tile_pool(name="psV", bufs=4, space="PSUM"))
            # 4 single-bank tiles per group (one st each — a PSUM bank may
            # only hold ONE accumulation group at a time); chunk-major order
            # lets group 0 ride the incoming xv chunk DMAs
            for g in range(4):
                pvt = {}
                for kc in range(8):
                    for j in range(4):
                        st = 4 * g + j
                        if kc == 0:
                            pvt[j] = psV.tile(
                                [128, 512], F32, tag="pv", name=f"pv{g}_{j}"
                            )
                        nc.tensor.matmul(
                            pvt[j][:, 0:256],
                            xv_all[:, kc, 128 * st : 128 * (st + 1)],
                            wv_s[:, kc, :],
                            start=(kc == 0),
                            stop=False,
                        )
                for j in range(4):
                    st = 4 * g + j
                    nc.tensor.matmul(
                        pvt[j][:, 0:256],
                        ones_s[0:1, 0:128],
                        bv_s[0:1, :],
                        start=False,
                        stop=True,
                        skip_group_check=True,
                    )
                    nc.vector.tensor_copy(
                        v_s[:, st, :, 0:64],
                        pvt[j][:, 0:256].rearrange("p (h x) -> p h x", h=4),
                    )
            stV.close()
            xvp_stack.close()

            psA = attn_stack.enter_context(
                tc.tile_pool(name="psA", bufs=2, space="PSUM")
            )

            # ---- stage (qbp0, hp1): scores + hp0 PVs ----
            for ki in range(8):
                emit_scores(0, 1, ki)
                emit_pv(0, 0, ki, psA)
            emit_norm(0, 0)

            # ---- stage (qbp1, hp0): scores + qbp0-hp1 PVs, then own  ----
            # ---- PVs + o-proj half0 as PE filler after norm(0,1)     ----
            # (ctx pool has 2 buffer pairs: (1,0)'s PVs may only start
            #  after norm(0,1) releases qbp0-hp1's ctx tiles)
            for ki in range(16):
                emit_scores(1, 0, ki)
                if ki < 8:
                    emit_pv(0, 1, ki, psA)
                if ki == 8:
                    emit_norm(0, 1)
                if ki >= 9:
                    emit_pv(1, 0, ki - 9, psA)
                if ki >= 8:
                    emit_oproj_sl(0, ki - 8, psS, "dve", [128, 2, 512])
            for k in range(7, 16):
                emit_pv(1, 0, k, psA)
            emit_norm(1, 0)
            emit_rs(0)

            # ---- stage (qbp1, hp1): scores + own PVs (1-ki lag); ctx ----
            # ---- cols [0:512) finish at ki=11, so their norm + the   ----
            # ---- first o-proj half1 slices interleave into the tail  ----
            for ki in range(16):
                emit_scores(1, 1, ki)
                if ki >= 1:
                    emit_pv(1, 1, ki - 1, psA)
                if ki == 13:
                    emit_norm(1, 1, cols=(0, 512), release=False)
                if ki >= 14:
                    emit_oproj_sl(1, ki - 14, psS, "dve", [128, 2, 512])
            emit_pv(1, 1, 15, psA)
            emit_norm(1, 1, cols=(512, 1024))
            emit_oproj_sl(1, 2, psS, "dve", [128, 2, 512])
            emit_oproj_sl(1, 3, psS, "act", [128, 2, 512])
            attn_stack.close()

            if debug_taps:
                nc.sync.dma_start(dbg_q.ap(), qT_s[:].rearrange("p a b -> p (a b)"))
                nc.sync.dma_start(dbg_k.ap(), kT_s[:].rearrange("p a b -> p (a b)"))
                nc.sync.dma_start(dbg_v.ap(), v_s[:].rearrange("p a b c -> p (a b c)"))
                nc.sync.dma_start(dbg_c.ap(), ctx_s[:].rearrange("p a b -> p (a b)"))

            # ---- o-proj half 1 + final ReduceScatter ----
            stO = ExitStack()
            psO = stO.enter_context(tc.tile_pool(name="psO", bufs=4, space="PSUM"))
            for sl in range(4, 8):
                emit_oproj_sl(1, sl, psO, "act" if sl % 2 else "dve", [128, 1024])
            stO.close()
            emit_rs(1)

    nc.compile()
    return nc


def _prep_inputs(query, key_, value, w_q, b_q, w_k, b_k, w_v, b_v, w_o, b_o):
    """Build the 8 per-core input maps (host-side sharding / re-layout)."""
    import ml_dtypes

    bf16 = ml_dtypes.bfloat16
    f32 = np.float32

    r = np.arange(128)[:, None, None]
    j = np.arange(128)[None, None, :]
    # allowed iff q >= k on the diagonal tile; doubled for the si-pair layout
    mtri = np.broadcast_to(j >= r, (128, 2, 128)).astype(bf16)

    wqT = np.ascontiguousarray(np.asarray(w_q, f32).T)  # [D_in, D_out]
    wkT = np.ascontiguousarray(np.asarray(w_k, f32).T)
    wvT = np.ascontiguousarray(np.asarray(w_v, f32).T)
    woT = np.ascontiguousarray(np.asarray(w_o, f32).T)

    xT = {}
    for g in range(B):
        xT[("q", g)] = np.ascontiguousarray(np.asarray(query[g], f32).T.astype(bf16))
        xT[("k", g)] = np.ascontiguousarray(np.asarray(key_[g], f32).T.astype(bf16))
        xT[("v", g)] = np.ascontiguousarray(np.asarray(value[g], f32).T.astype(bf16))

    bo4 = np.broadcast_to(
        (np.asarray(b_o, f32) / 4.0).reshape(1, D), (128, D)
    ).astype(bf16)

    in_maps = []
    for c in range(N_CORES):
        g, p = c // 4, c % 4
        fsel = slice(FPC * p, FPC * (p + 1))
        in_maps.append({
            "xq": xT[("q", g)],
            "xk": xT[("k", g)],
            "xv": xT[("v", g)],
            "wq": np.ascontiguousarray(wqT[:, fsel].astype(bf16)),
            "wk": np.ascontiguousarray(wkT[:, fsel].astype(bf16)),
            "wv": np.ascontiguousarray(wvT[:, fsel].astype(bf16)),
            "wo": np.ascontiguousarray(woT[fsel, :].astype(bf16)),
            "bq": np.ascontiguousarray(
                np.asarray(b_q, f32)[fsel].reshape(1, FPC).astype(bf16)),
            "bv": np.ascontiguousarray(
                np.asarray(b_v, f32)[fsel].reshape(1, FPC).astype(bf16)),
            "bo4": bo4,
            "mtri": mtri,
        })
    return in_maps


def run(inputs, trace=False):
    from concourse.bass_utils import run_bass_kernel_spmd

    if "nc" not in _CACHE:
        _CACHE["nc"] = _build_nc()
    nc = _CACHE["nc"]
    in_maps = _prep_inputs(
        inputs["query"], inputs["key_"], inputs["value"],
        inputs["w_q"], inputs["b_q"], inputs["w_k"], inputs["b_k"],
        inputs["w_v"], inputs["b_v"], inputs["w_o"], inputs["b_o"],
    )
    res = run_bass_kernel_spmd(
        nc, in_maps, core_ids=list(range(N_CORES)), trace=trace,
    )
    out = np.empty((B, S, D), np.float32)
    for c in range(N_CORES):
        g, p = c // 4, c % 4
        # RS half i scatters q rows [1024*i + 256*p, 1024*i + 256*(p+1))
        out[g, 256 * p : 256 * (p + 1), :] = res.results[c]["out"][0:256]
        out[g, 1024 + 256 * p : 1024 + 256 * (p + 1), :] = res.results[c]["out"][256:512]
    return out, res


def kernel(**inputs):
    out, _ = run(inputs, trace=False)
    return out


# revision 42
# speedup vs baseline: 1.5549x; 1.0703x over previous
"""Causal multi-head attention on 8 Trainium2 NeuronCores.

Sharding: core c -> (batch g = c // 4, head-group p = c % 4, heads 4p..4p+3).
Each core projects Q/K/V for its batch with its 256 feature columns
(column-sharded w_q/w_k/w_v), runs causal attention for its 4 heads, computes
the partial output projection with its 256 rows of w_o, and a ReduceScatter
over each batch group sums the partials.

All matmul operands are bf16 (fp32 PSUM accumulation).  Tricks:
  - K bias is dropped: softmax((q+bq)(k+bk)^T) == softmax((q+bq) k^T) since
    the (q+bq)*bk term is constant along the softmax (k) axis.
  - Q/V/O biases enter PSUM as rank-1 ones-row matmuls on the PE (no vector
    engine bias adds).
  - V is computed in natural [kpos, feature] layout directly (no transpose),
    with an appended ones column per head so PV accumulates the softmax
    denominators for free.
  - Scores/exp/PV touch only the causally valid column range of each k tile;
    only the [128,128] triangular diagonal block needs a mask multiply.
  - Software pipeline: the V projection pass is interleaved with the first
    head-pair's score matmuls (feeds the activation engine early); o-proj
    half 0 is interleaved into attention as PE filler so its ReduceScatter
    fully overlaps the remaining attention work.
"""

import numpy as np

B, S, D, H = 2, 2048, 1024, 16
DK = D // H  # 64
N_CORES = 8
FPC = 256  # features per core

_CACHE = {}


def _build_nc():
    import os as os_mod
    from contextlib import ExitStack

    import concourse.mybir as mybir
    import concourse.tile as tile
    from concourse import bacc

    F32 = mybir.dt.float32
    BF16 = mybir.dt.bfloat16
    Exp = mybir.ActivationFunctionType.Exp
    Copy = mybir.ActivationFunctionType.Copy

    nc = bacc.Bacc("TRN2", target_bir_lowering=False, debug=False, num_devices=8)

    xq = nc.dram_tensor("xq", [D, S], BF16, kind="ExternalInput")
    xk = nc.dram_tensor("xk", [D, S], BF16, kind="ExternalInput")
    xv = nc.dram_tensor("xv", [D, S], BF16, kind="ExternalInput")
    wq = nc.dram_tensor("wq", [D, FPC], BF16, kind="ExternalInput")
    wk = nc.dram_tensor("wk", [D, FPC], BF16, kind="ExternalInput")
    wv = nc.dram_tensor("wv", [D, FPC], BF16, kind="ExternalInput")
    wo = nc.dram_tensor("wo", [FPC, D], BF16, kind="ExternalInput")
    bq = nc.dram_tensor("bq", [1, FPC], BF16, kind="ExternalInput")
    bv = nc.dram_tensor("bv", [1, FPC], BF16, kind="ExternalInput")
    bo4 = nc.dram_tensor("bo4", [128, D], BF16, kind="ExternalInput")
    mtri = nc.dram_tensor("mtri", [128, 2, 128], BF16, kind="ExternalInput")
    out = nc.dram_tensor("out", [512, D], F32, kind="ExternalOutput")

    debug_taps = bool(os_mod.environ.get("BASS_DEBUG_TAPS"))
    if debug_taps:
        dbg_q = nc.dram_tensor("dbg_q", [128, 2 * S], BF16, kind="ExternalOutput")
        dbg_k = nc.dram_tensor("dbg_k", [128, 2 * S], BF16, kind="ExternalOutput")
        dbg_v = nc.dram_tensor("dbg_v", [128, 16 * 4 * 65], BF16, kind="ExternalOutput")
        dbg_c = nc.dram_tensor("dbg_c", [128, 2 * S], BF16, kind="ExternalOutput")

    with tile.TileContext(nc) as tc:
        with (
            tc.tile_pool(name="consts", bufs=1) as consts,
            tc.tile_pool(name="persist", bufs=1) as persist,
            tc.tile_pool(name="xin", bufs=3) as xin,
            tc.tile_pool(name="prs", bufs=20) as prs,
            tc.tile_pool(name="normp", bufs=2) as normp,
            tc.tile_pool(name="oout", bufs=3) as oout,
            tc.tile_pool(name="dram", bufs=1, space="DRAM") as dram,
        ):
            # ---- SBUF constants ----
            wq_s = consts.tile([128, 8, FPC], BF16, tag="wq")
            wk_s = consts.tile([128, 8, FPC], BF16, tag="wk")
            wv_s = consts.tile([128, 8, FPC], BF16, tag="wv")
            wo_s = consts.tile([128, 2, D], BF16, tag="wo")
            bq_s = consts.tile([1, FPC], BF16, tag="bq")
            bv_s = consts.tile([1, FPC], BF16, tag="bv")
            bo4_s = consts.tile([128, D], BF16, tag="bo4")
            mask_s = consts.tile([128, 2, 128], BF16, tag="mask")
            ones_s = consts.tile([1, 512], BF16, tag="ones")

            # ---- persistent activations ----
            # feature f of the core maps to (pt = f // 128, row = f % 128);
            # local head h lives at [64*(h%2) : 64*(h%2)+64, h//2, :]
            qT_s = persist.tile([128, 2, S], BF16, tag="qT")
            kT_s = persist.tile([128, 2, S], BF16, tag="kT")
            v_s = persist.tile([128, 16, 4, 65], BF16, tag="v")
            ctx_s = persist.tile([128, 2, S], BF16, tag="ctx")

            nc.vector.memset(ones_s[:], 1.0)
            nc.vector.memset(v_s[:, :, :, 64:65], 1.0)

            # ---- constant DMAs on the gpsimd queue ----
            # only what phase 1a needs up front; the rest is marker-gated so
            # it doesn't preempt the just-in-time xq/xk chunk streams in the
            # shared DMA pool
            gq = nc.gpsimd
            gq.dma_start(wq_s[:, 0:1, :], wq[0:128, :].rearrange(
                "(kc p) f -> p kc f", p=128))
            gq.dma_start(wq_s[:, 1:8, :], wq[128:1024, :].rearrange(
                "(kc p) f -> p kc f", p=128))
            gq.dma_start(bq_s[:], bq.ap())
            gq.dma_start(wk_s[:], wk.ap().rearrange("(kc p) f -> p kc f", p=128))

            def issue_gated_dmas(entries):
                # markers are 1-element DVE copies READING qT_s/kT_s, so they
                # get a real RAW dependency on the projection evacuations —
                # the scheduler cannot hoist these DMAs ahead of the jit
                # xq/xk chunk streams
                for marker, full, src, dep_ap in entries:
                    nc.vector.tensor_copy(marker, dep_ap)
                    gq.dma_start(full, src)

            def issue_v_const_dmas(dep_ap):
                issue_gated_dmas([
                    (wv_s[0:1, 0:1, 0:1], wv_s[:],
                     wv.ap().rearrange("(kc p) f -> p kc f", p=128), dep_ap),
                    (bv_s[0:1, 0:1], bv_s[:], bv.ap(), dep_ap),
                    (mask_s[0:1, 0:1, 0:1], mask_s[:], mtri.ap(), dep_ap),
                ])

            def issue_o_const_dmas():
                issue_gated_dmas([
                    (wo_s[0:1, 0:1, 0:1], wo_s[:],
                     wo.ap().rearrange("(fc p) d -> p fc d", p=128),
                     kT_s[0:1, 1, 1536:1537]),
                    (bo4_s[0:1, 0:1], bo4_s[:], bo4.ap(),
                     kT_s[0:1, 1, 1536:1537]),
                ])

            # xq chunk 0 split in 4 pieces so the first matmuls start early
            xt0 = xin.tile([128, S], BF16, tag="x", name="xt0")
            for qb in range(4):
                nc.sync.dma_start(
                    xt0[:, 512 * qb : 512 * (qb + 1)],
                    xq[0:128, 512 * qb : 512 * (qb + 1)],
                )

            # xv is resident for the natural-layout V pass.  Gate each chunk's
            # DMA on a DVE marker memset sequenced after the Q evacuations so
            # the xv transfers don't steal shared DMA bandwidth from the
            # just-in-time xq/xk chunk loads.
            xvp_stack = ExitStack()
            xvp = xvp_stack.enter_context(tc.tile_pool(name="xvp", bufs=1))
            xv_all = xvp.tile([128, 8, S], BF16, tag="xva")

            def issue_xv_dmas(dep_ap):
                # real RAW dependency on a late xk chunk landing keeps these
                # behind the jit xk stream in the FIFO transfer pool
                for kc in range(8):
                    nc.vector.tensor_copy(xv_all[0:1, kc, 0:1], dep_ap)
                    gq.dma_start(
                        xv_all[:, kc, :], xv[128 * kc : 128 * (kc + 1), :]
                    )

            # ---- phase 1a: Q and K projections (transposed layout) ----
            st1 = ExitStack()
            psP = st1.enter_context(tc.tile_pool(name="psP", bufs=8, space="PSUM"))

            def proj_pass(x_dram, w_tile, b_tile, dst, first):
                ps = {}
                xts = {}
                for kc in range(8):
                    if first and kc == 0:
                        xt = xt0
                    else:
                        xt = xin.tile([128, S], BF16, tag="x")
                        nc.sync.dma_start(xt[:], x_dram[128 * kc : 128 * (kc + 1), :])
                    xts[kc] = xt
                    for pt in range(2):
                        for qb in range(4):
                            if kc == 0:
                                ps[(pt, qb)] = psP.tile(
                                    [128, 512], F32, tag="pp", name=f"pp{pt}{qb}"
                                )
                            nc.tensor.matmul(
                                ps[(pt, qb)][:],
                                w_tile[:, kc, 128 * pt : 128 * (pt + 1)],
                                xt[:, 512 * qb : 512 * (qb + 1)],
                                start=(kc == 0),
                                stop=(kc == 7 and b_tile is None),
                            )
                for pt in range(2):
                    for qb in range(4):
                        if b_tile is not None:
                            nc.tensor.matmul(
                                ps[(pt, qb)][:],
                                b_tile[0:1, 128 * pt : 128 * (pt + 1)],
                                ones_s[0:1, 0:512],
                                start=False,
                                stop=True,
                                skip_group_check=True,
                            )
                        # split the evacuations across DVE and Act so the
                        # next pass's PSUM buffers free up twice as fast
                        dst_ap = dst[:, pt, 512 * qb : 512 * (qb + 1)]
                        if qb % 2:
                            nc.scalar.activation(dst_ap, ps[(pt, qb)][:], Copy)
                        else:
                            nc.vector.tensor_copy(dst_ap, ps[(pt, qb)][:])
                return xts

            proj_pass(xq, wq_s, bq_s, qT_s, first=True)
            xk_ts = proj_pass(xk, wk_s, None, kT_s, first=False)
            issue_v_const_dmas(xk_ts[5][0:1, 0:1])
            issue_xv_dmas(xk_ts[7][0:1, 0:1])
            issue_o_const_dmas()
            st1.close()

            # ---- attention state/helpers ----
            attn_stack = ExitStack()
            psS = attn_stack.enter_context(
                tc.tile_pool(name="psS", bufs=2, space="PSUM")
            )
            pr_map = {}
            ctx_map = {}

            def segments(s0):
                if s0 < 512:
                    return [(s0, 512), (512, 1024)]
                return [(s0, 1024)]

            def emit_scores(qbp, hp, ki):
                # both si heads of the pair share si-interleaved sc/pr tiles
                # so exp and the mask multiply cover two heads per
                # instruction; one sc tile per 512-column segment keeps the
                # PSUM footprint at 2 banks so bufs=2 still pipelines
                s0 = max(0, 128 * ki - 1024 * qbp)
                pr = prs.tile([128, 2, 1024], BF16, tag="pr", name=f"pr{qbp}{hp}_{ki}")
                for a, b in segments(s0):
                    sc = psS.tile(
                        [128, 2, 512], F32, tag="sc", name=f"sc{qbp}{hp}_{ki}_{a}"
                    )
                    for si in range(2):
                        nc.tensor.matmul(
                            sc[:, si, 0 : b - a],
                            kT_s[64 * si : 64 * si + 64, hp,
                                 128 * ki : 128 * (ki + 1)],
                            qT_s[64 * si : 64 * si + 64, hp,
                                 1024 * qbp + a : 1024 * qbp + b],
                            start=True,
                            stop=True,
                        )
                    nc.scalar.activation(
                        out=pr[:, :, a:b], in_=sc[:, :, 0 : b - a],
                        func=Exp, scale=0.125,
                    )
                if 128 * ki >= 1024 * qbp:  # diagonal tile inside this window
                    nc.vector.tensor_mul(
                        pr[:, :, s0 : s0 + 128], pr[:, :, s0 : s0 + 128], mask_s[:]
                    )
                pr_map[(qbp, hp, ki)] = (pr, s0)

            def emit_pv(qbp, hp, ki, psA):
                nkt = 8 * (qbp + 1)
                pr, s0 = pr_map.pop((qbp, hp, ki))
                last_a = (512 + 1024 * qbp) // 128 - 1
                for si in range(2):
                    key = (qbp, hp, si)
                    if key not in ctx_map:
                        ctx_map[key] = psA.tile(
                            [65, 1024], F32, tag="ctx", name=f"ctx{qbp}{hp}{si}"
                        )
                    ctx = ctx_map[key]
                    for a, b in segments(s0):
                        last = last_a if b == 512 else nkt - 1
                        nc.tensor.matmul(
                            ctx[:, a:b],
                            v_s[:, ki, 2 * hp + si, :],
                            pr[:, si, a:b],
                            start=(ki == 0),
                            stop=(ki == last),
                            skip_group_check=True,
                        )

            def emit_norm(qbp, hp, cols=(0, 1024), release=True):
                a, b = cols
                w = b - a
                ctmp, rc, rbc = {}, {}, {}
                for si in range(2):
                    ctx = ctx_map[(qbp, hp, si)]
                    if release and b == 1024:
                        ctx_map.pop((qbp, hp, si))
                    ctmp[si] = normp.tile([65, 1024], BF16, tag="ctmp", name=f"ctmp{si}")
                    nc.vector.tensor_copy(ctmp[si][:, 0:w], ctx[:, a:b])
                for si in range(2):
                    rc[si] = normp.tile([1, 1024], BF16, tag="rc", name=f"rc{si}")
                    with nc.allow_low_precision("softmax denom recip in bf16"):
                        nc.vector.reciprocal(rc[si][:, 0:w], ctmp[si][64:65, 0:w])
                for si in range(2):
                    rbc[si] = normp.tile([64, 1024], BF16, tag="rbc", name=f"rbc{si}")
                    nc.gpsimd.partition_broadcast(rbc[si][:, 0:w], rc[si][:, 0:w])
                for si in range(2):
                    nc.vector.tensor_mul(
                        ctx_s[64 * si : 64 * si + 64, hp,
                              1024 * qbp + a : 1024 * qbp + b],
                        ctmp[si][0:64, 0:w],
                        rbc[si][:, 0:w],
                    )

            rs_in = [dram.tile([S // 2, D], F32, name=f"rs_in{i}") for i in range(2)]
            rs_out = [dram.tile([256, D], F32, name=f"rs_out{i}") for i in range(2)]

            def emit_oproj_sl(h, sl, pool, evac, po_shape):
                st = 8 * h + sl
                po_t = pool.tile(po_shape, F32, tag="sc", name=f"po{h}_{sl}")
                three_d = len(po_shape) == 3
                pe_bias = evac == "act"
                for nb in range(2):
                    po_nb = po_t[:, nb, :] if three_d else po_t[:, 512 * nb : 512 * (nb + 1)]
                    for fc in range(2):
                        nc.tensor.matmul(
                            po_nb,
                            ctx_s[:, fc, 128 * st : 128 * (st + 1)],
                            wo_s[:, fc, 512 * nb : 512 * (nb + 1)],
                            start=(fc == 0),
                            stop=(fc == 1 and not pe_bias),
                        )
                    if pe_bias:
                        nc.tensor.matmul(
                            po_nb,
                            ones_s[0:1, 0:128],
                            bo4_s[0:1, 512 * nb : 512 * (nb + 1)],
                            start=False,
                            stop=True,
                            skip_group_check=True,
                        )
                ot = oout.tile([128, 1024], F32, tag="ot")
                po_v = po_t[:] if three_d else po_t[:].rearrange("p (n x) -> p n x", n=2)
                ot_v = ot[:].rearrange("p (n x) -> p n x", n=2)
                if evac == "act":
                    nc.scalar.activation(ot_v, po_v, Copy)
                else:
                    # fold the b_o/4 bias into the PSUM evacuation
                    nc.vector.tensor_add(
                        ot_v, po_v, bo4_s[:].rearrange("p (n x) -> p n x", n=2)
                    )
                nc.sync.dma_start(rs_in[h][128 * sl : 128 * (sl + 1), :], ot[:])

            def emit_rs(h):
                if not os_mod.environ.get("BASS_SIM_NO_RS"):
                    import concourse.mybir as mybir_mod

                    nc.gpsimd.collective_compute(
                        "ReduceScatter",
                        mybir_mod.AluOpType.add,
                        replica_groups=[[0, 1, 2, 3], [4, 5, 6, 7]],
                        ins=[rs_in[h].opt()],
                        outs=[rs_out[h].opt()],
                    )
                    nc.sync.dma_start(
                        out[256 * h : 256 * (h + 1), :], rs_out[h][:]
                    )
                else:
                    nc.sync.dma_start(
                        out[256 * h : 256 * (h + 1), :], rs_in[h][0:256, :]
                    )

            # ---- phase 1b: qbp0-hp0 scores (PE/Act filler while the xv ----
            # ---- chunks stream in) + chunk-major natural-layout V pass ----
            for ki in range(8):
                emit_scores(0, 0, ki)
            for ki in range(8):
                emit_scores(0, 1, ki)

            stV = ExitStack()
            psV = stV.enter_context(tc.tile_pool(name="psV", bufs=4, space="PSUM"))
            # 4 single-bank tiles per group (one st each — a PSUM bank may
            # only hold ONE accumulation group at a time); chunk-major order
            # lets group 0 ride the incoming xv chunk DMAs
            for g in range(4):
                pvt = {}
                for kc in range(8):
                    for j in range(4):
                        st = 4 * g + j
                        if kc == 0:
                            pvt[j] = psV.tile(
                                [128, 512], F32, tag="pv", name=f"pv{g}_{j}"
                            )
                        nc.tensor.matmul(
                            pvt[j][:, 0:256],
                            xv_all[:, kc, 128 * st : 128 * (st + 1)],
                            wv_s[:, kc, :],
                            start=(kc == 0),
                            stop=False,
                        )
                for j in range(4):
                    st = 4 * g + j
                    nc.tensor.matmul(
                        pvt[j][:, 0:256],
                        ones_s[0:1, 0:128],
                        bv_s[0:1, :],
                        start=False,
                        stop=True,
                        skip_group_check=True,
                    )
                    nc.vector.tensor_copy(
                        v_s[:, st, :, 0:64],
                        pvt[j][:, 0:256].rearrange("p (h x) -> p h x", h=4),
                    )
            stV.close()
            xvp_stack.close()

            psA = attn_stack.enter_context(
                tc.tile_pool(name="psA", bufs=2, space="PSUM")
            )

            # ---- stage (qbp0, hp1): hp0 + hp1 PVs (scores pre-emitted) ----
            for ki in range(8):
                emit_pv(0, 0, ki, psA)
            emit_norm(0, 0)

            # ---- stage (qbp1, hp0): scores + qbp0-hp1 PVs, then own  ----
            # ---- PVs + o-proj half0 as PE filler after norm(0,1)     ----
            # (ctx pool has 2 buffer pairs: (1,0)'s PVs may only start
            #  after norm(0,1) releases qbp0-hp1's ctx tiles)
            for ki in range(16):
                emit_scores(1, 0, ki)
                if ki < 8:
                    emit_pv(0, 1, ki, psA)
                if ki == 8:
                    emit_norm(0, 1)
                if ki >= 9:
                    emit_pv(1, 0, ki - 9, psA)
                if ki >= 8 and ki % 2 == 0:
                    emit_oproj_sl(0, (ki - 8) // 2, psS, "dve", [128, 2, 512])
            for k in range(7, 16):
                emit_pv(1, 0, k, psA)
            emit_norm(1, 0)
            emit_rs(0)

            # ---- stage (qbp1, hp1): scores + own PVs (1-ki lag); ctx ----
            # ---- cols [0:512) finish at ki=11, so their norm + the   ----
            # ---- first o-proj half1 slices interleave into the tail  ----
            for ki in range(16):
                emit_scores(1, 1, ki)
                if ki >= 1:
                    emit_pv(1, 1, ki - 1, psA)
                if ki < 8 and ki % 2 == 0:
                    emit_oproj_sl(0, 4 + ki // 2, psS, "dve", [128, 2, 512])
                if ki == 13:
                    emit_norm(1, 1, cols=(0, 512), release=False)
                if ki >= 14:
                    emit_oproj_sl(1, ki - 14, psS, "dve", [128, 2, 512])
            emit_pv(1, 1, 15, psA)
            emit_norm(1, 1, cols=(512, 1024))
            emit_oproj_sl(1, 2, psS, "dve", [128, 2, 512])
            emit_oproj_sl(1, 3, psS, "act", [128, 2, 512])
            attn_stack.close()

            if debug_taps:
                nc.sync.dma_start(dbg_q.ap(), qT_s[:].rearrange("p a b -> p (a b)"))
                nc.sync.dma_start(dbg_k.ap(), kT_s[:].rearrange("p a b -> p (a b)"))
                nc.sync.dma_start(dbg_v.ap(), v_s[:].rearrange("p a b c -> p (a b c)"))
                nc.sync.dma_start(dbg_c.ap(), ctx_s[:].rearrange("p a b -> p (a b)"))

            # ---- o-proj half 1 + final ReduceScatter ----
            stO = ExitStack()
            psO = stO.enter_context(tc.tile_pool(name="psO", bufs=4, space="PSUM"))
            for sl in range(4, 8):
                emit_oproj_sl(1, sl, psO, "act" if sl % 2 else "dve", [128, 1024])
            stO.close()
            emit_rs(1)

    nc.compile()
    return nc


def _prep_inputs(query, key_, value, w_q, b_q, w_k, b_k, w_v, b_v, w_o, b_o):
    """Build the 8 per-core input maps (host-side sharding / re-layout)."""
    import ml_dtypes

    bf16 = ml_dtypes.bfloat16
    f32 = np.float32

    r = np.arange(128)[:, None, None]
    j = np.arange(128)[None, None, :]
    # allowed iff q >= k on the diagonal tile; doubled for the si-pair layout
    mtri = np.broadcast_to(j >= r, (128, 2, 128)).astype(bf16)

    wqT = np.ascontiguousarray(np.asarray(w_q, f32).T)  # [D_in, D_out]
    wkT = np.ascontiguousarray(np.asarray(w_k, f32).T)
    wvT = np.ascontiguousarray(np.asarray(w_v, f32).T)
    woT = np.ascontiguousarray(np.asarray(w_o, f32).T)

    xT = {}
    for g in range(B):
        xT[("q", g)] = np.ascontiguousarray(np.asarray(query[g], f32).T.astype(bf16))
        xT[("k", g)] = np.ascontiguousarray(np.asarray(key_[g], f32).T.astype(bf16))
        xT[("v", g)] = np.ascontiguousarray(np.asarray(value[g], f32).T.astype(bf16))

    bo4 = np.broadcast_to(
        (np.asarray(b_o, f32) / 4.0).reshape(1, D), (128, D)
    ).astype(bf16)

    in_maps = []
    for c in range(N_CORES):
        g, p = c // 4, c % 4
        fsel = slice(FPC * p, FPC * (p + 1))
        in_maps.append({
            "xq": xT[("q", g)],
            "xk": xT[("k", g)],
            "xv": xT[("v", g)],
            "wq": np.ascontiguousarray(wqT[:, fsel].astype(bf16)),
            "wk": np.ascontiguousarray(wkT[:, fsel].astype(bf16)),
            "wv": np.ascontiguousarray(wvT[:, fsel].astype(bf16)),
            "wo": np.ascontiguousarray(woT[fsel, :].astype(bf16)),
            "bq": np.ascontiguousarray(
                np.asarray(b_q, f32)[fsel].reshape(1, FPC).astype(bf16)),
            "bv": np.ascontiguousarray(
                np.asarray(b_v, f32)[fsel].reshape(1, FPC).astype(bf16)),
            "bo4": bo4,
            "mtri": mtri,
        })
    return in_maps


def run(inputs, trace=False):
    from concourse.bass_utils import run_bass_kernel_spmd

    if "nc" not in _CACHE:
        _CACHE["nc"] = _build_nc()
    nc = _CACHE["nc"]
    in_maps = _prep_inputs(
        inputs["query"], inputs["key_"], inputs["value"],
        inputs["w_q"], inputs["b_q"], inputs["w_k"], inputs["b_k"],
        inputs["w_v"], inputs["b_v"], inputs["w_o"], inputs["b_o"],
    )
    res = run_bass_kernel_spmd(
        nc, in_maps, core_ids=list(range(N_CORES)), trace=trace,
    )
    out = np.empty((B, S, D), np.float32)
    for c in range(N_CORES):
        g, p = c // 4, c % 4
        # RS half i scatters q rows [1024*i + 256*p, 1024*i + 256*(p+1))
        out[g, 256 * p : 256 * (p + 1), :] = res.results[c]["out"][0:256]
        out[g, 1024 + 256 * p : 1024 + 256 * (p + 1), :] = res.results[c]["out"][256:512]
    return out, res


def kernel(**inputs):
    out, _ = run(inputs, trace=False)
    return out


# revision 47
# speedup vs baseline: 1.5949x; 1.0257x over previous
"""Causal multi-head attention on 8 Trainium2 NeuronCores.

Sharding: core c -> (batch g = c // 4, head-group p = c % 4, heads 4p..4p+3).
Each core projects Q/K/V for its batch with its 256 feature columns
(column-sharded w_q/w_k/w_v), runs causal attention for its 4 heads, computes
the partial output projection with its 256 rows of w_o, and a ReduceScatter
over each batch group sums the partials.

All matmul operands are bf16 (fp32 PSUM accumulation).  Tricks:
  - K bias is dropped: softmax((q+bq)(k+bk)^T) == softmax((q+bq) k^T) since
    the (q+bq)*bk term is constant along the softmax (k) axis.
  - Q/V/O biases enter PSUM as rank-1 ones-row matmuls on the PE (no vector
    engine bias adds).
  - V is computed in natural [kpos, feature] layout directly (no transpose),
    with an appended ones column per head so PV accumulates the softmax
    denominators for free.
  - Scores/exp/PV touch only the causally valid column range of each k tile;
    only the [128,128] triangular diagonal block needs a mask multiply.
  - Software pipeline: the V projection pass is interleaved with the first
    head-pair's score matmuls (feeds the activation engine early); o-proj
    half 0 is interleaved into attention as PE filler so its ReduceScatter
    fully overlaps the remaining attention work.
"""

import numpy as np

B, S, D, H = 2, 2048, 1024, 16
DK = D // H  # 64
N_CORES = 8
FPC = 256  # features per core

_CACHE = {}


def _build_nc():
    import os as os_mod
    from contextlib import ExitStack

    import concourse.mybir as mybir
    import concourse.tile as tile
    from concourse import bacc

    F32 = mybir.dt.float32
    BF16 = mybir.dt.bfloat16
    Exp = mybir.ActivationFunctionType.Exp
    Copy = mybir.ActivationFunctionType.Copy

    nc = bacc.Bacc("TRN2", target_bir_lowering=False, debug=False, num_devices=8)

    xq = nc.dram_tensor("xq", [D, S], BF16, kind="ExternalInput")
    xk = nc.dram_tensor("xk", [D, S], BF16, kind="ExternalInput")
    xv = nc.dram_tensor("xv", [D, S], BF16, kind="ExternalInput")
    wq = nc.dram_tensor("wq", [D, FPC], BF16, kind="ExternalInput")
    wk = nc.dram_tensor("wk", [D, FPC], BF16, kind="ExternalInput")
    wv = nc.dram_tensor("wv", [D, FPC], BF16, kind="ExternalInput")
    wo = nc.dram_tensor("wo", [FPC, D], BF16, kind="ExternalInput")
    bq = nc.dram_tensor("bq", [1, FPC], BF16, kind="ExternalInput")
    bv = nc.dram_tensor("bv", [1, FPC], BF16, kind="ExternalInput")
    bo4 = nc.dram_tensor("bo4", [128, D], BF16, kind="ExternalInput")
    mtri = nc.dram_tensor("mtri", [128, 2, 128], BF16, kind="ExternalInput")
    out = nc.dram_tensor("out", [512, D], BF16, kind="ExternalOutput")

    debug_taps = bool(os_mod.environ.get("BASS_DEBUG_TAPS"))
    if debug_taps:
        dbg_q = nc.dram_tensor("dbg_q", [128, 2 * S], BF16, kind="ExternalOutput")
        dbg_k = nc.dram_tensor("dbg_k", [128, 2 * S], BF16, kind="ExternalOutput")
        dbg_v = nc.dram_tensor("dbg_v", [128, 16 * 4 * 65], BF16, kind="ExternalOutput")
        dbg_c = nc.dram_tensor("dbg_c", [128, 2 * S], BF16, kind="ExternalOutput")

    with tile.TileContext(nc) as tc:
        with (
            tc.tile_pool(name="consts", bufs=1) as consts,
            tc.tile_pool(name="persist", bufs=1) as persist,
            tc.tile_pool(name="xin", bufs=3) as xin,
            tc.tile_pool(name="prs", bufs=20) as prs,
            tc.tile_pool(name="normp", bufs=2) as normp,
            tc.tile_pool(name="oout", bufs=3) as oout,
            tc.tile_pool(name="dram", bufs=1, space="DRAM") as dram,
        ):
            # ---- SBUF constants ----
            wq_s = consts.tile([128, 8, FPC], BF16, tag="wq")
            wk_s = consts.tile([128, 8, FPC], BF16, tag="wk")
            wv_s = consts.tile([128, 8, FPC], BF16, tag="wv")
            wo_s = consts.tile([128, 2, D], BF16, tag="wo")
            bq_s = consts.tile([1, FPC], BF16, tag="bq")
            bv_s = consts.tile([1, FPC], BF16, tag="bv")
            bo4_s = consts.tile([128, D], BF16, tag="bo4")
            mask_s = consts.tile([128, 2, 128], BF16, tag="mask")
            ones_s = consts.tile([1, 512], BF16, tag="ones")

            # ---- persistent activations ----
            # feature f of the core maps to (pt = f // 128, row = f % 128);
            # local head h lives at [64*(h%2) : 64*(h%2)+64, h//2, :]
            qT_s = persist.tile([128, 2, S], BF16, tag="qT")
            kT_s = persist.tile([128, 2, S], BF16, tag="kT")
            v_s = persist.tile([128, 16, 4, 65], BF16, tag="v")
            ctx_s = persist.tile([128, 2, S], BF16, tag="ctx")

            nc.vector.memset(ones_s[:], 1.0)
            nc.vector.memset(v_s[:, :, :, 64:65], 1.0)

            # ---- constant DMAs on the gpsimd queue ----
            # only what phase 1a needs up front; the rest is marker-gated so
            # it doesn't preempt the just-in-time xq/xk chunk streams in the
            # shared DMA pool
            gq = nc.gpsimd
            gq.dma_start(wq_s[:, 0:1, :], wq[0:128, :].rearrange(
                "(kc p) f -> p kc f", p=128))
            gq.dma_start(wq_s[:, 1:8, :], wq[128:1024, :].rearrange(
                "(kc p) f -> p kc f", p=128))
            gq.dma_start(bq_s[:], bq.ap())
            gq.dma_start(wk_s[:], wk.ap().rearrange("(kc p) f -> p kc f", p=128))

            def issue_gated_dmas(entries):
                # markers are 1-element DVE copies READING qT_s/kT_s, so they
                # get a real RAW dependency on the projection evacuations —
                # the scheduler cannot hoist these DMAs ahead of the jit
                # xq/xk chunk streams
                for marker, full, src, dep_ap in entries:
                    nc.vector.tensor_copy(marker, dep_ap)
                    gq.dma_start(full, src)

            def issue_v_const_dmas(dep_ap):
                issue_gated_dmas([
                    (wv_s[0:1, 0:1, 0:1], wv_s[:],
                     wv.ap().rearrange("(kc p) f -> p kc f", p=128), dep_ap),
                    (bv_s[0:1, 0:1], bv_s[:], bv.ap(), dep_ap),
                    (mask_s[0:1, 0:1, 0:1], mask_s[:], mtri.ap(), dep_ap),
                ])

            def issue_o_const_dmas():
                issue_gated_dmas([
                    (wo_s[0:1, 0:1, 0:1], wo_s[:],
                     wo.ap().rearrange("(fc p) d -> p fc d", p=128),
                     kT_s[0:1, 1, 1536:1537]),
                    (bo4_s[0:1, 0:1], bo4_s[:], bo4.ap(),
                     kT_s[0:1, 1, 1536:1537]),
                ])

            # xq chunk 0 split in 4 pieces so the first matmuls start early
            xt0 = xin.tile([128, S], BF16, tag="x", name="xt0")
            for qb in range(4):
                nc.sync.dma_start(
                    xt0[:, 512 * qb : 512 * (qb + 1)],
                    xq[0:128, 512 * qb : 512 * (qb + 1)],
                )

            # xv is resident for the natural-layout V pass.  Gate each chunk's
            # DMA on a DVE marker memset sequenced after the Q evacuations so
            # the xv transfers don't steal shared DMA bandwidth from the
            # just-in-time xq/xk chunk loads.
            xvp_stack = ExitStack()
            xvp = xvp_stack.enter_context(tc.tile_pool(name="xvp", bufs=1))
            xv_all = xvp.tile([128, 8, S], BF16, tag="xva")

            def issue_xv_dmas(dep_ap):
                # real RAW dependency on a late xk chunk landing keeps these
                # behind the jit xk stream in the FIFO transfer pool
                for kc in range(8):
                    nc.vector.tensor_copy(xv_all[0:1, kc, 0:1], dep_ap)
                    gq.dma_start(
                        xv_all[:, kc, :], xv[128 * kc : 128 * (kc + 1), :]
                    )

            # ---- phase 1a: Q and K projections (transposed layout) ----
            st1 = ExitStack()
            psP = st1.enter_context(tc.tile_pool(name="psP", bufs=8, space="PSUM"))

            def proj_pass(x_dram, w_tile, b_tile, dst, first):
                ps = {}
                xts = {}
                for kc in range(8):
                    if first and kc == 0:
                        xt = xt0
                    else:
                        xt = xin.tile([128, S], BF16, tag="x")
                        nc.sync.dma_start(xt[:], x_dram[128 * kc : 128 * (kc + 1), :])
                    xts[kc] = xt
                    for pt in range(2):
                        for qb in range(4):
                            if kc == 0:
                                ps[(pt, qb)] = psP.tile(
                                    [128, 512], F32, tag="pp", name=f"pp{pt}{qb}"
                                )
                            nc.tensor.matmul(
                                ps[(pt, qb)][:],
                                w_tile[:, kc, 128 * pt : 128 * (pt + 1)],
                                xt[:, 512 * qb : 512 * (qb + 1)],
                                start=(kc == 0),
                                stop=(kc == 7 and b_tile is None),
                            )
                for pt in range(2):
                    for qb in range(4):
                        if b_tile is not None:
                            nc.tensor.matmul(
                                ps[(pt, qb)][:],
                                b_tile[0:1, 128 * pt : 128 * (pt + 1)],
                                ones_s[0:1, 0:512],
                                start=False,
                                stop=True,
                                skip_group_check=True,
                            )
                        # split the evacuations across DVE and Act so the
                        # next pass's PSUM buffers free up twice as fast
                        dst_ap = dst[:, pt, 512 * qb : 512 * (qb + 1)]
                        if qb % 2:
                            nc.scalar.activation(dst_ap, ps[(pt, qb)][:], Copy)
                        else:
                            nc.vector.tensor_copy(dst_ap, ps[(pt, qb)][:])
                return xts

            proj_pass(xq, wq_s, bq_s, qT_s, first=True)
            xk_ts = proj_pass(xk, wk_s, None, kT_s, first=False)
            issue_v_const_dmas(xk_ts[5][0:1, 0:1])
            issue_xv_dmas(xk_ts[7][0:1, 0:1])
            issue_o_const_dmas()
            st1.close()

            # ---- attention state/helpers ----
            attn_stack = ExitStack()
            psS = attn_stack.enter_context(
                tc.tile_pool(name="psS", bufs=2, space="PSUM")
            )
            pr_map = {}
            ctx_map = {}

            def segments(s0):
                if s0 < 512:
                    return [(s0, 512), (512, 1024)]
                return [(s0, 1024)]

            def emit_scores(qbp, hp, ki):
                # both si heads of the pair share si-interleaved sc/pr tiles
                # so exp and the mask multiply cover two heads per
                # instruction; one sc tile per 512-column segment keeps the
                # PSUM footprint at 2 banks so bufs=2 still pipelines
                s0 = max(0, 128 * ki - 1024 * qbp)
                pr = prs.tile([128, 2, 1024], BF16, tag="pr", name=f"pr{qbp}{hp}_{ki}")
                for a, b in segments(s0):
                    sc = psS.tile(
                        [128, 2, 512], F32, tag="sc", name=f"sc{qbp}{hp}_{ki}_{a}"
                    )
                    for si in range(2):
                        nc.tensor.matmul(
                            sc[:, si, 0 : b - a],
                            kT_s[64 * si : 64 * si + 64, hp,
                                 128 * ki : 128 * (ki + 1)],
                            qT_s[64 * si : 64 * si + 64, hp,
                                 1024 * qbp + a : 1024 * qbp + b],
                            start=True,
                            stop=True,
                        )
                    nc.scalar.activation(
                        out=pr[:, :, a:b], in_=sc[:, :, 0 : b - a],
                        func=Exp, scale=0.125,
                    )
                if 128 * ki >= 1024 * qbp:  # diagonal tile inside this window
                    nc.vector.tensor_mul(
                        pr[:, :, s0 : s0 + 128], pr[:, :, s0 : s0 + 128], mask_s[:]
                    )
                pr_map[(qbp, hp, ki)] = (pr, s0)

            def emit_pv(qbp, hp, ki, psA):
                nkt = 8 * (qbp + 1)
                pr, s0 = pr_map.pop((qbp, hp, ki))
                last_a = (512 + 1024 * qbp) // 128 - 1
                for si in range(2):
                    key = (qbp, hp, si)
                    if key not in ctx_map:
                        ctx_map[key] = psA.tile(
                            [65, 1024], F32, tag="ctx", name=f"ctx{qbp}{hp}{si}"
                        )
                    ctx = ctx_map[key]
                    for a, b in segments(s0):
                        last = last_a if b == 512 else nkt - 1
                        nc.tensor.matmul(
                            ctx[:, a:b],
                            v_s[:, ki, 2 * hp + si, :],
                            pr[:, si, a:b],
                            start=(ki == 0),
                            stop=(ki == last),
                            skip_group_check=True,
                        )

            def emit_norm(qbp, hp, cols=(0, 1024), release=True):
                a, b = cols
                w = b - a
                ctmp, rc, rbc = {}, {}, {}
                for si in range(2):
                    ctx = ctx_map[(qbp, hp, si)]
                    if release and b == 1024:
                        ctx_map.pop((qbp, hp, si))
                    ctmp[si] = normp.tile([65, 1024], BF16, tag="ctmp", name=f"ctmp{si}")
                    nc.vector.tensor_copy(ctmp[si][:, 0:w], ctx[:, a:b])
                for si in range(2):
                    rc[si] = normp.tile([1, 1024], BF16, tag="rc", name=f"rc{si}")
                    with nc.allow_low_precision("softmax denom recip in bf16"):
                        nc.vector.reciprocal(rc[si][:, 0:w], ctmp[si][64:65, 0:w])
                for si in range(2):
                    rbc[si] = normp.tile([64, 1024], BF16, tag="rbc", name=f"rbc{si}")
                    nc.gpsimd.partition_broadcast(rbc[si][:, 0:w], rc[si][:, 0:w])
                for si in range(2):
                    nc.vector.tensor_mul(
                        ctx_s[64 * si : 64 * si + 64, hp,
                              1024 * qbp + a : 1024 * qbp + b],
                        ctmp[si][0:64, 0:w],
                        rbc[si][:, 0:w],
                    )

            rs_in = [dram.tile([S // 2, D], BF16, name=f"rs_in{i}") for i in range(2)]
            rs_out = [dram.tile([256, D], BF16, name=f"rs_out{i}") for i in range(2)]

            def emit_oproj_sl(h, sl, pool, evac, po_shape):
                st = 8 * h + sl
                po_t = pool.tile(po_shape, F32, tag="sc", name=f"po{h}_{sl}")
                three_d = len(po_shape) == 3
                pe_bias = evac == "act"
                for nb in range(2):
                    po_nb = po_t[:, nb, :] if three_d else po_t[:, 512 * nb : 512 * (nb + 1)]
                    for fc in range(2):
                        nc.tensor.matmul(
                            po_nb,
                            ctx_s[:, fc, 128 * st : 128 * (st + 1)],
                            wo_s[:, fc, 512 * nb : 512 * (nb + 1)],
                            start=(fc == 0),
                            stop=(fc == 1 and not pe_bias),
                        )
                    if pe_bias:
                        nc.tensor.matmul(
                            po_nb,
                            ones_s[0:1, 0:128],
                            bo4_s[0:1, 512 * nb : 512 * (nb + 1)],
                            start=False,
                            stop=True,
                            skip_group_check=True,
                        )
                ot = oout.tile([128, 1024], BF16, tag="ot")
                po_v = po_t[:] if three_d else po_t[:].rearrange("p (n x) -> p n x", n=2)
                ot_v = ot[:].rearrange("p (n x) -> p n x", n=2)
                if evac == "act":
                    nc.scalar.activation(ot_v, po_v, Copy)
                else:
                    # fold the b_o/4 bias into the PSUM evacuation
                    nc.vector.tensor_add(
                        ot_v, po_v, bo4_s[:].rearrange("p (n x) -> p n x", n=2)
                    )
                nc.sync.dma_start(rs_in[h][128 * sl : 128 * (sl + 1), :], ot[:])

            def emit_rs(h):
                if not os_mod.environ.get("BASS_SIM_NO_RS"):
                    import concourse.mybir as mybir_mod

                    nc.gpsimd.collective_compute(
                        "ReduceScatter",
                        mybir_mod.AluOpType.add,
                        replica_groups=[[0, 1, 2, 3], [4, 5, 6, 7]],
                        ins=[rs_in[h].opt()],
                        outs=[rs_out[h].opt()],
                    )
                    nc.sync.dma_start(
                        out[256 * h : 256 * (h + 1), :], rs_out[h][:]
                    )
                else:
                    nc.sync.dma_start(
                        out[256 * h : 256 * (h + 1), :], rs_in[h][0:256, :]
                    )

            # ---- phase 1b: qbp0-hp0 scores (PE/Act filler while the xv ----
            # ---- chunks stream in) + chunk-major natural-layout V pass ----
            for ki in range(8):
                emit_scores(0, 0, ki)
            for ki in range(8):
                emit_scores(0, 1, ki)

            stV = ExitStack()
            psV = stV.enter_context(tc.tile_pool(name="psV", bufs=4, space="PSUM"))
            # 4 single-bank tiles per group (one st each — a PSUM bank may
            # only hold ONE accumulation group at a time); chunk-major order
            # lets group 0 ride the incoming xv chunk DMAs
            for g in range(4):
                pvt = {}
                for kc in range(8):
                    for j in range(4):
                        st = 4 * g + j
                        if kc == 0:
                            pvt[j] = psV.tile(
                                [128, 512], F32, tag="pv", name=f"pv{g}_{j}"
                            )
                        nc.tensor.matmul(
                            pvt[j][:, 0:256],
                            xv_all[:, kc, 128 * st : 128 * (st + 1)],
                            wv_s[:, kc, :],
                            start=(kc == 0),
                            stop=False,
                        )
                for j in range(4):
                    st = 4 * g + j
                    nc.tensor.matmul(
                        pvt[j][:, 0:256],
                        ones_s[0:1, 0:128],
                        bv_s[0:1, :],
                        start=False,
                        stop=True,
                        skip_group_check=True,
                    )
                    nc.vector.tensor_copy(
                        v_s[:, st, :, 0:64],
                        pvt[j][:, 0:256].rearrange("p (h x) -> p h x", h=4),
                    )
            stV.close()
            xvp_stack.close()

            psA = attn_stack.enter_context(
                tc.tile_pool(name="psA", bufs=2, space="PSUM")
            )

            # ---- stage (qbp0, hp1): hp0 + hp1 PVs (scores pre-emitted) ----
            for ki in range(8):
                emit_pv(0, 0, ki, psA)
            emit_norm(0, 0)

            # ---- stage (qbp1, hp0): scores + qbp0-hp1 PVs, then own  ----
            # ---- PVs + o-proj half0 as PE filler after norm(0,1)     ----
            # (ctx pool has 2 buffer pairs: (1,0)'s PVs may only start
            #  after norm(0,1) releases qbp0-hp1's ctx tiles)
            for ki in range(16):
                emit_scores(1, 0, ki)
                if ki < 8:
                    emit_pv(0, 1, ki, psA)
                if ki == 8:
                    emit_norm(0, 1)
                if ki >= 9:
                    emit_pv(1, 0, ki - 9, psA)
                if ki >= 8:
                    emit_oproj_sl(0, ki - 8, psS, "dve", [128, 2, 512])
            for k in range(7, 16):
                emit_pv(1, 0, k, psA)
            emit_norm(1, 0)
            emit_rs(0)

            # ---- stage (qbp1, hp1): scores + own PVs (1-ki lag); ctx ----
            # ---- cols [0:512) finish at ki=11, so their norm + the   ----
            # ---- first o-proj half1 slices interleave into the tail  ----
            for ki in range(16):
                emit_scores(1, 1, ki)
                if ki >= 1:
                    emit_pv(1, 1, ki - 1, psA)
                if ki == 13:
                    emit_norm(1, 1, cols=(0, 512), release=False)
                if ki >= 14:
                    emit_oproj_sl(1, ki - 14, psS, "dve", [128, 2, 512])
            emit_pv(1, 1, 15, psA)
            emit_norm(1, 1, cols=(512, 1024))
            emit_oproj_sl(1, 2, psS, "dve", [128, 2, 512])
            emit_oproj_sl(1, 3, psS, "act", [128, 2, 512])
            attn_stack.close()

            if debug_taps:
                nc.sync.dma_start(dbg_q.ap(), qT_s[:].rearrange("p a b -> p (a b)"))
                nc.sync.dma_start(dbg_k.ap(), kT_s[:].rearrange("p a b -> p (a b)"))
                nc.sync.dma_start(dbg_v.ap(), v_s[:].rearrange("p a b c -> p (a b c)"))
                nc.sync.dma_start(dbg_c.ap(), ctx_s[:].rearrange("p a b -> p (a b)"))

            # ---- o-proj half 1 + final ReduceScatter ----
            stO = ExitStack()
            psO = stO.enter_context(tc.tile_pool(name="psO", bufs=4, space="PSUM"))
            for sl in range(4, 8):
                emit_oproj_sl(1, sl, psO, "act" if sl % 2 else "dve", [128, 1024])
            stO.close()
            emit_rs(1)

    nc.compile()
    return nc


def _prep_inputs(query, key_, value, w_q, b_q, w_k, b_k, w_v, b_v, w_o, b_o):
    """Build the 8 per-core input maps (host-side sharding / re-layout)."""
    import ml_dtypes

    bf16 = ml_dtypes.bfloat16
    f32 = np.float32

    r = np.arange(128)[:, None, None]
    j = np.arange(128)[None, None, :]
    # allowed iff q >= k on the diagonal tile; doubled for the si-pair layout
    mtri = np.broadcast_to(j >= r, (128, 2, 128)).astype(bf16)

    wqT = np.ascontiguousarray(np.asarray(w_q, f32).T)  # [D_in, D_out]
    wkT = np.ascontiguousarray(np.asarray(w_k, f32).T)
    wvT = np.ascontiguousarray(np.asarray(w_v, f32).T)
    woT = np.ascontiguousarray(np.asarray(w_o, f32).T)

    xT = {}
    for g in range(B):
        xT[("q", g)] = np.ascontiguousarray(np.asarray(query[g], f32).T.astype(bf16))
        xT[("k", g)] = np.ascontiguousarray(np.asarray(key_[g], f32).T.astype(bf16))
        xT[("v", g)] = np.ascontiguousarray(np.asarray(value[g], f32).T.astype(bf16))

    bo4 = np.broadcast_to(
        (np.asarray(b_o, f32) / 4.0).reshape(1, D), (128, D)
    ).astype(bf16)

    in_maps = []
    for c in range(N_CORES):
        g, p = c // 4, c % 4
        fsel = slice(FPC * p, FPC * (p + 1))
        in_maps.append({
            "xq": xT[("q", g)],
            "xk": xT[("k", g)],
            "xv": xT[("v", g)],
            "wq": np.ascontiguousarray(wqT[:, fsel].astype(bf16)),
            "wk": np.ascontiguousarray(wkT[:, fsel].astype(bf16)),
            "wv": np.ascontiguousarray(wvT[:, fsel].astype(bf16)),
            "wo": np.ascontiguousarray(woT[fsel, :].astype(bf16)),
            "bq": np.ascontiguousarray(
                np.asarray(b_q, f32)[fsel].reshape(1, FPC).astype(bf16)),
            "bv": np.ascontiguousarray(
                np.asarray(b_v, f32)[fsel].reshape(1, FPC).astype(bf16)),
            "bo4": bo4,
            "mtri": mtri,
        })
    return in_maps


def run(inputs, trace=False):
    from concourse.bass_utils import run_bass_kernel_spmd

    if "nc" not in _CACHE:
        _CACHE["nc"] = _build_nc()
    nc = _CACHE["nc"]
    in_maps = _prep_inputs(
        inputs["query"], inputs["key_"], inputs["value"],
        inputs["w_q"], inputs["b_q"], inputs["w_k"], inputs["b_k"],
        inputs["w_v"], inputs["b_v"], inputs["w_o"], inputs["b_o"],
    )
    res = run_bass_kernel_spmd(
        nc, in_maps, core_ids=list(range(N_CORES)), trace=trace,
    )
    out = np.empty((B, S, D), np.float32)
    for c in range(N_CORES):
        g, p = c // 4, c % 4
        # RS half i scatters q rows [1024*i + 256*p, 1024*i + 256*(p+1))
        o = np.asarray(res.results[c]["out"]).astype(np.float32)
        out[g, 256 * p : 256 * (p + 1), :] = o[0:256]
        out[g, 1024 + 256 * p : 1024 + 256 * (p + 1), :] = o[256:512]
    return out, res


def kernel(**inputs):
    out, _ = run(inputs, trace=False)
    return out


# revision 50
# speedup vs baseline: 1.6298x; 1.0219x over previous
"""Causal multi-head attention on 8 Trainium2 NeuronCores.

Sharding: core c -> (batch g = c // 4, head-group p = c % 4, heads 4p..4p+3).
Each core projects Q/K/V for its batch with its 256 feature columns
(column-sharded w_q/w_k/w_v), runs causal attention for its 4 heads, computes
the partial output projection with its 256 rows of w_o, and a ReduceScatter
over each batch group sums the partials.

All matmul operands are bf16 (fp32 PSUM accumulation).  Tricks:
  - K bias is dropped: softmax((q+bq)(k+bk)^T) == softmax((q+bq) k^T) since
    the (q+bq)*bk term is constant along the softmax (k) axis.
  - Q/V/O biases enter PSUM as rank-1 ones-row matmuls on the PE (no vector
    engine bias adds).
  - V is computed in natural [kpos, feature] layout directly (no transpose),
    with an appended ones column per head so PV accumulates the softmax
    denominators for free.
  - Scores/exp/PV touch only the causally valid column range of each k tile;
    only the [128,128] triangular diagonal block needs a mask multiply.
  - Software pipeline: the V projection pass is interleaved with the first
    head-pair's score matmuls (feeds the activation engine early); o-proj
    half 0 is interleaved into attention as PE filler so its ReduceScatter
    fully overlaps the remaining attention work.
"""

import numpy as np

B, S, D, H = 2, 2048, 1024, 16
DK = D // H  # 64
N_CORES = 8
FPC = 256  # features per core

_CACHE = {}


def _build_nc():
    import os as os_mod
    from contextlib import ExitStack

    import concourse.mybir as mybir
    import concourse.tile as tile
    from concourse import bacc

    F32 = mybir.dt.float32
    BF16 = mybir.dt.bfloat16
    Exp = mybir.ActivationFunctionType.Exp
    Copy = mybir.ActivationFunctionType.Copy
    Identity = mybir.ActivationFunctionType.Identity

    nc = bacc.Bacc("TRN2", target_bir_lowering=False, debug=False, num_devices=8)

    xq = nc.dram_tensor("xq", [D, S], BF16, kind="ExternalInput")
    xk = nc.dram_tensor("xk", [D, S], BF16, kind="ExternalInput")
    xv = nc.dram_tensor("xv", [D, S], BF16, kind="ExternalInput")
    wq = nc.dram_tensor("wq", [D, FPC], BF16, kind="ExternalInput")
    wk = nc.dram_tensor("wk", [D, FPC], BF16, kind="ExternalInput")
    wv = nc.dram_tensor("wv", [D, FPC], BF16, kind="ExternalInput")
    wo = nc.dram_tensor("wo", [FPC, D], BF16, kind="ExternalInput")
    bq = nc.dram_tensor("bq", [128, 2], F32, kind="ExternalInput")
    bv = nc.dram_tensor("bv", [128, FPC], BF16, kind="ExternalInput")
    bo4 = nc.dram_tensor("bo4", [128, D], BF16, kind="ExternalInput")
    mtri = nc.dram_tensor("mtri", [128, 2, 128], BF16, kind="ExternalInput")
    out = nc.dram_tensor("out", [512, D], BF16, kind="ExternalOutput")

    debug_taps = bool(os_mod.environ.get("BASS_DEBUG_TAPS"))
    if debug_taps:
        dbg_q = nc.dram_tensor("dbg_q", [128, 2 * S], BF16, kind="ExternalOutput")
        dbg_k = nc.dram_tensor("dbg_k", [128, 2 * S], BF16, kind="ExternalOutput")
        dbg_v = nc.dram_tensor("dbg_v", [128, 16 * 4 * 65], BF16, kind="ExternalOutput")
        dbg_c = nc.dram_tensor("dbg_c", [128, 2 * S], BF16, kind="ExternalOutput")

    with tile.TileContext(nc) as tc:
        with (
            tc.tile_pool(name="consts", bufs=1) as consts,
            tc.tile_pool(name="persist", bufs=1) as persist,
            tc.tile_pool(name="xin", bufs=3) as xin,
            tc.tile_pool(name="prs", bufs=20) as prs,
            tc.tile_pool(name="normp", bufs=2) as normp,
            tc.tile_pool(name="oout", bufs=3) as oout,
            tc.tile_pool(name="dram", bufs=1, space="DRAM") as dram,
        ):
            # ---- SBUF constants ----
            wq_s = consts.tile([128, 8, FPC], BF16, tag="wq")
            wk_s = consts.tile([128, 8, FPC], BF16, tag="wk")
            wv_s = consts.tile([128, 8, FPC], BF16, tag="wv")
            wo_s = consts.tile([128, 2, D], BF16, tag="wo")
            bq_s = consts.tile([128, 2], F32, tag="bq")
            bv_s = consts.tile([128, FPC], BF16, tag="bv")
            bo4_s = consts.tile([128, D], BF16, tag="bo4")
            mask_s = consts.tile([128, 2, 128], BF16, tag="mask")
            ones_s = consts.tile([1, 512], BF16, tag="ones")

            # ---- persistent activations ----
            # feature f of the core maps to (pt = f // 128, row = f % 128);
            # local head h lives at [64*(h%2) : 64*(h%2)+64, h//2, :]
            qT_s = persist.tile([128, 2, S], BF16, tag="qT")
            kT_s = persist.tile([128, 2, S], BF16, tag="kT")
            v_s = persist.tile([128, 16, 4, 65], BF16, tag="v")
            ctx_s = persist.tile([128, 2, S], BF16, tag="ctx")

            nc.vector.memset(ones_s[:], 1.0)
            nc.vector.memset(v_s[:, :, :, 64:65], 1.0)

            # ---- constant DMAs on the gpsimd queue ----
            # only what phase 1a needs up front; the rest is marker-gated so
            # it doesn't preempt the just-in-time xq/xk chunk streams in the
            # shared DMA pool
            gq = nc.gpsimd
            gq.dma_start(wq_s[:, 0:1, :], wq[0:128, :].rearrange(
                "(kc p) f -> p kc f", p=128))
            gq.dma_start(wq_s[:, 1:8, :], wq[128:1024, :].rearrange(
                "(kc p) f -> p kc f", p=128))
            gq.dma_start(bq_s[:], bq.ap())
            gq.dma_start(wk_s[:], wk.ap().rearrange("(kc p) f -> p kc f", p=128))

            def issue_gated_dmas(entries):
                # markers are 1-element DVE copies READING qT_s/kT_s, so they
                # get a real RAW dependency on the projection evacuations —
                # the scheduler cannot hoist these DMAs ahead of the jit
                # xq/xk chunk streams
                for marker, full, src, dep_ap in entries:
                    nc.vector.tensor_copy(marker, dep_ap)
                    gq.dma_start(full, src)

            def issue_v_const_dmas(dep_ap):
                issue_gated_dmas([
                    (wv_s[0:1, 0:1, 0:1], wv_s[:],
                     wv.ap().rearrange("(kc p) f -> p kc f", p=128), dep_ap),
                    (bv_s[0:1, 0:1], bv_s[:], bv.ap(), dep_ap),
                    (mask_s[0:1, 0:1, 0:1], mask_s[:], mtri.ap(), dep_ap),
                ])

            def issue_o_const_dmas():
                issue_gated_dmas([
                    (wo_s[0:1, 0:1, 0:1], wo_s[:],
                     wo.ap().rearrange("(fc p) d -> p fc d", p=128),
                     kT_s[0:1, 1, 1536:1537]),
                    (bo4_s[0:1, 0:1], bo4_s[:], bo4.ap(),
                     kT_s[0:1, 1, 1536:1537]),
                ])

            # xq chunk 0 split in 4 pieces so the first matmuls start early
            xt0 = xin.tile([128, S], BF16, tag="x", name="xt0")
            for qb in range(4):
                nc.sync.dma_start(
                    xt0[:, 512 * qb : 512 * (qb + 1)],
                    xq[0:128, 512 * qb : 512 * (qb + 1)],
                )

            # xv is resident for the natural-layout V pass.  Gate each chunk's
            # DMA on a DVE marker memset sequenced after the Q evacuations so
            # the xv transfers don't steal shared DMA bandwidth from the
            # just-in-time xq/xk chunk loads.
            xvp_stack = ExitStack()
            xvp = xvp_stack.enter_context(tc.tile_pool(name="xvp", bufs=1))
            xv_all = xvp.tile([128, 8, S], BF16, tag="xva")

            def issue_xv_dmas(dep_ap):
                # real RAW dependency on a late xk chunk landing keeps these
                # behind the jit xk stream in the FIFO transfer pool
                for kc in range(8):
                    nc.vector.tensor_copy(xv_all[0:1, kc, 0:1], dep_ap)
                    gq.dma_start(
                        xv_all[:, kc, :], xv[128 * kc : 128 * (kc + 1), :]
                    )

            # ---- phase 1a: Q and K projections (transposed layout) ----
            st1 = ExitStack()
            psP = st1.enter_context(tc.tile_pool(name="psP", bufs=8, space="PSUM"))

            def proj_pass(x_dram, w_tile, b_tile, dst, first):
                ps = {}
                xts = {}
                for kc in range(8):
                    if first and kc == 0:
                        xt = xt0
                    else:
                        xt = xin.tile([128, S], BF16, tag="x")
                        nc.sync.dma_start(xt[:], x_dram[128 * kc : 128 * (kc + 1), :])
                    xts[kc] = xt
                    for pt in range(2):
                        for qb in range(4):
                            if kc == 0:
                                ps[(pt, qb)] = psP.tile(
                                    [128, 512], F32, tag="pp", name=f"pp{pt}{qb}"
                                )
                            nc.tensor.matmul(
                                ps[(pt, qb)][:],
                                w_tile[:, kc, 128 * pt : 128 * (pt + 1)],
                                xt[:, 512 * qb : 512 * (qb + 1)],
                                start=(kc == 0),
                                stop=(kc == 7),
                            )
                for pt in range(2):
                    for qb in range(4):
                        # bias (per-partition in this transposed layout) is
                        # folded into the evacuation, which is split across
                        # DVE and Act so the next pass's PSUM buffers free
                        # up twice as fast
                        dst_ap = dst[:, pt, 512 * qb : 512 * (qb + 1)]
                        if qb % 2:
                            if b_tile is not None:
                                nc.scalar.activation(
                                    dst_ap, ps[(pt, qb)][:], Identity,
                                    bias=b_tile[:, pt : pt + 1],
                                )
                            else:
                                nc.scalar.activation(dst_ap, ps[(pt, qb)][:], Copy)
                        else:
                            if b_tile is not None:
                                nc.vector.tensor_scalar_add(
                                    dst_ap, ps[(pt, qb)][:], b_tile[:, pt : pt + 1]
                                )
                            else:
                                nc.vector.tensor_copy(dst_ap, ps[(pt, qb)][:])
                return xts

            proj_pass(xq, wq_s, bq_s, qT_s, first=True)
            xk_ts = proj_pass(xk, wk_s, None, kT_s, first=False)
            issue_v_const_dmas(xk_ts[5][0:1, 0:1])
            issue_xv_dmas(xk_ts[7][0:1, 0:1])
            issue_o_const_dmas()
            st1.close()

            # ---- attention state/helpers ----
            attn_stack = ExitStack()
            psS = attn_stack.enter_context(
                tc.tile_pool(name="psS", bufs=2, space="PSUM")
            )
            pr_map = {}
            ctx_map = {}

            def segments(s0):
                if s0 < 512:
                    return [(s0, 512), (512, 1024)]
                return [(s0, 1024)]

            def emit_scores(qbp, hp, ki):
                # both si heads of the pair share si-interleaved sc/pr tiles
                # so exp and the mask multiply cover two heads per
                # instruction; one sc tile per 512-column segment keeps the
                # PSUM footprint at 2 banks so bufs=2 still pipelines
                s0 = max(0, 128 * ki - 1024 * qbp)
                pr = prs.tile([128, 2, 1024], BF16, tag="pr", name=f"pr{qbp}{hp}_{ki}")
                for a, b in segments(s0):
                    sc = psS.tile(
                        [128, 2, 512], F32, tag="sc", name=f"sc{qbp}{hp}_{ki}_{a}"
                    )
                    for si in range(2):
                        nc.tensor.matmul(
                            sc[:, si, 0 : b - a],
                            kT_s[64 * si : 64 * si + 64, hp,
                                 128 * ki : 128 * (ki + 1)],
                            qT_s[64 * si : 64 * si + 64, hp,
                                 1024 * qbp + a : 1024 * qbp + b],
                            start=True,
                            stop=True,
                        )
                    nc.scalar.activation(
                        out=pr[:, :, a:b], in_=sc[:, :, 0 : b - a],
                        func=Exp, scale=0.125,
                    )
                if 128 * ki >= 1024 * qbp:  # diagonal tile inside this window
                    nc.vector.tensor_mul(
                        pr[:, :, s0 : s0 + 128], pr[:, :, s0 : s0 + 128], mask_s[:]
                    )
                pr_map[(qbp, hp, ki)] = (pr, s0)

            def emit_pv(qbp, hp, ki, psA):
                nkt = 8 * (qbp + 1)
                pr, s0 = pr_map.pop((qbp, hp, ki))
                last_a = (512 + 1024 * qbp) // 128 - 1
                for si in range(2):
                    key = (qbp, hp, si)
                    if key not in ctx_map:
                        ctx_map[key] = psA.tile(
                            [65, 1024], F32, tag="ctx", name=f"ctx{qbp}{hp}{si}"
                        )
                    ctx = ctx_map[key]
                    for a, b in segments(s0):
                        last = last_a if b == 512 else nkt - 1
                        nc.tensor.matmul(
                            ctx[:, a:b],
                            v_s[:, ki, 2 * hp + si, :],
                            pr[:, si, a:b],
                            start=(ki == 0),
                            stop=(ki == last),
                            skip_group_check=True,
                        )

            def emit_norm(qbp, hp, cols=(0, 1024), release=True):
                a, b = cols
                w = b - a
                ctmp, rc, rbc = {}, {}, {}
                for si in range(2):
                    ctx = ctx_map[(qbp, hp, si)]
                    if release and b == 1024:
                        ctx_map.pop((qbp, hp, si))
                    ctmp[si] = normp.tile([65, 1024], BF16, tag="ctmp", name=f"ctmp{si}")
                    nc.vector.tensor_copy(ctmp[si][:, 0:w], ctx[:, a:b])
                for si in range(2):
                    rc[si] = normp.tile([1, 1024], BF16, tag="rc", name=f"rc{si}")
                    with nc.allow_low_precision("softmax denom recip in bf16"):
                        nc.vector.reciprocal(rc[si][:, 0:w], ctmp[si][64:65, 0:w])
                for si in range(2):
                    rbc[si] = normp.tile([64, 1024], BF16, tag="rbc", name=f"rbc{si}")
                    nc.gpsimd.partition_broadcast(rbc[si][:, 0:w], rc[si][:, 0:w])
                for si in range(2):
                    nc.vector.tensor_mul(
                        ctx_s[64 * si : 64 * si + 64, hp,
                              1024 * qbp + a : 1024 * qbp + b],
                        ctmp[si][0:64, 0:w],
                        rbc[si][:, 0:w],
                    )

            rs_in = [dram.tile([S // 2, D], BF16, name=f"rs_in{i}") for i in range(2)]
            rs_out = [dram.tile([256, D], BF16, name=f"rs_out{i}") for i in range(2)]

            def emit_oproj_sl(h, sl, pool, evac, po_shape):
                st = 8 * h + sl
                po_t = pool.tile(po_shape, F32, tag="sc", name=f"po{h}_{sl}")
                three_d = len(po_shape) == 3
                pe_bias = evac == "act"
                for nb in range(2):
                    po_nb = po_t[:, nb, :] if three_d else po_t[:, 512 * nb : 512 * (nb + 1)]
                    for fc in range(2):
                        nc.tensor.matmul(
                            po_nb,
                            ctx_s[:, fc, 128 * st : 128 * (st + 1)],
                            wo_s[:, fc, 512 * nb : 512 * (nb + 1)],
                            start=(fc == 0),
                            stop=(fc == 1 and not pe_bias),
                        )
                    if pe_bias:
                        nc.tensor.matmul(
                            po_nb,
                            ones_s[0:1, 0:128],
                            bo4_s[0:1, 512 * nb : 512 * (nb + 1)],
                            start=False,
                            stop=True,
                            skip_group_check=True,
                        )
                ot = oout.tile([128, 1024], BF16, tag="ot")
                po_v = po_t[:] if three_d else po_t[:].rearrange("p (n x) -> p n x", n=2)
                ot_v = ot[:].rearrange("p (n x) -> p n x", n=2)
                if evac == "act":
                    nc.scalar.activation(ot_v, po_v, Copy)
                else:
                    # fold the b_o/4 bias into the PSUM evacuation
                    nc.vector.tensor_add(
                        ot_v, po_v, bo4_s[:].rearrange("p (n x) -> p n x", n=2)
                    )
                nc.sync.dma_start(rs_in[h][128 * sl : 128 * (sl + 1), :], ot[:])

            def emit_rs(h):
                if not os_mod.environ.get("BASS_SIM_NO_RS"):
                    import concourse.mybir as mybir_mod

                    nc.gpsimd.collective_compute(
                        "ReduceScatter",
                        mybir_mod.AluOpType.add,
                        replica_groups=[[0, 1, 2, 3], [4, 5, 6, 7]],
                        ins=[rs_in[h].opt()],
                        outs=[rs_out[h].opt()],
                    )
                    nc.sync.dma_start(
                        out[256 * h : 256 * (h + 1), :], rs_out[h][:]
                    )
                else:
                    nc.sync.dma_start(
                        out[256 * h : 256 * (h + 1), :], rs_in[h][0:256, :]
                    )

            # ---- phase 1b: qbp0-hp0 scores (PE/Act filler while the xv ----
            # ---- chunks stream in) + chunk-major natural-layout V pass ----
            for ki in range(8):
                emit_scores(0, 0, ki)
            for ki in range(8):
                emit_scores(0, 1, ki)

            stV = ExitStack()
            psV = stV.enter_context(tc.tile_pool(name="psV", bufs=4, space="PSUM"))
            # 4 single-bank tiles per group (one st each — a PSUM bank may
            # only hold ONE accumulation group at a time); chunk-major order
            # lets group 0 ride the incoming xv chunk DMAs
            for g in range(4):
                pvt = {}
                for kc in range(8):
                    for j in range(4):
                        st = 4 * g + j
                        if kc == 0:
                            pvt[j] = psV.tile(
                                [128, 512], F32, tag="pv", name=f"pv{g}_{j}"
                            )
                        nc.tensor.matmul(
                            pvt[j][:, 0:256],
                            xv_all[:, kc, 128 * st : 128 * (st + 1)],
                            wv_s[:, kc, :],
                            start=(kc == 0),
                            stop=(kc == 7),
                        )
                for j in range(4):
                    st = 4 * g + j
                    nc.vector.tensor_add(
                        v_s[:, st, :, 0:64],
                        pvt[j][:, 0:256].rearrange("p (h x) -> p h x", h=4),
                        bv_s[:].rearrange("p (h x) -> p h x", h=4),
                    )
            stV.close()
            xvp_stack.close()

            psA = attn_stack.enter_context(
                tc.tile_pool(name="psA", bufs=2, space="PSUM")
            )

            # ---- stage (qbp0, hp1): hp0 + hp1 PVs (scores pre-emitted) ----
            for ki in range(8):
                emit_pv(0, 0, ki, psA)
            emit_norm(0, 0)

            # ---- stage (qbp1, hp0): scores + qbp0-hp1 PVs, then own  ----
            # ---- PVs + o-proj half0 as PE filler after norm(0,1)     ----
            # (ctx pool has 2 buffer pairs: (1,0)'s PVs may only start
            #  after norm(0,1) releases qbp0-hp1's ctx tiles)
            for ki in range(16):
                emit_scores(1, 0, ki)
                if ki < 8:
                    emit_pv(0, 1, ki, psA)
                if ki == 8:
                    emit_norm(0, 1)
                if ki >= 9:
                    emit_pv(1, 0, ki - 9, psA)
                if ki >= 8:
                    emit_oproj_sl(0, ki - 8, psS, "dve", [128, 2, 512])
            for k in range(7, 16):
                emit_pv(1, 0, k, psA)
            emit_norm(1, 0)
            emit_rs(0)

            # ---- stage (qbp1, hp1): scores + own PVs (1-ki lag); ctx ----
            # ---- cols [0:512) finish at ki=11, so their norm + the   ----
            # ---- first o-proj half1 slices interleave into the tail  ----
            for ki in range(16):
                emit_scores(1, 1, ki)
                if ki >= 1:
                    emit_pv(1, 1, ki - 1, psA)
                if ki == 13:
                    emit_norm(1, 1, cols=(0, 512), release=False)
                if ki >= 14:
                    emit_oproj_sl(1, ki - 14, psS, "dve", [128, 2, 512])
            emit_pv(1, 1, 15, psA)
            emit_norm(1, 1, cols=(512, 1024))
            for sl in range(2, 8):
                emit_oproj_sl(1, sl, psS, "act" if sl % 2 else "dve", [128, 2, 512])
            attn_stack.close()

            if debug_taps:
                nc.sync.dma_start(dbg_q.ap(), qT_s[:].rearrange("p a b -> p (a b)"))
                nc.sync.dma_start(dbg_k.ap(), kT_s[:].rearrange("p a b -> p (a b)"))
                nc.sync.dma_start(dbg_v.ap(), v_s[:].rearrange("p a b c -> p (a b c)"))
                nc.sync.dma_start(dbg_c.ap(), ctx_s[:].rearrange("p a b -> p (a b)"))

            # ---- final ReduceScatter ----
            emit_rs(1)

    nc.compile()
    return nc


def _prep_inputs(query, key_, value, w_q, b_q, w_k, b_k, w_v, b_v, w_o, b_o):
    """Build the 8 per-core input maps (host-side sharding / re-layout)."""
    import ml_dtypes

    bf16 = ml_dtypes.bfloat16
    f32 = np.float32

    r = np.arange(128)[:, None, None]
    j = np.arange(128)[None, None, :]
    # allowed iff q >= k on the diagonal tile; doubled for the si-pair layout
    mtri = np.broadcast_to(j >= r, (128, 2, 128)).astype(bf16)

    wqT = np.ascontiguousarray(np.asarray(w_q, f32).T)  # [D_in, D_out]
    wkT = np.ascontiguousarray(np.asarray(w_k, f32).T)
    wvT = np.ascontiguousarray(np.asarray(w_v, f32).T)
    woT = np.ascontiguousarray(np.asarray(w_o, f32).T)

    xT = {}
    for g in range(B):
        xT[("q", g)] = np.ascontiguousarray(np.asarray(query[g], f32).T.astype(bf16))
        xT[("k", g)] = np.ascontiguousarray(np.asarray(key_[g], f32).T.astype(bf16))
        xT[("v", g)] = np.ascontiguousarray(np.asarray(value[g], f32).T.astype(bf16))

    bo4 = np.broadcast_to(
        (np.asarray(b_o, f32) / 4.0).reshape(1, D), (128, D)
    ).astype(bf16)

    in_maps = []
    for c in range(N_CORES):
        g, p = c // 4, c % 4
        fsel = slice(FPC * p, FPC * (p + 1))
        in_maps.append({
            "xq": xT[("q", g)],
            "xk": xT[("k", g)],
            "xv": xT[("v", g)],
            "wq": np.ascontiguousarray(wqT[:, fsel].astype(bf16)),
            "wk": np.ascontiguousarray(wkT[:, fsel].astype(bf16)),
            "wv": np.ascontiguousarray(wvT[:, fsel].astype(bf16)),
            "wo": np.ascontiguousarray(woT[fsel, :].astype(bf16)),
            "bq": np.ascontiguousarray(
                np.asarray(b_q, f32)[fsel].reshape(2, 128).T),
            "bv": np.ascontiguousarray(np.broadcast_to(
                np.asarray(b_v, f32)[fsel], (128, FPC)).astype(bf16)),
            "bo4": bo4,
            "mtri": mtri,
        })
    return in_maps


def run(inputs, trace=False):
    from concourse.bass_utils import run_bass_kernel_spmd

    if "nc" not in _CACHE:
        _CACHE["nc"] = _build_nc()
    nc = _CACHE["nc"]
    in_maps = _prep_inputs(
        inputs["query"], inputs["key_"], inputs["value"],
        inputs["w_q"], inputs["b_q"], inputs["w_k"], inputs["b_k"],
        inputs["w_v"], inputs["b_v"], inputs["w_o"], inputs["b_o"],
    )
    res = run_bass_kernel_spmd(
        nc, in_maps, core_ids=list(range(N_CORES)), trace=trace,
    )
    out = np.empty((B, S, D), np.float32)
    for c in range(N_CORES):
        g, p = c // 4, c % 4
        # RS half i scatters q rows [1024*i + 256*p, 1024*i + 256*(p+1))
        o = np.asarray(res.results[c]["out"]).astype(np.float32)
        out[g, 256 * p : 256 * (p + 1), :] = o[0:256]
        out[g, 1024 + 256 * p : 1024 + 256 * (p + 1), :] = o[256:512]
    return out, res


def kernel(**inputs):
    out, _ = run(inputs, trace=False)
    return out


# revision 59
# speedup vs baseline: 1.6503x; 1.0126x over previous
"""Causal multi-head attention on 8 Trainium2 NeuronCores.

Sharding: core c -> (batch g = c // 4, head-group p = c % 4, heads 4p..4p+3).
Each core projects Q/K/V for its batch with its 256 feature columns
(column-sharded w_q/w_k/w_v), runs causal attention for its 4 heads, computes
the partial output projection with its 256 rows of w_o, and a ReduceScatter
over each batch group sums the partials.

All matmul operands are bf16 (fp32 PSUM accumulation).  Tricks:
  - K bias is dropped: softmax((q+bq)(k+bk)^T) == softmax((q+bq) k^T) since
    the (q+bq)*bk term is constant along the softmax (k) axis.
  - Q/V/O biases enter PSUM as rank-1 ones-row matmuls on the PE (no vector
    engine bias adds).
  - V is computed in natural [kpos, feature] layout directly (no transpose),
    with an appended ones column per head so PV accumulates the softmax
    denominators for free.
  - Scores/exp/PV touch only the causally valid column range of each k tile;
    only the [128,128] triangular diagonal block needs a mask multiply.
  - Software pipeline: the V projection pass is interleaved with the first
    head-pair's score matmuls (feeds the activation engine early); o-proj
    half 0 is interleaved into attention as PE filler so its ReduceScatter
    fully overlaps the remaining attention work.
"""

import numpy as np

B, S, D, H = 2, 2048, 1024, 16
DK = D // H  # 64
N_CORES = 8
FPC = 256  # features per core

_CACHE = {}


def _build_nc():
    import os as os_mod
    from contextlib import ExitStack

    import concourse.mybir as mybir
    import concourse.tile as tile
    from concourse import bacc

    F32 = mybir.dt.float32
    BF16 = mybir.dt.bfloat16
    Exp = mybir.ActivationFunctionType.Exp
    Copy = mybir.ActivationFunctionType.Copy
    Identity = mybir.ActivationFunctionType.Identity

    nc = bacc.Bacc("TRN2", target_bir_lowering=False, debug=False, num_devices=8)

    xq = nc.dram_tensor("xq", [D, S], BF16, kind="ExternalInput")
    xk = nc.dram_tensor("xk", [D, S], BF16, kind="ExternalInput")
    xv = nc.dram_tensor("xv", [D, S], BF16, kind="ExternalInput")
    wq = nc.dram_tensor("wq", [D, FPC], BF16, kind="ExternalInput")
    wk = nc.dram_tensor("wk", [D, FPC], BF16, kind="ExternalInput")
    wv = nc.dram_tensor("wv", [D, FPC], BF16, kind="ExternalInput")
    wo = nc.dram_tensor("wo", [FPC, D], BF16, kind="ExternalInput")
    bq = nc.dram_tensor("bq", [128, 2], F32, kind="ExternalInput")
    bv = nc.dram_tensor("bv", [128, FPC], BF16, kind="ExternalInput")
    bo4 = nc.dram_tensor("bo4", [128, D], BF16, kind="ExternalInput")
    mtri = nc.dram_tensor("mtri", [128, 2, 128], BF16, kind="ExternalInput")
    out = nc.dram_tensor("out", [512, D], BF16, kind="ExternalOutput")

    debug_taps = bool(os_mod.environ.get("BASS_DEBUG_TAPS"))
    if debug_taps:
        dbg_q = nc.dram_tensor("dbg_q", [128, 2 * S], BF16, kind="ExternalOutput")
        dbg_k = nc.dram_tensor("dbg_k", [128, 2 * S], BF16, kind="ExternalOutput")
        dbg_v = nc.dram_tensor("dbg_v", [128, 16 * 4 * 65], BF16, kind="ExternalOutput")
        dbg_c = nc.dram_tensor("dbg_c", [128, 2 * S], BF16, kind="ExternalOutput")

    with tile.TileContext(nc) as tc:
        with (
            tc.tile_pool(name="consts", bufs=1) as consts,
            tc.tile_pool(name="persist", bufs=1) as persist,
            tc.tile_pool(name="prs", bufs=21) as prs,
            tc.tile_pool(name="normp", bufs=3) as normp,
            tc.tile_pool(name="oout", bufs=4) as oout,
            tc.tile_pool(name="dram", bufs=1, space="DRAM") as dram,
        ):
            # ---- SBUF constants (wq/wk/xin are freed after phase 1a) ----
            ph1_stack = ExitStack()
            ph1 = ph1_stack.enter_context(tc.tile_pool(name="ph1", bufs=1))
            xin = ph1_stack.enter_context(tc.tile_pool(name="xin", bufs=3))
            wq_s = ph1.tile([128, 8, FPC], BF16, tag="wq")
            wk_s = ph1.tile([128, 8, FPC], BF16, tag="wk")
            wv_s = consts.tile([128, 8, FPC], BF16, tag="wv")
            wo_s = consts.tile([128, 2, D], BF16, tag="wo")
            bq_s = consts.tile([128, 2], F32, tag="bq")
            bv_s = consts.tile([128, FPC], BF16, tag="bv")
            bo4_s = consts.tile([128, D], BF16, tag="bo4")
            mask_s = consts.tile([128, 2, 128], BF16, tag="mask")
            ones_s = consts.tile([1, 512], BF16, tag="ones")

            # ---- persistent activations ----
            # feature f of the core maps to (pt = f // 128, row = f % 128);
            # local head h lives at [64*(h%2) : 64*(h%2)+64, h//2, :]
            qT_s = persist.tile([128, 2, S], BF16, tag="qT")
            kT_s = persist.tile([128, 2, S], BF16, tag="kT")
            v_s = persist.tile([128, 16, 4, 65], BF16, tag="v")
            ctx_s = persist.tile([128, 2, S], BF16, tag="ctx")

            nc.vector.memset(ones_s[:], 1.0)
            nc.vector.memset(v_s[:, :, :, 64:65], 1.0)

            # ---- constant DMAs on the gpsimd queue ----
            # only what phase 1a needs up front; the rest is marker-gated so
            # it doesn't preempt the just-in-time xq/xk chunk streams in the
            # shared DMA pool
            gq = nc.gpsimd
            gq.dma_start(wq_s[:, 0:1, :], wq[0:128, :].rearrange(
                "(kc p) f -> p kc f", p=128))
            gq.dma_start(wq_s[:, 1:8, :], wq[128:1024, :].rearrange(
                "(kc p) f -> p kc f", p=128))
            gq.dma_start(bq_s[:], bq.ap())
            gq.dma_start(wk_s[:], wk.ap().rearrange("(kc p) f -> p kc f", p=128))

            def issue_gated_dmas(entries):
                # markers are 1-element DVE copies READING qT_s/kT_s, so they
                # get a real RAW dependency on the projection evacuations —
                # the scheduler cannot hoist these DMAs ahead of the jit
                # xq/xk chunk streams
                for marker, full, src, dep_ap in entries:
                    nc.vector.tensor_copy(marker, dep_ap)
                    gq.dma_start(full, src)

            def issue_v_const_dmas(dep_ap):
                issue_gated_dmas([
                    (wv_s[0:1, 0:1, 0:1], wv_s[:],
                     wv.ap().rearrange("(kc p) f -> p kc f", p=128), dep_ap),
                    (bv_s[0:1, 0:1], bv_s[:], bv.ap(), dep_ap),
                    (mask_s[0:1, 0:1, 0:1], mask_s[:], mtri.ap(), dep_ap),
                ])

            def issue_o_const_dmas():
                issue_gated_dmas([
                    (wo_s[0:1, 0:1, 0:1], wo_s[:],
                     wo.ap().rearrange("(fc p) d -> p fc d", p=128),
                     kT_s[0:1, 1, 1536:1537]),
                    (bo4_s[0:1, 0:1], bo4_s[:], bo4.ap(),
                     kT_s[0:1, 1, 1536:1537]),
                ])

            # xq chunk 0 split in 4 pieces so the first matmuls start early
            xt0 = xin.tile([128, S], BF16, tag="x", name="xt0")
            for qb in range(4):
                nc.sync.dma_start(
                    xt0[:, 512 * qb : 512 * (qb + 1)],
                    xq[0:128, 512 * qb : 512 * (qb + 1)],
                )

            # xv is resident for the natural-layout V pass.  Gate each chunk's
            # DMA on a DVE marker memset sequenced after the Q evacuations so
            # the xv transfers don't steal shared DMA bandwidth from the
            # just-in-time xq/xk chunk loads.
            xvp_stack = ExitStack()
            xvp = xvp_stack.enter_context(tc.tile_pool(name="xvp", bufs=1))
            xv_all = xvp.tile([128, 8, S], BF16, tag="xva")

            def issue_xv_dmas(dep_ap):
                # real RAW dependency on a late xk chunk landing keeps these
                # behind the jit xk stream in the FIFO transfer pool
                for kc in range(8):
                    nc.vector.tensor_copy(xv_all[0:1, kc, 0:1], dep_ap)
                    gq.dma_start(
                        xv_all[:, kc, :], xv[128 * kc : 128 * (kc + 1), :]
                    )

            # ---- phase 1a: Q and K projections (transposed layout) ----
            st1 = ExitStack()
            psP = st1.enter_context(tc.tile_pool(name="psP", bufs=8, space="PSUM"))

            def proj_pass(x_dram, w_tile, b_tile, dst, first):
                ps = {}
                xts = {}
                for kc in range(8):
                    if first and kc == 0:
                        xt = xt0
                    else:
                        xt = xin.tile([128, S], BF16, tag="x")
                        nc.sync.dma_start(xt[:], x_dram[128 * kc : 128 * (kc + 1), :])
                    xts[kc] = xt
                    for pt in range(2):
                        for qb in range(4):
                            if kc == 0:
                                ps[(pt, qb)] = psP.tile(
                                    [128, 512], F32, tag="pp", name=f"pp{pt}{qb}"
                                )
                            nc.tensor.matmul(
                                ps[(pt, qb)][:],
                                w_tile[:, kc, 128 * pt : 128 * (pt + 1)],
                                xt[:, 512 * qb : 512 * (qb + 1)],
                                start=(kc == 0),
                                stop=(kc == 7),
                            )
                for pt in range(2):
                    for qb in range(4):
                        # bias (per-partition in this transposed layout) is
                        # folded into the evacuation, which is split across
                        # DVE and Act so the next pass's PSUM buffers free
                        # up twice as fast
                        dst_ap = dst[:, pt, 512 * qb : 512 * (qb + 1)]
                        if qb % 2:
                            if b_tile is not None:
                                nc.scalar.activation(
                                    dst_ap, ps[(pt, qb)][:], Identity,
                                    bias=b_tile[:, pt : pt + 1],
                                )
                            else:
                                nc.scalar.activation(dst_ap, ps[(pt, qb)][:], Copy)
                        else:
                            if b_tile is not None:
                                nc.vector.tensor_scalar_add(
                                    dst_ap, ps[(pt, qb)][:], b_tile[:, pt : pt + 1]
                                )
                            else:
                                nc.vector.tensor_copy(dst_ap, ps[(pt, qb)][:])
                return xts

            proj_pass(xq, wq_s, bq_s, qT_s, first=True)
            xk_ts = proj_pass(xk, wk_s, None, kT_s, first=False)
            issue_v_const_dmas(xk_ts[5][0:1, 0:1])
            issue_xv_dmas(xk_ts[7][0:1, 0:1])
            issue_o_const_dmas()
            st1.close()

            # ---- attention state/helpers ----
            attn_stack = ExitStack()
            psS = attn_stack.enter_context(
                tc.tile_pool(name="psS", bufs=2, space="PSUM")
            )
            pr_map = {}
            ctx_map = {}

            def segments(s0):
                if s0 < 512:
                    return [(s0, 512), (512, 1024)]
                return [(s0, 1024)]

            def emit_scores(qbp, hp, ki):
                # both si heads of the pair share si-interleaved sc/pr tiles
                # so exp and the mask multiply cover two heads per
                # instruction; one sc tile per 512-column segment keeps the
                # PSUM footprint at 2 banks so bufs=2 still pipelines
                s0 = max(0, 128 * ki - 1024 * qbp)
                pr = prs.tile([128, 2, 1024], BF16, tag="pr", name=f"pr{qbp}{hp}_{ki}")
                for a, b in segments(s0):
                    sc = psS.tile(
                        [128, 2, 512], F32, tag="sc", name=f"sc{qbp}{hp}_{ki}_{a}"
                    )
                    for si in range(2):
                        nc.tensor.matmul(
                            sc[:, si, 0 : b - a],
                            kT_s[64 * si : 64 * si + 64, hp,
                                 128 * ki : 128 * (ki + 1)],
                            qT_s[64 * si : 64 * si + 64, hp,
                                 1024 * qbp + a : 1024 * qbp + b],
                            start=True,
                            stop=True,
                        )
                    nc.scalar.activation(
                        out=pr[:, :, a:b], in_=sc[:, :, 0 : b - a],
                        func=Exp, scale=0.125,
                    )
                if 128 * ki >= 1024 * qbp:  # diagonal tile inside this window
                    nc.vector.tensor_mul(
                        pr[:, :, s0 : s0 + 128], pr[:, :, s0 : s0 + 128], mask_s[:]
                    )
                pr_map[(qbp, hp, ki)] = (pr, s0)

            def emit_pv(qbp, hp, ki, psA):
                nkt = 8 * (qbp + 1)
                pr, s0 = pr_map.pop((qbp, hp, ki))
                last_a = (512 + 1024 * qbp) // 128 - 1
                for si in range(2):
                    key = (qbp, hp, si)
                    if key not in ctx_map:
                        ctx_map[key] = psA.tile(
                            [65, 1024], F32, tag="ctx", name=f"ctx{qbp}{hp}{si}"
                        )
                    ctx = ctx_map[key]
                    for a, b in segments(s0):
                        last = last_a if b == 512 else nkt - 1
                        nc.tensor.matmul(
                            ctx[:, a:b],
                            v_s[:, ki, 2 * hp + si, :],
                            pr[:, si, a:b],
                            start=(ki == 0),
                            stop=(ki == last),
                            skip_group_check=True,
                        )

            def emit_norm(qbp, hp, cols=(0, 1024), release=True):
                a, b = cols
                w = b - a
                ctmp, rc, rbc = {}, {}, {}
                for si in range(2):
                    ctx = ctx_map[(qbp, hp, si)]
                    if release and b == 1024:
                        ctx_map.pop((qbp, hp, si))
                    ctmp[si] = normp.tile([65, 1024], BF16, tag="ctmp", name=f"ctmp{si}")
                    nc.vector.tensor_copy(ctmp[si][:, 0:w], ctx[:, a:b])
                for si in range(2):
                    rc[si] = normp.tile([1, 1024], BF16, tag="rc", name=f"rc{si}")
                    with nc.allow_low_precision("softmax denom recip in bf16"):
                        nc.vector.reciprocal(rc[si][:, 0:w], ctmp[si][64:65, 0:w])
                for si in range(2):
                    rbc[si] = normp.tile([64, 1024], BF16, tag="rbc", name=f"rbc{si}")
                    nc.gpsimd.partition_broadcast(rbc[si][:, 0:w], rc[si][:, 0:w])
                for si in range(2):
                    nc.vector.tensor_mul(
                        ctx_s[64 * si : 64 * si + 64, hp,
                              1024 * qbp + a : 1024 * qbp + b],
                        ctmp[si][0:64, 0:w],
                        rbc[si][:, 0:w],
                    )

            rs_in = [dram.tile([S // 2, D], BF16, name=f"rs_in{i}") for i in range(2)]
            rs_out = [dram.tile([256, D], BF16, name=f"rs_out{i}") for i in range(2)]

            def emit_oproj_sl(h, sl, pool, evac, po_shape):
                st = 8 * h + sl
                po_t = pool.tile(po_shape, F32, tag="sc", name=f"po{h}_{sl}")
                three_d = len(po_shape) == 3
                pe_bias = evac == "act"
                for nb in range(2):
                    po_nb = po_t[:, nb, :] if three_d else po_t[:, 512 * nb : 512 * (nb + 1)]
                    for fc in range(2):
                        nc.tensor.matmul(
                            po_nb,
                            ctx_s[:, fc, 128 * st : 128 * (st + 1)],
                            wo_s[:, fc, 512 * nb : 512 * (nb + 1)],
                            start=(fc == 0),
                            stop=(fc == 1 and not pe_bias),
                        )
                    if pe_bias:
                        nc.tensor.matmul(
                            po_nb,
                            ones_s[0:1, 0:128],
                            bo4_s[0:1, 512 * nb : 512 * (nb + 1)],
                            start=False,
                            stop=True,
                            skip_group_check=True,
                        )
                ot = oout.tile([128, 1024], BF16, tag="ot")
                po_v = po_t[:] if three_d else po_t[:].rearrange("p (n x) -> p n x", n=2)
                ot_v = ot[:].rearrange("p (n x) -> p n x", n=2)
                if evac == "act":
                    nc.scalar.activation(ot_v, po_v, Copy)
                else:
                    # fold the b_o/4 bias into the PSUM evacuation
                    nc.vector.tensor_add(
                        ot_v, po_v, bo4_s[:].rearrange("p (n x) -> p n x", n=2)
                    )
                nc.sync.dma_start(rs_in[h][128 * sl : 128 * (sl + 1), :], ot[:])

            def emit_rs(h):
                if not os_mod.environ.get("BASS_SIM_NO_RS"):
                    import concourse.mybir as mybir_mod

                    nc.gpsimd.collective_compute(
                        "ReduceScatter",
                        mybir_mod.AluOpType.add,
                        replica_groups=[[0, 1, 2, 3], [4, 5, 6, 7]],
                        ins=[rs_in[h].opt()],
                        outs=[rs_out[h].opt()],
                    )
                    nc.sync.dma_start(
                        out[256 * h : 256 * (h + 1), :], rs_out[h][:]
                    )
                else:
                    nc.sync.dma_start(
                        out[256 * h : 256 * (h + 1), :], rs_in[h][0:256, :]
                    )

            # ---- phase 1b: qbp0-hp0 scores (PE/Act filler while the xv ----
            # ---- chunks stream in) + chunk-major natural-layout V pass ----
            for ki in range(8):
                emit_scores(0, 0, ki)
            for ki in range(8):
                emit_scores(0, 1, ki)
            stV = ExitStack()
            psV = stV.enter_context(tc.tile_pool(name="psV", bufs=4, space="PSUM"))
            # 4 single-bank tiles per group (one st each — a PSUM bank may
            # only hold ONE accumulation group at a time); chunk-major order
            # lets group 0 ride the incoming xv chunk DMAs
            for g in range(4):
                pvt = {}
                for kc in range(8):
                    for j in range(4):
                        st = 4 * g + j
                        if kc == 0:
                            pvt[j] = psV.tile(
                                [128, 512], F32, tag="pv", name=f"pv{g}_{j}"
                            )
                        nc.tensor.matmul(
                            pvt[j][:, 0:256],
                            xv_all[:, kc, 128 * st : 128 * (st + 1)],
                            wv_s[:, kc, :],
                            start=(kc == 0),
                            stop=(kc == 7),
                        )
                for j in range(4):
                    st = 4 * g + j
                    nc.vector.tensor_add(
                        v_s[:, st, :, 0:64],
                        pvt[j][:, 0:256].rearrange("p (h x) -> p h x", h=4),
                        bv_s[:].rearrange("p (h x) -> p h x", h=4),
                    )
            stV.close()
            xvp_stack.close()
            ph1_stack.close()

            psA = attn_stack.enter_context(
                tc.tile_pool(name="psA", bufs=2, space="PSUM")
            )

            # ---- stage (qbp0, hp1): hp0 + hp1 PVs (scores pre-emitted) ----
            for ki in range(8):
                emit_pv(0, 0, ki, psA)
            emit_norm(0, 0)

            # ---- stage (qbp1, hp0): scores + qbp0-hp1 PVs, then own  ----
            # ---- PVs + o-proj half0 as PE filler after norm(0,1)     ----
            # (ctx pool has 2 buffer pairs: (1,0)'s PVs may only start
            #  after norm(0,1) releases qbp0-hp1's ctx tiles)
            for ki in range(16):
                emit_scores(1, 0, ki)
                if ki < 8:
                    emit_pv(0, 1, ki, psA)
                if ki == 8:
                    emit_norm(0, 1)
                if ki >= 9:
                    emit_pv(1, 0, ki - 9, psA)
                if ki >= 8:
                    emit_oproj_sl(0, ki - 8, psS, "dve", [128, 2, 512])
            for k in range(7, 16):
                emit_pv(1, 0, k, psA)
            emit_norm(1, 0)
            emit_rs(0)

            # ---- stage (qbp1, hp1): scores + own PVs (1-ki lag); ctx ----
            # ---- cols [0:512) finish at ki=11, so their norm + the   ----
            # ---- first o-proj half1 slices interleave into the tail  ----
            for ki in range(16):
                emit_scores(1, 1, ki)
                if ki >= 3:
                    emit_pv(1, 1, ki - 3, psA)
                if ki == 14:
                    emit_norm(1, 1, cols=(0, 512), release=False)
                if ki >= 15:
                    emit_oproj_sl(1, ki - 15, psS, "dve", [128, 2, 512])
            emit_pv(1, 1, 13, psA)
            emit_pv(1, 1, 14, psA)
            emit_pv(1, 1, 15, psA)
            emit_norm(1, 1, cols=(512, 1024))
            for sl in range(1, 8):
                emit_oproj_sl(1, sl, psS, "act" if sl % 2 else "dve", [128, 2, 512])
            attn_stack.close()

            if debug_taps:
                nc.sync.dma_start(dbg_q.ap(), qT_s[:].rearrange("p a b -> p (a b)"))
                nc.sync.dma_start(dbg_k.ap(), kT_s[:].rearrange("p a b -> p (a b)"))
                nc.sync.dma_start(dbg_v.ap(), v_s[:].rearrange("p a b c -> p (a b c)"))
                nc.sync.dma_start(dbg_c.ap(), ctx_s[:].rearrange("p a b -> p (a b)"))

            # ---- final ReduceScatter ----
            emit_rs(1)

    nc.compile()
    return nc


def _prep_inputs(query, key_, value, w_q, b_q, w_k, b_k, w_v, b_v, w_o, b_o):
    """Build the 8 per-core input maps (host-side sharding / re-layout)."""
    import ml_dtypes

    bf16 = ml_dtypes.bfloat16
    f32 = np.float32

    r = np.arange(128)[:, None, None]
    j = np.arange(128)[None, None, :]
    # allowed iff q >= k on the diagonal tile; doubled for the si-pair layout
    mtri = np.broadcast_to(j >= r, (128, 2, 128)).astype(bf16)

    wqT = np.ascontiguousarray(np.asarray(w_q, f32).T)  # [D_in, D_out]
    wkT = np.ascontiguousarray(np.asarray(w_k, f32).T)
    wvT = np.ascontiguousarray(np.asarray(w_v, f32).T)
    woT = np.ascontiguousarray(np.asarray(w_o, f32).T)

    xT = {}
    for g in range(B):
        xT[("q", g)] = np.ascontiguousarray(np.asarray(query[g], f32).T.astype(bf16))
        xT[("k", g)] = np.ascontiguousarray(np.asarray(key_[g], f32).T.astype(bf16))
        xT[("v", g)] = np.ascontiguousarray(np.asarray(value[g], f32).T.astype(bf16))

    bo4 = np.broadcast_to(
        (np.asarray(b_o, f32) / 4.0).reshape(1, D), (128, D)
    ).astype(bf16)

    in_maps = []
    for c in range(N_CORES):
        g, p = c // 4, c % 4
        fsel = slice(FPC * p, FPC * (p + 1))
        in_maps.append({
            "xq": xT[("q", g)],
            "xk": xT[("k", g)],
            "xv": xT[("v", g)],
            "wq": np.ascontiguousarray(wqT[:, fsel].astype(bf16)),
            "wk": np.ascontiguousarray(wkT[:, fsel].astype(bf16)),
            "wv": np.ascontiguousarray(wvT[:, fsel].astype(bf16)),
            "wo": np.ascontiguousarray(woT[fsel, :].astype(bf16)),
            "bq": np.ascontiguousarray(
                np.asarray(b_q, f32)[fsel].reshape(2, 128).T),
            "bv": np.ascontiguousarray(np.broadcast_to(
                np.asarray(b_v, f32)[fsel], (128, FPC)).astype(bf16)),
            "bo4": bo4,
            "mtri": mtri,
        })
    return in_maps


def run(inputs, trace=False):
    from concourse.bass_utils import run_bass_kernel_spmd

    if "nc" not in _CACHE:
        _CACHE["nc"] = _build_nc()
    nc = _CACHE["nc"]
    in_maps = _prep_inputs(
        inputs["query"], inputs["key_"], inputs["value"],
        inputs["w_q"], inputs["b_q"], inputs["w_k"], inputs["b_k"],
        inputs["w_v"], inputs["b_v"], inputs["w_o"], inputs["b_o"],
    )
    res = run_bass_kernel_spmd(
        nc, in_maps, core_ids=list(range(N_CORES)), trace=trace,
    )
    out = np.empty((B, S, D), np.float32)
    for c in range(N_CORES):
        g, p = c // 4, c % 4
        # RS half i scatters q rows [1024*i + 256*p, 1024*i + 256*(p+1))
        o = np.asarray(res.results[c]["out"]).astype(np.float32)
        out[g, 256 * p : 256 * (p + 1), :] = o[0:256]
        out[g, 1024 + 256 * p : 1024 + 256 * (p + 1), :] = o[256:512]
    return out, res


def kernel(**inputs):
    out, _ = run(inputs, trace=False)
    return out
